# revision 20
# baseline (speedup 1.0000x reference)
"""Causal multi-head attention layer for Trainium2 (Bass/Tile), 8 NeuronCores.

Problem: x[B=2,S=2048,D=1024], H=16 heads, Dh=64.
Sharding: data-parallel over batch (2) x tensor-parallel over head groups (4):
each of the 8 cores handles one batch element and 4 heads, producing a partial
output [S, D]; the host sums the 4 head-group partials per batch (the
"all-reduce after the W_O contraction" done host-side since we return full
output anyway) and adds biases that commute out (b_O and sum_h b_V[h] @ W_O[h],
exact because softmax rows sum to 1).

Device kernel (per core). The SCORE path (QK projections + S=Q.K^T) runs in
fp8e4m3 with MatmulPerfMode.DoubleRow (2 fp8 MACs/PE-cell/cycle = 2x f16
matmul throughput, measured 216ns for K_eff=256,N=512 — same as one f16
K=128 matmul); the VALUE path (V projection, PV, output projection) stays
f16: fp8 quantization error on the score path averages out through softmax
(host-sim rel_absmax ~1.4e-2 vs the 2e-2 gate), but value-path fp8 error
(~3.6% rms) passes straight to the output and would fail.

  - x is fed twice: f16 x^T [128, KT=8, S] (V path) and fp8 x8
    [128, 2, KT2=4, S] with D-pairs packed in the DoubleRow slot dim
    (d = kt2*256 + slot*128 + p).
  - W_Q/W_K are host-packed fp8 at 64x scale (keeps the 0.02-std weights
    out of e4m3 denormals), wqk8 [128, 2, KT2, 2, NPAIR, 128]. A QK
    projection group is 4 DoubleRow matmuls (K_eff=256 each) instead of 8
    f16 ones. PSUM holds 64*q; eviction applies *1/16 (+4*bias) and writes
    fp8 Q8/K8 at 4x scale (sigma~2.6, e4m3-friendly); the *16 in the score
    product is folded into the exp scale (inv_sqrt_dh/16).
  - Scores matmul per j is a DoubleRow PAIR: head A packed [32,2] (e =
    slot*32 + p) at PE rows 0:32, head B at rows 64:96. Rows {0,64} are
    different PE quadrants so the two matmuls run fully concurrent
    (measured): one N-cycle pass for both heads vs ~1.7N for the old f16
    K=64 pair trick. Operand base partitions are restricted to {0,32,64}
    and quadrant concurrency needs {0,64}, so only 2 heads can fly at once.
  - Q8/K8 eviction cannot write the [32,2]-packed layout directly (it folds
    64 PSUM partitions onto 32) — evict full-width [128,SC] fp8 scratch
    (one DVE/ACT op, same cost as the old f16 eviction), then 4 tiny
    SBUF->SBUF DMAs fold it into Q8/K8. DMA queues are idle mid-kernel.
  - V computed in [k, e] layout from f16 x^T (stationary x^T tile, moving
    W_V, all 4 heads at once), stored as V'=[V|1...1] with the ones block
    replicated 64x so the PV matmul broadcasts the softmax denominator l
    across output partitions 64:128.
  - Scores computed TRANSPOSED: S^T[k, q], so softmax's sum rides the PV
    contraction: Z'[e|l, q] = V'.T @ exp(S^T) accumulated over k-tiles. No
    max-subtraction (scores are O(1), exp safe in f32).
  - Causal masking is multiplicative on exp(S^T), diagonal chunks only (on
    GpSimd); fully-masked column ranges are skipped via c0 slicing.
  - Normalization: l arrives pre-broadcast on PV-accumulator partitions
    64:128; wide DVE reciprocal_approx_fast + multiply. (Must stage l
    through SBUF — reciprocal on multi-matmul PSUM directly is garbage.)
  - Schedule (all tuned on HW, see git history of the f16 version):
      * Phase 1 computes only the first two q-chunks' Q/K projections
        (8 PSUM groups fed kt2-by-kt2 as the x8 DMA lands) and the first
        8 V tiles; the rest ride the flash loop as deadline-ordered PE
        fill work (fill_queue).
      * DMA order: bqk, wqk8+x8[chunks 0-1 cols] interleaved, f16 x^T
        ktile-by-ktile with wv at midpoint, x8[chunks 2-3 cols], wo.
      * exp->PV pipeline depth 2 (pends); out-proj METERED (every 3rd j)
        through middle chunks; pr-boundary cover steps; drain-phase
        normalize sliced per q-tile with out-proj interleaved.
      * Out-proj PSUM evicted on DVE during flash, ACT during drain; casts
        to f16 so the out DMA halves (host accumulates partials in f32).
  - CAUTION: instruction timings are extremely sensitive to SBUF tile
    layout (port contention). A/B any pool/tile change against the
    previous layout.
"""

import os
import numpy as np

P = 128
SC = 512  # q-chunk width (one PSUM bank of fp32)

_BUILD_CACHE = {}

WSCALE = 64.0   # host scale on W_Q/W_K before fp8 quantization
QSCALE = 4.0    # scale of Q8/K8 relative to true q,k
# eviction: psum = WSCALE * q  ->  Q8 = psum * (QSCALE/WSCALE) + QSCALE*b
EVSCALE = QSCALE / WSCALE
# score psum = QSCALE^2 * (q.k); fold into exp scale
SSCALE = 1.0 / (QSCALE * QSCALE)


def build_nc(S, Dm, NH, Dh, stage=99):
    """Build (and cache) the per-core Bass module. NH = heads per core."""
    key = (S, Dm, NH, Dh, stage)
    if key in _BUILD_CACHE:
        return _BUILD_CACHE[key]

    import concourse.bacc as bacc
    import concourse.mybir as mybir
    import concourse.tile as tile

    f32 = mybir.dt.float32
    f16 = mybir.dt.float16
    f8 = mybir.dt.float8e4
    DR = mybir.MatmulPerfMode.DoubleRow
    dt_w = f16   # value-path matmul dtype
    dt_m = f16   # mask dtype

    KT = Dm // P       # f16 k-tiles over the model dim
    KT2 = Dm // (2 * P)  # fp8 DoubleRow k-tiles (256 contraction each)
    NPAIR = NH // 2    # head pairs
    QC = S // SC       # q chunks
    NKT = S // P       # k-position tiles
    DH2 = Dm // SC     # output free-dim chunks
    assert Dh == 64 and NH % 2 == 0 and S % SC == 0 and Dm % SC == 0

    nc = bacc.Bacc(
        "TRN2",
        debug=False,
        enable_asserts=False,
        target_bir_lowering=False,
        num_devices=1,
    )

    xT_d = nc.dram_tensor("xT", [P, KT, S], f16, kind="ExternalInput")
    x8_d = nc.dram_tensor("x8", [P, 2, KT2, S], f8, kind="ExternalInput")
    wqk8_d = nc.dram_tensor(
        "wqk8", [P, 2, KT2, 2, NPAIR, P], f8, kind="ExternalInput"
    )
    wv_d = nc.dram_tensor("wv", [P, KT, NH * Dh], f16, kind="ExternalInput")
    wo_d = nc.dram_tensor("wo", [P, NPAIR, Dm], f16, kind="ExternalInput")
    bqk_d = nc.dram_tensor("bqk", [P, 2, NPAIR], f32, kind="ExternalInput")
    # output in f16 (halves the output DMA; host accumulates in f32)
    out_d = nc.dram_tensor("out", [S, Dm], f16, kind="ExternalOutput")

    Exp = mybir.ActivationFunctionType.Exp
    Ident = mybir.ActivationFunctionType.Identity
    exp_scale = float(SSCALE / np.sqrt(Dh))

    with tile.TileContext(nc) as tc:
        with tc.tile_pool(name="const", bufs=1) as cpool:
            wv = cpool.tile([P, KT, NH * Dh], f16)
            wo = cpool.tile([P, NPAIR, Dm], f16)
            bqk = cpool.tile([P, 2, NPAIR], f32)

            # fp8 Q/K in DoubleRow-packed layout: partition p in [0,32) +
            # slot s hold head A's e = s*32+p; partitions 64:96 head B.
            # (32:64 and 96:128 are dead — operand bases are {0,32,64} and
            # quadrant concurrency needs {0,64}.)
            Q8 = cpool.tile([P, 2, NPAIR, S], f8)
            K8 = cpool.tile([P, 2, NPAIR, S], f8)
            Vt = cpool.tile([P, NKT, NH, 2 * Dh], f16)

            # causal masks for the diagonal-chunk variants (S^T layout:
            # partition=k, free=q), built on GpSimd during the DMA wait
            masks = cpool.tile([P, SC // P, SC], dt_m)
            nc.gpsimd.memset(masks[:], 1.0)
            for v in range(SC // P):
                nc.gpsimd.affine_select(
                    out=masks[:, v, :],
                    in_=masks[:, v, :],
                    compare_op=mybir.AluOpType.is_ge,
                    fill=0.0,
                    base=-(v * P),
                    pattern=[[1, SC]],
                    channel_multiplier=-1,
                )

            # ---------- phase 1: projections for the first two q-chunks ----
            with (
                tc.tile_pool(name="p1", bufs=1) as p1pool,
                tc.tile_pool(name="ps1", bufs=8, space="PSUM") as ps1,
            ):
                wqk8 = cpool.tile([P, 2, KT2, 2, NPAIR, P], f8)
                x8 = cpool.tile([P, 2, KT2, S], f8)
                xT = cpool.tile([P, KT, S], f16)
                # DMA order: the fp8 QK stream first (it gates the flash
                # start), then the f16 x for the V path, then the deferred
                # x8 columns (feed the in-flash qk fills), then wo.
                nc.sync.dma_start(bqk[:], bqk_d[:])
                # f16 xT split across BOTH HWDGE queues (sync takes even
                # k-tiles, scalar odd): a single queue sustains well under
                # the per-core HBM bandwidth, and xT's arrival gates the
                # first V tiles (and with them the first PV of the flash).
                for kt in range(1, KT, 2):
                    nc.scalar.dma_start(xT[:, kt, :], xT_d[:, kt, :])
                for kt2 in range(KT2):
                    nc.sync.dma_start(wqk8[:, :, kt2], wqk8_d[:, :, kt2])
                    nc.sync.dma_start(
                        x8[:, :, kt2, 0 : 2 * SC], x8_d[:, :, kt2, 0 : 2 * SC]
                    )
                for kt in range(0, KT, 2):
                    nc.sync.dma_start(xT[:, kt, :], xT_d[:, kt, :])
                    if kt == KT // 2:
                        nc.sync.dma_start(wv[:], wv_d[:])
                for kt2 in range(KT2):
                    nc.sync.dma_start(
                        x8[:, :, kt2, 2 * SC : S], x8_d[:, :, kt2, 2 * SC : S]
                    )
                nc.sync.dma_start(wo[:], wo_d[:])

                # HAM warm-up: dummy matmuls during the initial DMA wait so
                # the PE clock-gate is at 8/8 when real work arrives
                wst = p1pool.tile([P, SC], f32)
                nc.vector.memset(wst[:], 1.0)
                # preload the Exp table on the Scalar engine now (idle)
                tpre = p1pool.tile([1, 2], f32)
                nc.scalar.activation(tpre[:], wst[0:1, 0:2], Exp)
                wrm = p1pool.tile([P, SC], f16)
                nc.vector.tensor_copy(wrm[:], wst[:])
                nwu = 10
                pwu = ps1.tile([P, SC], f32, tag="mm")
                for i in range(nwu):
                    nc.tensor.matmul(
                        pwu[:], wrm[:, 0:P], wrm[:],
                        start=(i == 0), stop=(i == nwu - 1),
                    )

                def fold_qk(u8, pj, pr, qc):
                    """4 SBUF->SBUF DMAs: unpacked fp8 [128,SC] eviction ->
                    DoubleRow-packed Q8/K8 slices. On the scalar HWDGE
                    queue: the sync queue is serialized behind the whole
                    multi-MB input stream (in-order per queue), which would
                    delay these folds — and the first flash scores — to
                    ~36us. The scalar queue is empty, and SBUF->SBUF steals
                    no HBM bandwidth."""
                    dst = Q8 if pj == 0 else K8
                    qs = slice(qc * SC, (qc + 1) * SC)
                    for base in (0, 64):
                        for s in (0, 1):
                            src = u8[base + 32 * s : base + 32 * s + 32, :]
                            nc.scalar.dma_start(
                                dst[base : base + 32, s, pr, qs], src
                            )

                # Q/K projections (first two q-chunks): 8 PSUM groups fed
                # kt2-by-kt2 as the x8 DMA lands
                for qg in range(0, min(2, QC), 2):
                    qcs = list(range(qg, min(qg + 2, QC)))
                    pss = {
                        (pr, pj, qc): ps1.tile(
                            [P, SC], f32, tag="mm", name=f"psqk_{pr}_{pj}_{qc}"
                        )
                        for pr in range(NPAIR)
                        for pj in range(2)
                        for qc in qcs
                    }
                    for kt2 in range(KT2):
                        st, sp = kt2 == 0, kt2 == KT2 - 1
                        for pr in range(NPAIR):
                            for pj in range(2):
                                for qc in qcs:
                                    xs = x8[:, :, kt2, qc * SC : (qc + 1) * SC]
                                    nc.tensor.matmul(
                                        pss[(pr, pj, qc)][:],
                                        wqk8[:, :, kt2, pj, pr, :], xs,
                                        start=st, stop=sp, perf_mode=DR,
                                    )
                    for qc in qcs:  # qc-major: chunk 0's folds land first
                        for pr in range(NPAIR):
                            for pj in range(2):
                                # evict via ACT (idle in phase 1; Identity
                                # shares the Exp table) to full-width fp8
                                # scratch, then DMA-fold into Q8/K8
                                u8 = p1pool.tile(
                                    [P, SC], f8, name=f"u8_{pr}_{pj}_{qc}"
                                )
                                nc.scalar.activation(
                                    u8[:], pss[(pr, pj, qc)][:], Ident,
                                    bias=bqk[:, pj, pr : pr + 1],
                                    scale=EVSCALE,
                                )
                                fold_qk(u8, pj, pr, qc)

                # V' ones block (broadcasts l onto PV partitions 64:128)
                cstage = p1pool.tile([P, 1, 1, Dh], f32)
                nc.vector.memset(cstage[:], 1.0)
                nc.vector.tensor_copy(
                    Vt[:, :, :, Dh : 2 * Dh],
                    cstage[:].to_broadcast((P, NKT, NH, Dh)),
                )

            # ---------- phases 2+3 ----------
            with tc.tile_pool(name="zt", bufs=1) as ztpool:
                ZTt = ztpool.tile([P, NPAIR, S], f16)
                self_flash(
                    nc, tc, stage, Exp, exp_scale, mybir,
                    Q8, K8, Vt, ZTt, wo, out_d, masks, xT, x8, wv, wqk8, bqk,
                    S, Dm, Dh, NPAIR, QC, SC, P, DH2, KT, KT2, NKT,
                    f16, dt_m, f32, f8, DR,
                )

    nc.compile()
    _BUILD_CACHE[key] = nc
    return nc


def self_flash(
    nc, tc, stage, Exp, exp_scale, mybir,
    Q8, K8, Vt, ZTt, wo, out_d, masks, xT, x8, wv, wqk8, bqk,
    S, Dm, Dh, NPAIR, QC, SC, P, DH2, KT, KT2, NKT,
    dt_w, dt_m, f32, f8, DR,
):
    NH = Vt.shape[2]
    # ---------- phases 2+3: flash attention (scores transposed, fp8
    # DoubleRow) with the output projection interleaved one q-chunk behind
    out_dt = dt_w
    mult, add = mybir.AluOpType.mult, mybir.AluOpType.add
    with (
        tc.tile_pool(name="e", bufs=4) as epool,
        tc.tile_pool(name="r", bufs=4) as rpool,
        tc.tile_pool(name="o", bufs=4) as opool,
        tc.tile_pool(name="pss", bufs=2, space="PSUM") as ps_s,
        tc.tile_pool(name="psz", bufs=4, space="PSUM") as psz,
    ):
        if stage <= 1:
            nc.sync.dma_start(out_d[0:P, :], ZTt[:, 0, 0:Dm])

        drain = [False]  # final-drain mode: outproj evictions move DVE->ACT

        def normalize(pr, qc, zA, zB):
            """ZT[:, q] = Z'[0:64, q] * (1 / l[q]); l arrives pre-broadcast
            on partitions 64:128 of the PV accumulators. DVE-only."""
            qs = slice(qc * SC, (qc + 1) * SC)
            rb = rpool.tile([64, 2, SC], f32, tag="rb")
            ls = rpool.tile([64, 2, SC], f32, tag="ls")
            nc.vector.tensor_copy(ls[:, 0, :], zA[Dh : 2 * Dh, :])
            nc.vector.tensor_copy(ls[:, 1, :], zB[Dh : 2 * Dh, :])
            nc.vector.reciprocal_approx_fast(rb[:], ls[:])
            nc.vector.tensor_mul(ZTt[0:64, pr, qs], zA[0:Dh, :], rb[:, 0, :])
            nc.vector.tensor_mul(ZTt[64:128, pr, qs], zB[0:Dh, :], rb[:, 1, :])

        def outproj_steps(qc):
            """Closures for this q-chunk's output projection, injected one at
            a time between later j-iterations to keep PE density high."""
            def step(t, dh2):
                def emit():
                    po = psz.tile([P, SC], f32, tag="z")
                    ds = slice(dh2 * SC, (dh2 + 1) * SC)
                    zs = slice(t * P, (t + 1) * P)
                    for pr in range(NPAIR):
                        nc.tensor.matmul(
                            po[:], ZTt[:, pr, zs], wo[:, pr, ds],
                            start=(pr == 0), stop=(pr == NPAIR - 1),
                        )
                    ot = opool.tile([P, SC], out_dt, tag="o")
                    # evict via DVE during flash, ACT during the final drain
                    if drain[0]:
                        nc.scalar.activation(
                            ot[:], po[:], mybir.ActivationFunctionType.Copy
                        )
                    else:
                        nc.vector.tensor_copy(ot[:], po[:])
                    nc.sync.dma_start(out_d[t * P : (t + 1) * P, ds], ot[:])
                return emit

            return [
                step(t, dh2)
                for t in range(qc * (SC // P), (qc + 1) * (SC // P))
                for dh2 in range(DH2)
            ]

        def v_step(qt):
            """One deferred V-projection group (f16): PE fill work."""
            def emit():
                psV = psz.tile([P, NH * Dh], f32, tag="z", name=f"psv_{qt}")
                for kt in range(KT):
                    nc.tensor.matmul(
                        psV[:],
                        xT[:, kt, qt * P : (qt + 1) * P],
                        wv[:, kt, :],
                        start=(kt == 0), stop=(kt == KT - 1),
                    )
                nc.vector.tensor_copy(
                    Vt[:, qt, :, 0:Dh],
                    psV[:].rearrange("p (h e) -> p h e", e=Dh),
                )
            return emit

        def qk_step(qc, pr, pj):
            """One deferred Q/K-projection group: 4 fp8 DoubleRow matmuls,
            DVE eviction to fp8 scratch, DMA-fold into Q8/K8."""
            def emit():
                ps = psz.tile([P, SC], f32, tag="z", name=f"psqk{qc}_{pr}_{pj}")
                qs = slice(qc * SC, (qc + 1) * SC)
                for kt2 in range(KT2):
                    nc.tensor.matmul(
                        ps[:], wqk8[:, :, kt2, pj, pr, :],
                        x8[:, :, kt2, qs],
                        start=(kt2 == 0), stop=(kt2 == KT2 - 1), perf_mode=DR,
                    )
                u8 = rpool.tile([P, SC], f8, tag="u8")
                nc.vector.tensor_scalar(
                    u8[:], ps[:], EVSCALE, bqk[:, pj, pr : pr + 1], mult, add
                )
                dst = Q8 if pj == 0 else K8
                for base in (0, 64):
                    for s in (0, 1):
                        nc.scalar.dma_start(
                            dst[base : base + 32, s, pr, qs],
                            u8[base + 32 * s : base + 32 * s + 32, :],
                        )
            return emit

        # Fill work split by PE mode: fp8-DR qk fills pop adjacent to the
        # (DR) scores; f16 v fills and outproj pops ride after the (f16)
        # PVs. Mixing modes exposes a ~107ns stationary load per switch.
        # V projections for chunks 0/1 ride the fill machinery too (phase 1
        # no longer computes them — they'd stall the PE on the f16 x DMA
        # behind the much shorter fp8 QK projections).
        fill_dr = []
        fill_16 = []
        for qc2 in range(0, QC):
            if qc2 >= 2:
                for pr2 in range(NPAIR):
                    for pj2 in range(2):
                        fill_dr.append((qc2, qk_step(qc2, pr2, pj2)))
            for qt in range(qc2 * (SC // P), (qc2 + 1) * (SC // P)):
                fill_16.append((qc2, v_step(qt)))
        op_queue = []
        chunk_tail = None
        for qc in range(QC if stage >= 2 else 0):
            # deadline safety net (skipped for chunk 0: its v fills are
            # xT-DMA-gated and would stall the PE before the first scores;
            # the jj==0/jj==2 in-loop pops cover chunk 0's needs)
            while qc >= 1 and fill_dr and fill_dr[0][0] <= qc:
                fill_dr.pop(0)[1]()
            while qc >= 1 and fill_16 and fill_16[0][0] <= qc:
                fill_16.pop(0)[1]()
            for pr in range(NPAIR):
                hA, hB = 2 * pr, 2 * pr + 1
                zA = psz.tile([P, SC], f32, tag="z")
                zB = psz.tile([P, SC], f32, tag="z")
                jmax = (qc + 1) * (SC // P)
                pends = []  # exp->PV pipeline, depth 2

                def emit_pv(j, eAB, c0, jmax=jmax, zA=zA, zB=zB, hA=hA, hB=hB):
                    st, sp = j == 0, j == jmax - 1
                    cs = slice(c0, SC)
                    nc.tensor.matmul(
                        zA[:, cs], Vt[:, j, hA, :], eAB[:, 0, cs],
                        start=st, stop=sp,
                    )
                    nc.tensor.matmul(
                        zB[:, cs], Vt[:, j, hB, :], eAB[:, 1, cs],
                        start=st, stop=sp,
                    )

                # j-batching x2: the PE pays an exposed ~107ns stationary
                # load at every fp8-DR <-> f16 mode switch (the weight
                # preload doesn't pipeline across modes). Emitting two j's
                # of DR scores, then two j's of f16 PV + f16 fill work,
                # halves the switch count.
                for jj in range(0, jmax, 2):
                    batch = []
                    for j in (jj, jj + 1):
                        v = j - (jmax - SC // P)
                        c0 = v * P if v > 0 else 0
                        cs = slice(c0, SC)
                        qf = slice(qc * SC + c0, (qc + 1) * SC)
                        sAB = ps_s.tile([P, 2, SC], f32, tag="s")
                        ks = slice(j * P, (j + 1) * P)
                        # fp8 DoubleRow score pair: head A rows 0:32, head B
                        # rows 64:96 (different PE quadrants -> concurrent)
                        nc.tensor.matmul(
                            sAB[:, 0, cs],
                            K8[0:32, :, pr, ks], Q8[0:32, :, pr, qf],
                            start=True, stop=True, perf_mode=DR,
                        )
                        nc.tensor.matmul(
                            sAB[:, 1, cs],
                            K8[64:96, :, pr, ks], Q8[64:96, :, pr, qf],
                            start=True, stop=True, perf_mode=DR,
                        )
                        batch.append((j, sAB, c0, v))
                    # DR-mode fill rides directly after the DR scores; gated
                    # to the chunk before its deadline so it can't stall the
                    # PE on the (late-arriving) deferred x8 columns
                    if jj >= 4 and fill_dr and fill_dr[0][0] <= qc + 1:
                        fill_dr.pop(0)[1]()
                    for j, sAB, c0, v in batch:
                        cs = slice(c0, SC)
                        eAB = epool.tile([P, 2, SC], dt_w, tag="e")
                        nc.scalar.activation(
                            eAB[:, :, cs], sAB[:, :, cs], Exp, scale=exp_scale
                        )
                        if v >= 0:  # chunk contains the causal diagonal
                            mv = slice(c0, min((v + 1) * P, SC))
                            nc.gpsimd.tensor_mul(
                                eAB[:, 0, mv], eAB[:, 0, mv], masks[:, v, mv]
                            )
                            nc.gpsimd.tensor_mul(
                                eAB[:, 1, mv], eAB[:, 1, mv], masks[:, v, mv]
                            )
                        if stage >= 3:
                            pends.append((j, eAB, c0))
                        else:
                            last_e = eAB
                    if stage >= 3:
                        while len(pends) > 3:
                            emit_pv(*pends.pop(0))
                        if jj == 0:
                            if chunk_tail is not None:
                                chunk_tail()
                                chunk_tail = None
                            # two pops: chunk 0 must emit v0..v3 before
                            # pr=0's chunk_tail (which reads Vt[3]) — jj==0
                            # and jj==2 each contribute two
                            for _ in range(2):
                                if fill_16:
                                    fill_16.pop(0)[1]()
                        elif jj >= 2:
                            for s in range(2):
                                if fill_16:
                                    fill_16.pop(0)[1]()
                                elif op_queue and (
                                    qc == QC - 1 or (jj + s) % 3 == 0
                                ):
                                    op_queue.pop(0)()
                if stage < 3:
                    if pr == 0 and qc == 0:
                        nc.sync.dma_start(out_d[0:P, 0:SC], last_e[:, 0, :])
                    continue

                # pr-boundary cover
                if op_queue and qc >= 1:
                    op_queue.pop(0)()

                def chunk_tail(pends=pends, pr=pr, qc=qc, zA=zA, zB=zB,
                               emit_pv=emit_pv):
                    for p in pends:
                        emit_pv(*p)
                    normalize(pr, qc, zA, zB)
                    return pr, qc, zA, zB

            if stage >= 5:
                op_queue.extend(outproj_steps(qc))
        drain[0] = True
        if chunk_tail is not None:
            pends, pr, qc, zA, zB = (chunk_tail.__defaults__[:5])
            for p in pends:
                chunk_tail.__defaults__[5](*p)
            rb = rpool.tile([64, 2, SC], f32, tag="rb")
            ls = rpool.tile([64, 2, SC], f32, tag="ls")
            for ti in range(SC // P):
                cl = slice(ti * P, (ti + 1) * P)
                qsl = slice(qc * SC + ti * P, qc * SC + (ti + 1) * P)
                nc.vector.tensor_copy(ls[:, 0, cl], zA[Dh : 2 * Dh, cl])
                nc.vector.tensor_copy(ls[:, 1, cl], zB[Dh : 2 * Dh, cl])
                nc.vector.reciprocal_approx_fast(rb[:, :, cl], ls[:, :, cl])
                nc.vector.tensor_mul(ZTt[0:64, pr, qsl], zA[0:Dh, cl], rb[:, 0, cl])
                nc.vector.tensor_mul(ZTt[64:128, pr, qsl], zB[0:Dh, cl], rb[:, 1, cl])
                for _ in range(DH2):
                    if op_queue:
                        op_queue.pop(0)()
        for step in op_queue:
            step()
        if stage == 4:
            nc.sync.dma_start(out_d[0:P, :], ZTt[:, 0, 0:Dm])


def pack_inputs(x_b, W_Q, W_K, W_V, W_O, b_Q, b_K, hds):
    """Host-side packing of one core's shard into the kernel's layouts."""
    import ml_dtypes

    f8 = ml_dtypes.float8_e4m3
    Dm, Dh = W_Q.shape[1], W_Q.shape[2]
    S = x_b.shape[0]
    NH = len(hds)
    NPAIR = NH // 2
    KT = Dm // P
    KT2 = Dm // (2 * P)

    xb = np.asarray(x_b, np.float32)
    xT = np.ascontiguousarray(
        xb.T.reshape(KT, P, S).transpose(1, 0, 2)
    ).astype(np.float16)
    # fp8 x for the QK path: d = kt2*256 + slot*128 + p
    x8 = np.ascontiguousarray(
        xb.T.reshape(KT2, 2, P, S).transpose(2, 1, 0, 3)
    ).astype(f8)

    def pack_w8(W):  # [H, Dm, Dh] -> [P, 2, KT2, NPAIR, 128] fp8 at 64x
        W4 = np.asarray(W, np.float32)[hds]  # [NH, Dm, Dh]
        t = W4.reshape(NPAIR, 2, KT2, 2, P, Dh).transpose(4, 3, 2, 0, 1, 5)
        return (WSCALE * t.reshape(P, 2, KT2, NPAIR, 2 * Dh))

    wqk8 = np.ascontiguousarray(
        np.stack([pack_w8(W_Q), pack_w8(W_K)], axis=3)  # [P,2,KT2,2,NPAIR,128]
    ).astype(f8)

    WV4 = np.asarray(W_V, np.float32)[hds]
    wv = np.ascontiguousarray(
        WV4.reshape(NH, KT, P, Dh).transpose(2, 1, 0, 3).reshape(P, KT, NH * Dh)
    ).astype(np.float16)

    WO4 = np.asarray(W_O, np.float32)[hds]
    wo = np.ascontiguousarray(
        WO4.reshape(NPAIR, 2, Dh, Dm).transpose(1, 2, 0, 3).reshape(P, NPAIR, Dm)
    ).astype(np.float16)

    def pack_b(b):  # [H, Dh] -> [P, NPAIR], pre-scaled by QSCALE
        b4 = np.asarray(b, np.float32)[hds]
        return QSCALE * b4.reshape(NPAIR, 2, Dh).transpose(1, 2, 0).reshape(P, NPAIR)

    bqk = np.ascontiguousarray(
        np.stack([pack_b(b_Q), pack_b(b_K)], axis=1)  # [P, 2, NPAIR]
    ).astype(np.float32)

    return {"xT": xT, "x8": x8, "wqk8": wqk8, "wv": wv, "wo": wo, "bqk": bqk}


def kernel(x, W_Q, W_K, W_V, W_O, b_Q, b_K, b_V, b_O, _trace=False):
    from concourse.bass_utils import run_bass_kernel_spmd

    x = np.asarray(x, np.float32)
    B, S, Dm = x.shape
    H, _, Dh = W_Q.shape
    NCORES = 8
    GB = NCORES // B        # head groups per batch element
    NH = H // GB            # heads per core

    nc = build_nc(S, Dm, NH, Dh)

    in_maps = []
    for c in range(NCORES):
        b, g = c // GB, c % GB
        hds = list(range(g * NH, (g + 1) * NH))
        in_maps.append(
            pack_inputs(x[b], W_Q, W_K, W_V, W_O, b_Q, b_K, hds)
        )

    try:
        res = run_bass_kernel_spmd(
            nc, in_maps, core_ids=list(range(NCORES)), trace=_trace
        )
    except Exception:
        # transient device hiccups usually clear on retry
        res = run_bass_kernel_spmd(
            nc, in_maps, core_ids=list(range(NCORES)), trace=_trace
        )

    out = np.zeros((B, S, Dm), np.float32)
    for c in range(NCORES):
        out[c // GB] += res.results[c]["out"]

    # biases that commute out of the device kernel (softmax rows sum to 1)
    corr = np.asarray(b_O, np.float32) + np.einsum(
        "he,hed->d",
        np.asarray(b_V, np.float32),
        np.asarray(W_O, np.float32),
    )
    out += corr[None, None, :]

    if _trace:
        kernel.last_results = res
    return out


# revision 21
# speedup vs baseline: 1.0219x; 1.0219x over previous
"""Causal multi-head attention layer for Trainium2 (Bass/Tile), 8 NeuronCores.

Problem: x[B=2,S=2048,D=1024], H=16 heads, Dh=64.
Sharding: data-parallel over batch (2) x tensor-parallel over head groups (4):
each of the 8 cores handles one batch element and 4 heads, producing a partial
output [S, D]; the host sums the 4 head-group partials per batch (the
"all-reduce after the W_O contraction" done host-side since we return full
output anyway) and adds biases that commute out (b_O and sum_h b_V[h] @ W_O[h],
exact because softmax rows sum to 1).

Device kernel (per core). The SCORE path (QK projections + S=Q.K^T) runs in
fp8e4m3 with MatmulPerfMode.DoubleRow (2 fp8 MACs/PE-cell/cycle = 2x f16
matmul throughput, measured 216ns for K_eff=256,N=512 — same as one f16
K=128 matmul); the VALUE path (V projection, PV, output projection) stays
f16: fp8 quantization error on the score path averages out through softmax
(host-sim rel_absmax ~1.4e-2 vs the 2e-2 gate), but value-path fp8 error
(~3.6% rms) passes straight to the output and would fail.

  - x is fed twice: f16 x^T [128, KT=8, S] (V path) and fp8 x8
    [128, 2, KT2=4, S] with D-pairs packed in the DoubleRow slot dim
    (d = kt2*256 + slot*128 + p).
  - W_Q/W_K are host-packed fp8 at 64x scale (keeps the 0.02-std weights
    out of e4m3 denormals), wqk8 [128, 2, KT2, 2, NPAIR, 128]. A QK
    projection group is 4 DoubleRow matmuls (K_eff=256 each) instead of 8
    f16 ones. PSUM holds 64*q; eviction applies *1/16 (+4*bias) and writes
    fp8 Q8/K8 at 4x scale (sigma~2.6, e4m3-friendly); the *16 in the score
    product is folded into the exp scale (inv_sqrt_dh/16).
  - Scores matmul per j is a DoubleRow PAIR: head A packed [32,2] (e =
    slot*32 + p) at PE rows 0:32, head B at rows 64:96. Rows {0,64} are
    different PE quadrants so the two matmuls run fully concurrent
    (measured): one N-cycle pass for both heads vs ~1.7N for the old f16
    K=64 pair trick. Operand base partitions are restricted to {0,32,64}
    and quadrant concurrency needs {0,64}, so only 2 heads can fly at once.
  - Q8/K8 eviction cannot write the [32,2]-packed layout directly (it folds
    64 PSUM partitions onto 32) — evict full-width [128,SC] fp8 scratch
    (one DVE/ACT op, same cost as the old f16 eviction), then 4 tiny
    SBUF->SBUF DMAs fold it into Q8/K8. DMA queues are idle mid-kernel.
  - V computed in [k, e] layout from f16 x^T (stationary x^T tile, moving
    W_V, all 4 heads at once), stored as V'=[V|1...1] with the ones block
    replicated 64x so the PV matmul broadcasts the softmax denominator l
    across output partitions 64:128.
  - Scores computed TRANSPOSED: S^T[k, q], so softmax's sum rides the PV
    contraction: Z'[e|l, q] = V'.T @ exp(S^T) accumulated over k-tiles. No
    max-subtraction (scores are O(1), exp safe in f32).
  - Causal masking is multiplicative on exp(S^T), diagonal chunks only (on
    GpSimd); fully-masked column ranges are skipped via c0 slicing.
  - Normalization: l arrives pre-broadcast on PV-accumulator partitions
    64:128; wide DVE reciprocal_approx_fast + multiply. (Must stage l
    through SBUF — reciprocal on multi-matmul PSUM directly is garbage.)
  - Schedule (all tuned on HW, see git history of the f16 version):
      * Phase 1 computes only the first two q-chunks' Q/K projections
        (8 PSUM groups fed kt2-by-kt2 as the x8 DMA lands) and the first
        8 V tiles; the rest ride the flash loop as deadline-ordered PE
        fill work (fill_queue).
      * DMA order: bqk, wqk8+x8[chunks 0-1 cols] interleaved, f16 x^T
        ktile-by-ktile with wv at midpoint, x8[chunks 2-3 cols], wo.
      * exp->PV pipeline depth 2 (pends); out-proj METERED (every 3rd j)
        through middle chunks; pr-boundary cover steps; drain-phase
        normalize sliced per q-tile with out-proj interleaved.
      * Out-proj PSUM evicted on DVE during flash, ACT during drain; casts
        to f16 so the out DMA halves (host accumulates partials in f32).
  - CAUTION: instruction timings are extremely sensitive to SBUF tile
    layout (port contention). A/B any pool/tile change against the
    previous layout.
"""

import os
import numpy as np

P = 128
SC = 512  # q-chunk width (one PSUM bank of fp32)

_BUILD_CACHE = {}

WSCALE = 64.0   # host scale on W_Q/W_K before fp8 quantization
QSCALE = 4.0    # scale of Q8/K8 relative to true q,k
# eviction: psum = WSCALE * q  ->  Q8 = psum * (QSCALE/WSCALE) + QSCALE*b
EVSCALE = QSCALE / WSCALE
# score psum = QSCALE^2 * (q.k); fold into exp scale
SSCALE = 1.0 / (QSCALE * QSCALE)


def build_nc(S, Dm, NH, Dh, stage=99):
    """Build (and cache) the per-core Bass module. NH = heads per core."""
    key = (S, Dm, NH, Dh, stage)
    if key in _BUILD_CACHE:
        return _BUILD_CACHE[key]

    import concourse.bacc as bacc
    import concourse.mybir as mybir
    import concourse.tile as tile

    f32 = mybir.dt.float32
    f16 = mybir.dt.float16
    f8 = mybir.dt.float8e4
    DR = mybir.MatmulPerfMode.DoubleRow
    dt_w = f16   # value-path matmul dtype
    dt_m = f16   # mask dtype

    KT = Dm // P       # f16 k-tiles over the model dim
    KT2 = Dm // (2 * P)  # fp8 DoubleRow k-tiles (256 contraction each)
    NPAIR = NH // 2    # head pairs
    QC = S // SC       # q chunks
    NKT = S // P       # k-position tiles
    DH2 = Dm // SC     # output free-dim chunks
    assert Dh == 64 and NH % 2 == 0 and S % SC == 0 and Dm % SC == 0

    nc = bacc.Bacc(
        "TRN2",
        debug=False,
        enable_asserts=False,
        target_bir_lowering=False,
        num_devices=1,
    )

    xT_d = nc.dram_tensor("xT", [P, KT, S], f16, kind="ExternalInput")
    x8_d = nc.dram_tensor("x8", [P, 2, KT2, S], f8, kind="ExternalInput")
    wqk8_d = nc.dram_tensor(
        "wqk8", [P, 2, KT2, 2, NPAIR, P], f8, kind="ExternalInput"
    )
    wv_d = nc.dram_tensor("wv", [P, KT, NH * Dh], f16, kind="ExternalInput")
    wo_d = nc.dram_tensor("wo", [P, NPAIR, Dm], f16, kind="ExternalInput")
    bqk_d = nc.dram_tensor("bqk", [P, 2, NPAIR], f32, kind="ExternalInput")
    # output in f16 (halves the output DMA; host accumulates in f32)
    out_d = nc.dram_tensor("out", [S, Dm], f16, kind="ExternalOutput")

    Exp = mybir.ActivationFunctionType.Exp
    Ident = mybir.ActivationFunctionType.Identity
    exp_scale = float(SSCALE / np.sqrt(Dh))

    with tile.TileContext(nc) as tc:
        with tc.tile_pool(name="const", bufs=1) as cpool:
            wv = cpool.tile([P, KT, NH * Dh], f16)
            wo = cpool.tile([P, NPAIR, Dm], f16)
            bqk = cpool.tile([P, 2, NPAIR], f32)

            # fp8 Q/K in DoubleRow-packed layout: partition p in [0,32) +
            # slot s hold head A's e = s*32+p; partitions 64:96 head B.
            # (32:64 and 96:128 are dead — operand bases are {0,32,64} and
            # quadrant concurrency needs {0,64}.)
            Q8 = cpool.tile([P, 2, NPAIR, S], f8)
            K8 = cpool.tile([P, 2, NPAIR, S], f8)
            Vt = cpool.tile([P, NKT, NH, 2 * Dh], f16)

            # causal masks for the diagonal-chunk variants (S^T layout:
            # partition=k, free=q), built on GpSimd during the DMA wait
            masks = cpool.tile([P, SC // P, SC], dt_m)
            nc.gpsimd.memset(masks[:], 1.0)
            for v in range(SC // P):
                nc.gpsimd.affine_select(
                    out=masks[:, v, :],
                    in_=masks[:, v, :],
                    compare_op=mybir.AluOpType.is_ge,
                    fill=0.0,
                    base=-(v * P),
                    pattern=[[1, SC]],
                    channel_multiplier=-1,
                )

            # ---------- phase 1: projections for the first two q-chunks ----
            with (
                tc.tile_pool(name="p1", bufs=1) as p1pool,
                tc.tile_pool(name="ps1", bufs=8, space="PSUM") as ps1,
            ):
                wqk8 = cpool.tile([P, 2, KT2, 2, NPAIR, P], f8)
                x8 = cpool.tile([P, 2, KT2, S], f8)
                xT = cpool.tile([P, KT, S], f16)
                # DMA order: the fp8 QK stream first (it gates the flash
                # start), then the f16 x for the V path, then the deferred
                # x8 columns (feed the in-flash qk fills), then wo.
                nc.sync.dma_start(bqk[:], bqk_d[:])
                for kt2 in range(KT2):
                    nc.sync.dma_start(wqk8[:, :, kt2], wqk8_d[:, :, kt2])
                    nc.sync.dma_start(
                        x8[:, :, kt2, 0 : 2 * SC], x8_d[:, :, kt2, 0 : 2 * SC]
                    )
                for kt in range(KT):
                    nc.sync.dma_start(xT[:, kt, :], xT_d[:, kt, :])
                    if kt == KT // 2 - 1:
                        nc.sync.dma_start(wv[:], wv_d[:])
                for kt2 in range(KT2):
                    nc.sync.dma_start(
                        x8[:, :, kt2, 2 * SC : S], x8_d[:, :, kt2, 2 * SC : S]
                    )
                nc.sync.dma_start(wo[:], wo_d[:])

                # HAM warm-up: dummy matmuls during the initial DMA wait so
                # the PE clock-gate is at 8/8 when real work arrives
                wst = p1pool.tile([P, SC], f32)
                nc.vector.memset(wst[:], 1.0)
                # preload the Exp table on the Scalar engine now (idle)
                tpre = p1pool.tile([1, 2], f32)
                nc.scalar.activation(tpre[:], wst[0:1, 0:2], Exp)
                wrm = p1pool.tile([P, SC], f16)
                nc.vector.tensor_copy(wrm[:], wst[:])
                nwu = 10
                pwu = ps1.tile([P, SC], f32, tag="mm")
                for i in range(nwu):
                    nc.tensor.matmul(
                        pwu[:], wrm[:, 0:P], wrm[:],
                        start=(i == 0), stop=(i == nwu - 1),
                    )

                def fold_qk(u8, pj, pr, qc):
                    """4 SBUF->SBUF DMAs: unpacked fp8 [128,SC] eviction ->
                    DoubleRow-packed Q8/K8 slices. On the scalar HWDGE
                    queue: the sync queue is serialized behind the whole
                    multi-MB input stream (in-order per queue), which would
                    delay these folds — and the first flash scores — to
                    ~36us. The scalar queue is empty, and SBUF->SBUF steals
                    no HBM bandwidth."""
                    dst = Q8 if pj == 0 else K8
                    qs = slice(qc * SC, (qc + 1) * SC)
                    for base in (0, 64):
                        for s in (0, 1):
                            src = u8[base + 32 * s : base + 32 * s + 32, :]
                            nc.scalar.dma_start(
                                dst[base : base + 32, s, pr, qs], src
                            )

                # Q/K projections (first two q-chunks): 8 PSUM groups fed
                # kt2-by-kt2 as the x8 DMA lands
                for qg in range(0, min(2, QC), 2):
                    qcs = list(range(qg, min(qg + 2, QC)))
                    pss = {
                        (pr, pj, qc): ps1.tile(
                            [P, SC], f32, tag="mm", name=f"psqk_{pr}_{pj}_{qc}"
                        )
                        for pr in range(NPAIR)
                        for pj in range(2)
                        for qc in qcs
                    }
                    for kt2 in range(KT2):
                        st, sp = kt2 == 0, kt2 == KT2 - 1
                        for pr in range(NPAIR):
                            for pj in range(2):
                                for qc in qcs:
                                    xs = x8[:, :, kt2, qc * SC : (qc + 1) * SC]
                                    nc.tensor.matmul(
                                        pss[(pr, pj, qc)][:],
                                        wqk8[:, :, kt2, pj, pr, :], xs,
                                        start=st, stop=sp, perf_mode=DR,
                                    )
                    for qc in qcs:  # qc-major: chunk 0's folds land first
                        for pr in range(NPAIR):
                            for pj in range(2):
                                # evict via ACT (idle in phase 1; Identity
                                # shares the Exp table) to full-width fp8
                                # scratch, then DMA-fold into Q8/K8
                                u8 = p1pool.tile(
                                    [P, SC], f8, name=f"u8_{pr}_{pj}_{qc}"
                                )
                                nc.scalar.activation(
                                    u8[:], pss[(pr, pj, qc)][:], Ident,
                                    bias=bqk[:, pj, pr : pr + 1],
                                    scale=EVSCALE,
                                )
                                fold_qk(u8, pj, pr, qc)

                # V' ones block (broadcasts l onto PV partitions 64:128)
                cstage = p1pool.tile([P, 1, 1, Dh], f32)
                nc.vector.memset(cstage[:], 1.0)
                nc.vector.tensor_copy(
                    Vt[:, :, :, Dh : 2 * Dh],
                    cstage[:].to_broadcast((P, NKT, NH, Dh)),
                )

            # ---------- phases 2+3 ----------
            with tc.tile_pool(name="zt", bufs=1) as ztpool:
                ZTt = ztpool.tile([P, NPAIR, S], f16)
                self_flash(
                    nc, tc, stage, Exp, exp_scale, mybir,
                    Q8, K8, Vt, ZTt, wo, out_d, masks, xT, x8, wv, wqk8, bqk,
                    S, Dm, Dh, NPAIR, QC, SC, P, DH2, KT, KT2, NKT,
                    f16, dt_m, f32, f8, DR,
                )

    nc.compile()
    _BUILD_CACHE[key] = nc
    return nc


def self_flash(
    nc, tc, stage, Exp, exp_scale, mybir,
    Q8, K8, Vt, ZTt, wo, out_d, masks, xT, x8, wv, wqk8, bqk,
    S, Dm, Dh, NPAIR, QC, SC, P, DH2, KT, KT2, NKT,
    dt_w, dt_m, f32, f8, DR,
):
    NH = Vt.shape[2]
    # ---------- phases 2+3: flash attention (scores transposed, fp8
    # DoubleRow) with the output projection interleaved one q-chunk behind
    out_dt = dt_w
    mult, add = mybir.AluOpType.mult, mybir.AluOpType.add
    with (
        tc.tile_pool(name="e", bufs=4) as epool,
        tc.tile_pool(name="r", bufs=4) as rpool,
        tc.tile_pool(name="o", bufs=4) as opool,
        tc.tile_pool(name="pss", bufs=2, space="PSUM") as ps_s,
        tc.tile_pool(name="psz", bufs=4, space="PSUM") as psz,
    ):
        if stage <= 1:
            nc.sync.dma_start(out_d[0:P, :], ZTt[:, 0, 0:Dm])

        drain = [False]  # final-drain mode: outproj evictions move DVE->ACT

        def normalize(pr, qc, zA, zB):
            """ZT[:, q] = Z'[0:64, q] * (1 / l[q]); l arrives pre-broadcast
            on partitions 64:128 of the PV accumulators. DVE-only."""
            qs = slice(qc * SC, (qc + 1) * SC)
            rb = rpool.tile([64, 2, SC], f32, tag="rb")
            ls = rpool.tile([64, 2, SC], f32, tag="ls")
            nc.vector.tensor_copy(ls[:, 0, :], zA[Dh : 2 * Dh, :])
            nc.vector.tensor_copy(ls[:, 1, :], zB[Dh : 2 * Dh, :])
            nc.vector.reciprocal_approx_fast(rb[:], ls[:])
            nc.vector.tensor_mul(ZTt[0:64, pr, qs], zA[0:Dh, :], rb[:, 0, :])
            nc.vector.tensor_mul(ZTt[64:128, pr, qs], zB[0:Dh, :], rb[:, 1, :])

        def outproj_steps(qc):
            """Closures for this q-chunk's output projection, injected one at
            a time between later j-iterations to keep PE density high."""
            def step(t, dh2):
                def emit():
                    po = psz.tile([P, SC], f32, tag="z")
                    ds = slice(dh2 * SC, (dh2 + 1) * SC)
                    zs = slice(t * P, (t + 1) * P)
                    for pr in range(NPAIR):
                        nc.tensor.matmul(
                            po[:], ZTt[:, pr, zs], wo[:, pr, ds],
                            start=(pr == 0), stop=(pr == NPAIR - 1),
                        )
                    ot = opool.tile([P, SC], out_dt, tag="o")
                    # evict via DVE during flash, ACT during the final drain
                    if drain[0]:
                        nc.scalar.activation(
                            ot[:], po[:], mybir.ActivationFunctionType.Copy
                        )
                    else:
                        nc.vector.tensor_copy(ot[:], po[:])
                    nc.sync.dma_start(out_d[t * P : (t + 1) * P, ds], ot[:])
                return emit

            return [
                step(t, dh2)
                for t in range(qc * (SC // P), (qc + 1) * (SC // P))
                for dh2 in range(DH2)
            ]

        def v_step(qt):
            """One deferred V-projection group (f16): PE fill work."""
            def emit():
                psV = psz.tile([P, NH * Dh], f32, tag="z", name=f"psv_{qt}")
                for kt in range(KT):
                    nc.tensor.matmul(
                        psV[:],
                        xT[:, kt, qt * P : (qt + 1) * P],
                        wv[:, kt, :],
                        start=(kt == 0), stop=(kt == KT - 1),
                    )
                nc.vector.tensor_copy(
                    Vt[:, qt, :, 0:Dh],
                    psV[:].rearrange("p (h e) -> p h e", e=Dh),
                )
            return emit

        def qk_step(qc, pr, pj):
            """One deferred Q/K-projection group: 4 fp8 DoubleRow matmuls,
            DVE eviction to fp8 scratch, DMA-fold into Q8/K8."""
            def emit():
                ps = psz.tile([P, SC], f32, tag="z", name=f"psqk{qc}_{pr}_{pj}")
                qs = slice(qc * SC, (qc + 1) * SC)
                for kt2 in range(KT2):
                    nc.tensor.matmul(
                        ps[:], wqk8[:, :, kt2, pj, pr, :],
                        x8[:, :, kt2, qs],
                        start=(kt2 == 0), stop=(kt2 == KT2 - 1), perf_mode=DR,
                    )
                u8 = rpool.tile([P, SC], f8, tag="u8")
                nc.vector.tensor_scalar(
                    u8[:], ps[:], EVSCALE, bqk[:, pj, pr : pr + 1], mult, add
                )
                dst = Q8 if pj == 0 else K8
                for base in (0, 64):
                    for s in (0, 1):
                        nc.scalar.dma_start(
                            dst[base : base + 32, s, pr, qs],
                            u8[base + 32 * s : base + 32 * s + 32, :],
                        )
            return emit

        # Fill work split by PE mode: fp8-DR qk fills pop adjacent to the
        # (DR) scores; f16 v fills and outproj pops ride after the (f16)
        # PVs. Mixing modes exposes a ~107ns stationary load per switch.
        # V projections for chunks 0/1 ride the fill machinery too (phase 1
        # no longer computes them — they'd stall the PE on the f16 x DMA
        # behind the much shorter fp8 QK projections).
        fill_dr = []
        fill_16 = []
        for qc2 in range(0, QC):
            if qc2 >= 2:
                for pr2 in range(NPAIR):
                    for pj2 in range(2):
                        fill_dr.append((qc2, qk_step(qc2, pr2, pj2)))
            for qt in range(qc2 * (SC // P), (qc2 + 1) * (SC // P)):
                fill_16.append((qc2, v_step(qt)))
        op_queue = []
        chunk_tail = None
        for qc in range(QC if stage >= 2 else 0):
            # deadline safety net (skipped for chunk 0: its v fills are
            # xT-DMA-gated and would stall the PE before the first scores;
            # the jj==0/jj==2 in-loop pops cover chunk 0's needs)
            while qc >= 1 and fill_dr and fill_dr[0][0] <= qc:
                fill_dr.pop(0)[1]()
            while qc >= 1 and fill_16 and fill_16[0][0] <= qc:
                fill_16.pop(0)[1]()
            for pr in range(NPAIR):
                hA, hB = 2 * pr, 2 * pr + 1
                zA = psz.tile([P, SC], f32, tag="z")
                zB = psz.tile([P, SC], f32, tag="z")
                jmax = (qc + 1) * (SC // P)
                pends = []  # exp->PV pipeline, depth 2

                def emit_pv(j, eAB, c0, jmax=jmax, zA=zA, zB=zB, hA=hA, hB=hB):
                    st, sp = j == 0, j == jmax - 1
                    cs = slice(c0, SC)
                    nc.tensor.matmul(
                        zA[:, cs], Vt[:, j, hA, :], eAB[:, 0, cs],
                        start=st, stop=sp,
                    )
                    nc.tensor.matmul(
                        zB[:, cs], Vt[:, j, hB, :], eAB[:, 1, cs],
                        start=st, stop=sp,
                    )

                # j-batching x2: the PE pays an exposed ~107ns stationary
                # load at every fp8-DR <-> f16 mode switch (the weight
                # preload doesn't pipeline across modes). Emitting two j's
                # of DR scores, then two j's of f16 PV + f16 fill work,
                # halves the switch count.
                for jj in range(0, jmax, 2):
                    batch = []
                    for j in (jj, jj + 1):
                        v = j - (jmax - SC // P)
                        c0 = v * P if v > 0 else 0
                        cs = slice(c0, SC)
                        qf = slice(qc * SC + c0, (qc + 1) * SC)
                        sAB = ps_s.tile([P, 2, SC], f32, tag="s")
                        ks = slice(j * P, (j + 1) * P)
                        # fp8 DoubleRow score pair: head A rows 0:32, head B
                        # rows 64:96 (different PE quadrants -> concurrent)
                        nc.tensor.matmul(
                            sAB[:, 0, cs],
                            K8[0:32, :, pr, ks], Q8[0:32, :, pr, qf],
                            start=True, stop=True, perf_mode=DR,
                        )
                        nc.tensor.matmul(
                            sAB[:, 1, cs],
                            K8[64:96, :, pr, ks], Q8[64:96, :, pr, qf],
                            start=True, stop=True, perf_mode=DR,
                        )
                        batch.append((j, sAB, c0, v))
                    # DR-mode fill rides directly after the DR scores; gated
                    # to the chunk before its deadline so it can't stall the
                    # PE on the (late-arriving) deferred x8 columns
                    if jj >= 4 and fill_dr and fill_dr[0][0] <= qc + 1:
                        fill_dr.pop(0)[1]()
                    for j, sAB, c0, v in batch:
                        cs = slice(c0, SC)
                        eAB = epool.tile([P, 2, SC], dt_w, tag="e")
                        nc.scalar.activation(
                            eAB[:, :, cs], sAB[:, :, cs], Exp, scale=exp_scale
                        )
                        if v >= 0:  # chunk contains the causal diagonal
                            mv = slice(c0, min((v + 1) * P, SC))
                            nc.gpsimd.tensor_mul(
                                eAB[:, 0, mv], eAB[:, 0, mv], masks[:, v, mv]
                            )
                            nc.gpsimd.tensor_mul(
                                eAB[:, 1, mv], eAB[:, 1, mv], masks[:, v, mv]
                            )
                        if stage >= 3:
                            pends.append((j, eAB, c0))
                        else:
                            last_e = eAB
                    if stage >= 3:
                        while len(pends) > 3:
                            emit_pv(*pends.pop(0))
                        if jj == 0:
                            if chunk_tail is not None:
                                chunk_tail()
                                chunk_tail = None
                            # two pops: chunk 0 must emit v0..v3 before
                            # pr=0's chunk_tail (which reads Vt[3]) — jj==0
                            # and jj==2 each contribute two
                            for _ in range(2):
                                if fill_16:
                                    fill_16.pop(0)[1]()
                        elif jj >= 2:
                            for s in range(2):
                                if fill_16:
                                    fill_16.pop(0)[1]()
                                elif op_queue and (
                                    qc == QC - 1 or (jj + s) % 3 == 0
                                ):
                                    op_queue.pop(0)()
                if stage < 3:
                    if pr == 0 and qc == 0:
                        nc.sync.dma_start(out_d[0:P, 0:SC], last_e[:, 0, :])
                    continue

                # pr-boundary cover
                if op_queue and qc >= 1:
                    op_queue.pop(0)()

                def chunk_tail(pends=pends, pr=pr, qc=qc, zA=zA, zB=zB,
                               emit_pv=emit_pv):
                    for p in pends:
                        emit_pv(*p)
                    normalize(pr, qc, zA, zB)
                    return pr, qc, zA, zB

            if stage >= 5:
                op_queue.extend(outproj_steps(qc))
        drain[0] = True
        if chunk_tail is not None:
            pends, pr, qc, zA, zB = (chunk_tail.__defaults__[:5])
            for p in pends:
                chunk_tail.__defaults__[5](*p)
            rb = rpool.tile([64, 2, SC], f32, tag="rb")
            ls = rpool.tile([64, 2, SC], f32, tag="ls")
            for ti in range(SC // P):
                cl = slice(ti * P, (ti + 1) * P)
                qsl = slice(qc * SC + ti * P, qc * SC + (ti + 1) * P)
                nc.vector.tensor_copy(ls[:, 0, cl], zA[Dh : 2 * Dh, cl])
                nc.vector.tensor_copy(ls[:, 1, cl], zB[Dh : 2 * Dh, cl])
                nc.vector.reciprocal_approx_fast(rb[:, :, cl], ls[:, :, cl])
                nc.vector.tensor_mul(ZTt[0:64, pr, qsl], zA[0:Dh, cl], rb[:, 0, cl])
                nc.vector.tensor_mul(ZTt[64:128, pr, qsl], zB[0:Dh, cl], rb[:, 1, cl])
                for _ in range(DH2):
                    if op_queue:
                        op_queue.pop(0)()
        for step in op_queue:
            step()
        if stage == 4:
            nc.sync.dma_start(out_d[0:P, :], ZTt[:, 0, 0:Dm])


def pack_inputs(x_b, W_Q, W_K, W_V, W_O, b_Q, b_K, hds):
    """Host-side packing of one core's shard into the kernel's layouts."""
    import ml_dtypes

    f8 = ml_dtypes.float8_e4m3
    Dm, Dh = W_Q.shape[1], W_Q.shape[2]
    S = x_b.shape[0]
    NH = len(hds)
    NPAIR = NH // 2
    KT = Dm // P
    KT2 = Dm // (2 * P)

    xb = np.asarray(x_b, np.float32)
    xT = np.ascontiguousarray(
        xb.T.reshape(KT, P, S).transpose(1, 0, 2)
    ).astype(np.float16)
    # fp8 x for the QK path: d = kt2*256 + slot*128 + p
    x8 = np.ascontiguousarray(
        xb.T.reshape(KT2, 2, P, S).transpose(2, 1, 0, 3)
    ).astype(f8)

    def pack_w8(W):  # [H, Dm, Dh] -> [P, 2, KT2, NPAIR, 128] fp8 at 64x
        W4 = np.asarray(W, np.float32)[hds]  # [NH, Dm, Dh]
        t = W4.reshape(NPAIR, 2, KT2, 2, P, Dh).transpose(4, 3, 2, 0, 1, 5)
        return (WSCALE * t.reshape(P, 2, KT2, NPAIR, 2 * Dh))

    wqk8 = np.ascontiguousarray(
        np.stack([pack_w8(W_Q), pack_w8(W_K)], axis=3)  # [P,2,KT2,2,NPAIR,128]
    ).astype(f8)

    WV4 = np.asarray(W_V, np.float32)[hds]
    wv = np.ascontiguousarray(
        WV4.reshape(NH, KT, P, Dh).transpose(2, 1, 0, 3).reshape(P, KT, NH * Dh)
    ).astype(np.float16)

    WO4 = np.asarray(W_O, np.float32)[hds]
    wo = np.ascontiguousarray(
        WO4.reshape(NPAIR, 2, Dh, Dm).transpose(1, 2, 0, 3).reshape(P, NPAIR, Dm)
    ).astype(np.float16)

    def pack_b(b):  # [H, Dh] -> [P, NPAIR], pre-scaled by QSCALE
        b4 = np.asarray(b, np.float32)[hds]
        return QSCALE * b4.reshape(NPAIR, 2, Dh).transpose(1, 2, 0).reshape(P, NPAIR)

    bqk = np.ascontiguousarray(
        np.stack([pack_b(b_Q), pack_b(b_K)], axis=1)  # [P, 2, NPAIR]
    ).astype(np.float32)

    return {"xT": xT, "x8": x8, "wqk8": wqk8, "wv": wv, "wo": wo, "bqk": bqk}


def kernel(x, W_Q, W_K, W_V, W_O, b_Q, b_K, b_V, b_O, _trace=False):
    from concourse.bass_utils import run_bass_kernel_spmd

    x = np.asarray(x, np.float32)
    B, S, Dm = x.shape
    H, _, Dh = W_Q.shape
    NCORES = 8
    GB = NCORES // B        # head groups per batch element
    NH = H // GB            # heads per core

    nc = build_nc(S, Dm, NH, Dh)

    in_maps = []
    for c in range(NCORES):
        b, g = c // GB, c % GB
        hds = list(range(g * NH, (g + 1) * NH))
        in_maps.append(
            pack_inputs(x[b], W_Q, W_K, W_V, W_O, b_Q, b_K, hds)
        )

    try:
        res = run_bass_kernel_spmd(
            nc, in_maps, core_ids=list(range(NCORES)), trace=_trace
        )
    except Exception:
        # transient device hiccups usually clear on retry
        res = run_bass_kernel_spmd(
            nc, in_maps, core_ids=list(range(NCORES)), trace=_trace
        )

    out = np.zeros((B, S, Dm), np.float32)
    for c in range(NCORES):
        out[c // GB] += res.results[c]["out"]

    # biases that commute out of the device kernel (softmax rows sum to 1)
    corr = np.asarray(b_O, np.float32) + np.einsum(
        "he,hed->d",
        np.asarray(b_V, np.float32),
        np.asarray(W_O, np.float32),
    )
    out += corr[None, None, :]

    if _trace:
        kernel.last_results = res
    return out


# revision 25
# speedup vs baseline: 1.0322x; 1.0101x over previous
"""Causal multi-head attention layer for Trainium2 (Bass/Tile), 8 NeuronCores.

Problem: x[B=2,S=2048,D=1024], H=16 heads, Dh=64.
Sharding: data-parallel over batch (2) x tensor-parallel over head groups (4):
each of the 8 cores handles one batch element and 4 heads, producing a partial
output [S, D]; the host sums the 4 head-group partials per batch (the
"all-reduce after the W_O contraction" done host-side since we return full
output anyway) and adds biases that commute out (b_O and sum_h b_V[h] @ W_O[h],
exact because softmax rows sum to 1).

Device kernel (per core). The SCORE path (QK projections + S=Q.K^T) runs in
fp8e4m3 with MatmulPerfMode.DoubleRow (2 fp8 MACs/PE-cell/cycle = 2x f16
matmul throughput, measured 216ns for K_eff=256,N=512 — same as one f16
K=128 matmul); the VALUE path (V projection, PV, output projection) stays
f16: fp8 quantization error on the score path averages out through softmax
(host-sim rel_absmax ~1.4e-2 vs the 2e-2 gate), but value-path fp8 error
(~3.6% rms) passes straight to the output and would fail.

  - x is fed twice: f16 x^T [128, KT=8, S] (V path) and fp8 x8
    [128, 2, KT2=4, S] with D-pairs packed in the DoubleRow slot dim
    (d = kt2*256 + slot*128 + p).
  - W_Q/W_K are host-packed fp8 at 64x scale (keeps the 0.02-std weights
    out of e4m3 denormals), wqk8 [128, 2, KT2, 2, NPAIR, 128]. A QK
    projection group is 4 DoubleRow matmuls (K_eff=256 each) instead of 8
    f16 ones. PSUM holds 64*q; eviction applies *1/16 (+4*bias) and writes
    fp8 Q8/K8 at 4x scale (sigma~2.6, e4m3-friendly); the *16 in the score
    product is folded into the exp scale (inv_sqrt_dh/16).
  - Scores matmul per j is a DoubleRow PAIR: head A packed [32,2] (e =
    slot*32 + p) at PE rows 0:32, head B at rows 64:96. Rows {0,64} are
    different PE quadrants so the two matmuls run fully concurrent
    (measured): one N-cycle pass for both heads vs ~1.7N for the old f16
    K=64 pair trick. Operand base partitions are restricted to {0,32,64}
    and quadrant concurrency needs {0,64}, so only 2 heads can fly at once.
  - Q8/K8 eviction cannot write the [32,2]-packed layout directly (it folds
    64 PSUM partitions onto 32) — evict full-width [128,SC] fp8 scratch
    (one DVE/ACT op, same cost as the old f16 eviction), then 4 tiny
    SBUF->SBUF DMAs fold it into Q8/K8. DMA queues are idle mid-kernel.
  - V computed in [k, e] layout from f16 x^T (stationary x^T tile, moving
    W_V, all 4 heads at once), stored as V'=[V|1...1] with the ones block
    replicated 64x so the PV matmul broadcasts the softmax denominator l
    across output partitions 64:128.
  - Scores computed TRANSPOSED: S^T[k, q], so softmax's sum rides the PV
    contraction: Z'[e|l, q] = V'.T @ exp(S^T) accumulated over k-tiles. No
    max-subtraction (scores are O(1), exp safe in f32).
  - Causal masking is multiplicative on exp(S^T), diagonal chunks only (on
    GpSimd); fully-masked column ranges are skipped via c0 slicing.
  - Normalization: l arrives pre-broadcast on PV-accumulator partitions
    64:128; wide DVE reciprocal_approx_fast + multiply. (Must stage l
    through SBUF — reciprocal on multi-matmul PSUM directly is garbage.)
  - Schedule (all tuned on HW, see git history of the f16 version):
      * Phase 1 computes only the first two q-chunks' Q/K projections
        (8 PSUM groups fed kt2-by-kt2 as the x8 DMA lands) and the first
        8 V tiles; the rest ride the flash loop as deadline-ordered PE
        fill work (fill_queue).
      * DMA order: bqk, wqk8+x8[chunks 0-1 cols] interleaved, f16 x^T
        ktile-by-ktile with wv at midpoint, x8[chunks 2-3 cols], wo.
      * exp->PV pipeline depth 2 (pends); out-proj METERED (every 3rd j)
        through middle chunks; pr-boundary cover steps; drain-phase
        normalize sliced per q-tile with out-proj interleaved.
      * Out-proj PSUM evicted on DVE during flash, ACT during drain; casts
        to f16 so the out DMA halves (host accumulates partials in f32).
  - CAUTION: instruction timings are extremely sensitive to SBUF tile
    layout (port contention). A/B any pool/tile change against the
    previous layout.
"""

import os
import numpy as np

P = 128
SC = 512  # q-chunk width (one PSUM bank of fp32)

_BUILD_CACHE = {}

WSCALE = 64.0   # host scale on W_Q/W_K before fp8 quantization
QSCALE = 4.0    # scale of Q8/K8 relative to true q,k
# eviction: psum = WSCALE * q  ->  Q8 = psum * (QSCALE/WSCALE) + QSCALE*b
EVSCALE = QSCALE / WSCALE
# score psum = QSCALE^2 * (q.k); fold into exp scale
SSCALE = 1.0 / (QSCALE * QSCALE)


def build_nc(S, Dm, NH, Dh, stage=99):
    """Build (and cache) the per-core Bass module. NH = heads per core."""
    key = (S, Dm, NH, Dh, stage)
    if key in _BUILD_CACHE:
        return _BUILD_CACHE[key]

    import concourse.bacc as bacc
    import concourse.mybir as mybir
    import concourse.tile as tile

    f32 = mybir.dt.float32
    f16 = mybir.dt.float16
    f8 = mybir.dt.float8e4
    DR = mybir.MatmulPerfMode.DoubleRow
    dt_w = f16   # value-path matmul dtype
    dt_m = f16   # mask dtype

    KT = Dm // P       # f16 k-tiles over the model dim
    KT2 = Dm // (2 * P)  # fp8 DoubleRow k-tiles (256 contraction each)
    NPAIR = NH // 2    # head pairs
    QC = S // SC       # q chunks
    NKT = S // P       # k-position tiles
    DH2 = Dm // SC     # output free-dim chunks
    assert Dh == 64 and NH % 2 == 0 and S % SC == 0 and Dm % SC == 0

    nc = bacc.Bacc(
        "TRN2",
        debug=False,
        enable_asserts=False,
        target_bir_lowering=False,
        num_devices=1,
    )

    xT_d = nc.dram_tensor("xT", [P, KT, S], f16, kind="ExternalInput")
    x8_d = nc.dram_tensor("x8", [P, 2, KT2, S], f8, kind="ExternalInput")
    wqk8_d = nc.dram_tensor(
        "wqk8", [P, 2, KT2, 2, NPAIR, P], f8, kind="ExternalInput"
    )
    wv_d = nc.dram_tensor("wv", [P, KT, NH * Dh], f16, kind="ExternalInput")
    wo_d = nc.dram_tensor("wo", [P, NPAIR, Dm], f16, kind="ExternalInput")
    bqk_d = nc.dram_tensor("bqk", [P, 2, NPAIR], f32, kind="ExternalInput")
    # output in f16 (halves the output DMA; host accumulates in f32)
    out_d = nc.dram_tensor("out", [S, Dm], f16, kind="ExternalOutput")

    Exp = mybir.ActivationFunctionType.Exp
    Ident = mybir.ActivationFunctionType.Identity
    exp_scale = float(SSCALE / np.sqrt(Dh))

    with tile.TileContext(nc) as tc:
        with tc.tile_pool(name="const", bufs=1) as cpool:
            wv = cpool.tile([P, KT, NH * Dh], f16)
            wo = cpool.tile([P, NPAIR, Dm], f16)
            bqk = cpool.tile([P, 2, NPAIR], f32)

            # fp8 Q/K in DoubleRow-packed layout: partition p in [0,32) +
            # slot s hold head A's e = s*32+p; partitions 64:96 head B.
            # (32:64 and 96:128 are dead — operand bases are {0,32,64} and
            # quadrant concurrency needs {0,64}.)
            Q8 = cpool.tile([P, 2, NPAIR, S], f8)
            K8 = cpool.tile([P, 2, NPAIR, S], f8)
            Vt = cpool.tile([P, NKT, NH, 2 * Dh], f16)

            # causal masks for the diagonal-chunk variants (S^T layout:
            # partition=k, free=q), built on GpSimd during the DMA wait
            masks = cpool.tile([P, SC // P, SC], dt_m)
            nc.gpsimd.memset(masks[:], 1.0)
            for v in range(SC // P):
                nc.gpsimd.affine_select(
                    out=masks[:, v, :],
                    in_=masks[:, v, :],
                    compare_op=mybir.AluOpType.is_ge,
                    fill=0.0,
                    base=-(v * P),
                    pattern=[[1, SC]],
                    channel_multiplier=-1,
                )

            # ---------- phase 1: projections for the first two q-chunks ----
            with (
                tc.tile_pool(name="p1", bufs=1) as p1pool,
                tc.tile_pool(name="ps1", bufs=8, space="PSUM") as ps1,
            ):
                wqk8 = cpool.tile([P, 2, KT2, 2, NPAIR, P], f8)
                x8 = cpool.tile([P, 2, KT2, S], f8)
                xT = cpool.tile([P, KT, S], f16)
                # DMA order: the fp8 QK stream first (it gates the flash
                # start), then the f16 x for the V path, then the deferred
                # x8 columns (feed the in-flash qk fills), then wo.
                nc.sync.dma_start(bqk[:], bqk_d[:])
                for kt2 in range(KT2):
                    nc.sync.dma_start(wqk8[:, :, kt2], wqk8_d[:, :, kt2])
                    nc.sync.dma_start(
                        x8[:, :, kt2, 0 : 2 * SC], x8_d[:, :, kt2, 0 : 2 * SC]
                    )
                for kt in range(KT):
                    nc.sync.dma_start(xT[:, kt, :], xT_d[:, kt, :])
                    if kt == KT // 2 - 1:
                        nc.sync.dma_start(wv[:], wv_d[:])
                for kt2 in range(KT2):
                    nc.sync.dma_start(
                        x8[:, :, kt2, 2 * SC : S], x8_d[:, :, kt2, 2 * SC : S]
                    )
                nc.sync.dma_start(wo[:], wo_d[:])

                # HAM warm-up: dummy matmuls during the initial DMA wait so
                # the PE clock-gate is at 8/8 when real work arrives
                wst = p1pool.tile([P, SC], f32)
                nc.vector.memset(wst[:], 1.0)
                # preload the Exp table on the Scalar engine now (idle)
                tpre = p1pool.tile([1, 2], f32)
                nc.scalar.activation(tpre[:], wst[0:1, 0:2], Exp)
                wrm = p1pool.tile([P, SC], f16)
                nc.vector.tensor_copy(wrm[:], wst[:])
                nwu = 10
                pwu = ps1.tile([P, SC], f32, tag="mm")
                for i in range(nwu):
                    nc.tensor.matmul(
                        pwu[:], wrm[:, 0:P], wrm[:],
                        start=(i == 0), stop=(i == nwu - 1),
                    )

                def fold_qk(u8, pj, pr, qc):
                    """4 SBUF->SBUF DMAs: unpacked fp8 [128,SC] eviction ->
                    DoubleRow-packed Q8/K8 slices. On the scalar HWDGE
                    queue: the sync queue is serialized behind the whole
                    multi-MB input stream (in-order per queue), which would
                    delay these folds — and the first flash scores — to
                    ~36us. The scalar queue is empty, and SBUF->SBUF steals
                    no HBM bandwidth."""
                    dst = Q8 if pj == 0 else K8
                    qs = slice(qc * SC, (qc + 1) * SC)
                    for base in (0, 64):
                        for s in (0, 1):
                            src = u8[base + 32 * s : base + 32 * s + 32, :]
                            nc.scalar.dma_start(
                                dst[base : base + 32, s, pr, qs], src
                            )

                # Q/K projections (first two q-chunks): 8 PSUM groups fed
                # kt2-by-kt2 as the x8 DMA lands
                for qg in range(0, min(2, QC), 2):
                    qcs = list(range(qg, min(qg + 2, QC)))
                    pss = {
                        (pr, pj, qc): ps1.tile(
                            [P, SC], f32, tag="mm", name=f"psqk_{pr}_{pj}_{qc}"
                        )
                        for pr in range(NPAIR)
                        for pj in range(2)
                        for qc in qcs
                    }
                    for kt2 in range(KT2):
                        st, sp = kt2 == 0, kt2 == KT2 - 1
                        for pr in range(NPAIR):
                            for pj in range(2):
                                for qc in qcs:
                                    xs = x8[:, :, kt2, qc * SC : (qc + 1) * SC]
                                    nc.tensor.matmul(
                                        pss[(pr, pj, qc)][:],
                                        wqk8[:, :, kt2, pj, pr, :], xs,
                                        start=st, stop=sp, perf_mode=DR,
                                    )
                    for qc in qcs:  # qc-major: chunk 0's folds land first
                        for pr in range(NPAIR):
                            for pj in range(2):
                                # evict via ACT (idle in phase 1; Identity
                                # shares the Exp table) to full-width fp8
                                # scratch, then DMA-fold into Q8/K8
                                u8 = p1pool.tile(
                                    [P, SC], f8, name=f"u8_{pr}_{pj}_{qc}"
                                )
                                nc.scalar.activation(
                                    u8[:], pss[(pr, pj, qc)][:], Ident,
                                    bias=bqk[:, pj, pr : pr + 1],
                                    scale=EVSCALE,
                                )
                                fold_qk(u8, pj, pr, qc)

                # V tiles the first two flash chunks touch; rest deferred
                for qt in range(min(2 * (SC // P), NKT)):
                    psV = ps1.tile([P, NH * Dh], f32, tag="mm")
                    for kt in range(KT):
                        nc.tensor.matmul(
                            psV[:],
                            xT[:, kt, qt * P : (qt + 1) * P],
                            wv[:, kt, :],
                            start=(kt == 0), stop=(kt == KT - 1),
                        )
                    nc.vector.tensor_copy(
                        Vt[:, qt, :, 0:Dh],
                        psV[:].rearrange("p (h e) -> p h e", e=Dh),
                    )

                # V' ones block (broadcasts l onto PV partitions 64:128)
                cstage = p1pool.tile([P, 1, 1, Dh], f32)
                nc.vector.memset(cstage[:], 1.0)
                nc.vector.tensor_copy(
                    Vt[:, :, :, Dh : 2 * Dh],
                    cstage[:].to_broadcast((P, NKT, NH, Dh)),
                )

            # ---------- phases 2+3 ----------
            with tc.tile_pool(name="zt", bufs=1) as ztpool:
                ZTt = ztpool.tile([P, NPAIR, S], f16)
                self_flash(
                    nc, tc, stage, Exp, exp_scale, mybir,
                    Q8, K8, Vt, ZTt, wo, out_d, masks, xT, x8, wv, wqk8, bqk,
                    S, Dm, Dh, NPAIR, QC, SC, P, DH2, KT, KT2, NKT,
                    f16, dt_m, f32, f8, DR,
                )

    nc.compile()
    _BUILD_CACHE[key] = nc
    return nc


def self_flash(
    nc, tc, stage, Exp, exp_scale, mybir,
    Q8, K8, Vt, ZTt, wo, out_d, masks, xT, x8, wv, wqk8, bqk,
    S, Dm, Dh, NPAIR, QC, SC, P, DH2, KT, KT2, NKT,
    dt_w, dt_m, f32, f8, DR,
):
    NH = Vt.shape[2]
    # ---------- phases 2+3: flash attention (scores transposed, fp8
    # DoubleRow) with the output projection interleaved one q-chunk behind
    out_dt = dt_w
    mult, add = mybir.AluOpType.mult, mybir.AluOpType.add
    with (
        tc.tile_pool(name="e", bufs=4) as epool,
        tc.tile_pool(name="r", bufs=4) as rpool,
        tc.tile_pool(name="o", bufs=4) as opool,
        tc.tile_pool(name="pss", bufs=2, space="PSUM") as ps_s,
        tc.tile_pool(name="psz", bufs=4, space="PSUM") as psz,
    ):
        if stage <= 1:
            nc.sync.dma_start(out_d[0:P, :], ZTt[:, 0, 0:Dm])

        drain = [False]  # final-drain mode: outproj evictions move DVE->ACT

        def normalize(pr, qc, zA, zB):
            """ZT[:, q] = Z'[0:64, q] * (1 / l[q]); l arrives pre-broadcast
            on partitions 64:128 of the PV accumulators. DVE-only."""
            qs = slice(qc * SC, (qc + 1) * SC)
            rb = rpool.tile([64, 2, SC], f32, tag="rb")
            ls = rpool.tile([64, 2, SC], f32, tag="ls")
            nc.vector.tensor_copy(ls[:, 0, :], zA[Dh : 2 * Dh, :])
            nc.vector.tensor_copy(ls[:, 1, :], zB[Dh : 2 * Dh, :])
            nc.vector.reciprocal_approx_fast(rb[:], ls[:])
            nc.vector.tensor_mul(ZTt[0:64, pr, qs], zA[0:Dh, :], rb[:, 0, :])
            nc.vector.tensor_mul(ZTt[64:128, pr, qs], zB[0:Dh, :], rb[:, 1, :])

        def outproj_steps(qc):
            """Closures for this q-chunk's output projection, injected one at
            a time between later j-iterations to keep PE density high."""
            def step(t, dh2):
                def emit():
                    po = psz.tile([P, SC], f32, tag="z")
                    ds = slice(dh2 * SC, (dh2 + 1) * SC)
                    zs = slice(t * P, (t + 1) * P)
                    for pr in range(NPAIR):
                        nc.tensor.matmul(
                            po[:], ZTt[:, pr, zs], wo[:, pr, ds],
                            start=(pr == 0), stop=(pr == NPAIR - 1),
                        )
                    ot = opool.tile([P, SC], out_dt, tag="o")
                    # evict via DVE during flash, ACT during the final drain
                    if drain[0]:
                        nc.scalar.activation(
                            ot[:], po[:], mybir.ActivationFunctionType.Copy
                        )
                    else:
                        nc.vector.tensor_copy(ot[:], po[:])
                    nc.sync.dma_start(out_d[t * P : (t + 1) * P, ds], ot[:])
                return emit

            return [
                step(t, dh2)
                for t in range(qc * (SC // P), (qc + 1) * (SC // P))
                for dh2 in range(DH2)
            ]

        def v_step(qt):
            """One deferred V-projection group (f16): PE fill work."""
            def emit():
                psV = psz.tile([P, NH * Dh], f32, tag="z", name=f"psv_{qt}")
                for kt in range(KT):
                    nc.tensor.matmul(
                        psV[:],
                        xT[:, kt, qt * P : (qt + 1) * P],
                        wv[:, kt, :],
                        start=(kt == 0), stop=(kt == KT - 1),
                    )
                nc.vector.tensor_copy(
                    Vt[:, qt, :, 0:Dh],
                    psV[:].rearrange("p (h e) -> p h e", e=Dh),
                )
            return emit

        def qk_step(qc, pr, pj):
            """One deferred Q/K-projection group: 4 fp8 DoubleRow matmuls,
            DVE eviction to fp8 scratch, DMA-fold into Q8/K8."""
            def emit():
                ps = psz.tile([P, SC], f32, tag="z", name=f"psqk{qc}_{pr}_{pj}")
                qs = slice(qc * SC, (qc + 1) * SC)
                for kt2 in range(KT2):
                    nc.tensor.matmul(
                        ps[:], wqk8[:, :, kt2, pj, pr, :],
                        x8[:, :, kt2, qs],
                        start=(kt2 == 0), stop=(kt2 == KT2 - 1), perf_mode=DR,
                    )
                u8 = rpool.tile([P, SC], f8, tag="u8")
                nc.vector.tensor_scalar(
                    u8[:], ps[:], EVSCALE, bqk[:, pj, pr : pr + 1], mult, add
                )
                dst = Q8 if pj == 0 else K8
                for base in (0, 64):
                    for s in (0, 1):
                        nc.scalar.dma_start(
                            dst[base : base + 32, s, pr, qs],
                            u8[base + 32 * s : base + 32 * s + 32, :],
                        )
            return emit

        fill_queue = []
        for qc2 in range(2, QC):
            for pr2 in range(NPAIR):
                for pj2 in range(2):
                    fill_queue.append((qc2, qk_step(qc2, pr2, pj2)))
            for qt in range(qc2 * (SC // P), (qc2 + 1) * (SC // P)):
                fill_queue.append((qc2, v_step(qt)))
        op_queue = []
        chunk_tail = None
        for qc in range(QC if stage >= 2 else 0):
            while fill_queue and fill_queue[0][0] <= qc:
                fill_queue.pop(0)[1]()
            for pr in range(NPAIR):
                hA, hB = 2 * pr, 2 * pr + 1
                zA = psz.tile([P, SC], f32, tag="z")
                zB = psz.tile([P, SC], f32, tag="z")
                jmax = (qc + 1) * (SC // P)
                pends = []  # exp->PV pipeline, depth 2

                def emit_pv(j, eAB, c0, jmax=jmax, zA=zA, zB=zB, hA=hA, hB=hB):
                    st, sp = j == 0, j == jmax - 1
                    cs = slice(c0, SC)
                    nc.tensor.matmul(
                        zA[:, cs], Vt[:, j, hA, :], eAB[:, 0, cs],
                        start=st, stop=sp,
                    )
                    nc.tensor.matmul(
                        zB[:, cs], Vt[:, j, hB, :], eAB[:, 1, cs],
                        start=st, stop=sp,
                    )

                for j in range(jmax):
                    v = j - (jmax - SC // P)
                    c0 = v * P if v > 0 else 0
                    cs = slice(c0, SC)
                    qf = slice(qc * SC + c0, (qc + 1) * SC)
                    sAB = ps_s.tile([P, 2, SC], f32, tag="s")
                    ks = slice(j * P, (j + 1) * P)
                    # fp8 DoubleRow score pair: head A rows 0:32, head B
                    # rows 64:96 (different PE quadrants -> concurrent)
                    nc.tensor.matmul(
                        sAB[:, 0, cs],
                        K8[0:32, :, pr, ks], Q8[0:32, :, pr, qf],
                        start=True, stop=True, perf_mode=DR,
                    )
                    nc.tensor.matmul(
                        sAB[:, 1, cs],
                        K8[64:96, :, pr, ks], Q8[64:96, :, pr, qf],
                        start=True, stop=True, perf_mode=DR,
                    )
                    eAB = epool.tile([P, 2, SC], dt_w, tag="e")
                    nc.scalar.activation(
                        eAB[:, :, cs], sAB[:, :, cs], Exp, scale=exp_scale
                    )
                    if v >= 0:  # chunk contains the causal diagonal
                        mv = slice(c0, min((v + 1) * P, SC))
                        nc.gpsimd.tensor_mul(
                            eAB[:, 0, mv], eAB[:, 0, mv], masks[:, v, mv]
                        )
                        nc.gpsimd.tensor_mul(
                            eAB[:, 1, mv], eAB[:, 1, mv], masks[:, v, mv]
                        )
                    if stage >= 3:
                        pends.append((j, eAB, c0))
                        if len(pends) > 3:
                            emit_pv(*pends.pop(0))
                        if j == 0 and chunk_tail is not None:
                            chunk_tail()
                            chunk_tail = None
                        elif j >= 1 and fill_queue:
                            fill_queue.pop(0)[1]()
                        elif j >= 2 and op_queue and (
                            qc == QC - 1 or j % 3 == 0
                        ):
                            op_queue.pop(0)()
                    else:
                        last_e = eAB
                if stage < 3:
                    if pr == 0 and qc == 0:
                        nc.sync.dma_start(out_d[0:P, 0:SC], last_e[:, 0, :])
                    continue

                # pr-boundary cover
                if op_queue and qc >= 1:
                    op_queue.pop(0)()

                def chunk_tail(pends=pends, pr=pr, qc=qc, zA=zA, zB=zB,
                               emit_pv=emit_pv):
                    for p in pends:
                        emit_pv(*p)
                    normalize(pr, qc, zA, zB)
                    return pr, qc, zA, zB

            if stage >= 5:
                op_queue.extend(outproj_steps(qc))
        drain[0] = True
        if chunk_tail is not None:
            pends, pr, qc, zA, zB = (chunk_tail.__defaults__[:5])
            for p in pends:
                chunk_tail.__defaults__[5](*p)
            rb = rpool.tile([64, 2, SC], f32, tag="rb")
            ls = rpool.tile([64, 2, SC], f32, tag="ls")
            for ti in range(SC // P):
                cl = slice(ti * P, (ti + 1) * P)
                qsl = slice(qc * SC + ti * P, qc * SC + (ti + 1) * P)
                nc.vector.tensor_copy(ls[:, 0, cl], zA[Dh : 2 * Dh, cl])
                nc.vector.tensor_copy(ls[:, 1, cl], zB[Dh : 2 * Dh, cl])
                nc.vector.reciprocal_approx_fast(rb[:, :, cl], ls[:, :, cl])
                nc.vector.tensor_mul(ZTt[0:64, pr, qsl], zA[0:Dh, cl], rb[:, 0, cl])
                nc.vector.tensor_mul(ZTt[64:128, pr, qsl], zB[0:Dh, cl], rb[:, 1, cl])
                for _ in range(DH2):
                    if op_queue:
                        op_queue.pop(0)()
        for step in op_queue:
            step()
        if stage == 4:
            nc.sync.dma_start(out_d[0:P, :], ZTt[:, 0, 0:Dm])


def pack_inputs(x_b, W_Q, W_K, W_V, W_O, b_Q, b_K, hds):
    """Host-side packing of one core's shard into the kernel's layouts."""
    import ml_dtypes

    f8 = ml_dtypes.float8_e4m3
    Dm, Dh = W_Q.shape[1], W_Q.shape[2]
    S = x_b.shape[0]
    NH = len(hds)
    NPAIR = NH // 2
    KT = Dm // P
    KT2 = Dm // (2 * P)

    xb = np.asarray(x_b, np.float32)
    xT = np.ascontiguousarray(
        xb.T.reshape(KT, P, S).transpose(1, 0, 2)
    ).astype(np.float16)
    # fp8 x for the QK path: d = kt2*256 + slot*128 + p
    x8 = np.ascontiguousarray(
        xb.T.reshape(KT2, 2, P, S).transpose(2, 1, 0, 3)
    ).astype(f8)

    def pack_w8(W):  # [H, Dm, Dh] -> [P, 2, KT2, NPAIR, 128] fp8 at 64x
        W4 = np.asarray(W, np.float32)[hds]  # [NH, Dm, Dh]
        t = W4.reshape(NPAIR, 2, KT2, 2, P, Dh).transpose(4, 3, 2, 0, 1, 5)
        return (WSCALE * t.reshape(P, 2, KT2, NPAIR, 2 * Dh))

    wqk8 = np.ascontiguousarray(
        np.stack([pack_w8(W_Q), pack_w8(W_K)], axis=3)  # [P,2,KT2,2,NPAIR,128]
    ).astype(f8)

    WV4 = np.asarray(W_V, np.float32)[hds]
    wv = np.ascontiguousarray(
        WV4.reshape(NH, KT, P, Dh).transpose(2, 1, 0, 3).reshape(P, KT, NH * Dh)
    ).astype(np.float16)

    WO4 = np.asarray(W_O, np.float32)[hds]
    wo = np.ascontiguousarray(
        WO4.reshape(NPAIR, 2, Dh, Dm).transpose(1, 2, 0, 3).reshape(P, NPAIR, Dm)
    ).astype(np.float16)

    def pack_b(b):  # [H, Dh] -> [P, NPAIR], pre-scaled by QSCALE
        b4 = np.asarray(b, np.float32)[hds]
        return QSCALE * b4.reshape(NPAIR, 2, Dh).transpose(1, 2, 0).reshape(P, NPAIR)

    bqk = np.ascontiguousarray(
        np.stack([pack_b(b_Q), pack_b(b_K)], axis=1)  # [P, 2, NPAIR]
    ).astype(np.float32)

    return {"xT": xT, "x8": x8, "wqk8": wqk8, "wv": wv, "wo": wo, "bqk": bqk}


def kernel(x, W_Q, W_K, W_V, W_O, b_Q, b_K, b_V, b_O, _trace=False):
    from concourse.bass_utils import run_bass_kernel_spmd

    x = np.asarray(x, np.float32)
    B, S, Dm = x.shape
    H, _, Dh = W_Q.shape
    NCORES = 8
    GB = NCORES // B        # head groups per batch element
    NH = H // GB            # heads per core

    nc = build_nc(S, Dm, NH, Dh)

    in_maps = []
    for c in range(NCORES):
        b, g = c // GB, c % GB
        hds = list(range(g * NH, (g + 1) * NH))
        in_maps.append(
            pack_inputs(x[b], W_Q, W_K, W_V, W_O, b_Q, b_K, hds)
        )

    try:
        res = run_bass_kernel_spmd(
            nc, in_maps, core_ids=list(range(NCORES)), trace=_trace
        )
    except Exception:
        # transient device hiccups usually clear on retry
        res = run_bass_kernel_spmd(
            nc, in_maps, core_ids=list(range(NCORES)), trace=_trace
        )

    out = np.zeros((B, S, Dm), np.float32)
    for c in range(NCORES):
        out[c // GB] += res.results[c]["out"]

    # biases that commute out of the device kernel (softmax rows sum to 1)
    corr = np.asarray(b_O, np.float32) + np.einsum(
        "he,hed->d",
        np.asarray(b_V, np.float32),
        np.asarray(W_O, np.float32),
    )
    out += corr[None, None, :]

    if _trace:
        kernel.last_results = res
    return out


# revision 26
# speedup vs baseline: 1.1271x; 1.0920x over previous
"""Causal multi-head attention layer for Trainium2 (Bass/Tile), 8 NeuronCores.

Problem: x[B=2,S=2048,D=1024], H=16 heads, Dh=64.
Sharding: data-parallel over batch (2) x tensor-parallel over head groups (4):
each of the 8 cores handles one batch element and 4 heads, producing a partial
output [S, D]; the host sums the 4 head-group partials per batch (the
"all-reduce after the W_O contraction" done host-side since we return full
output anyway) and adds biases that commute out (b_O and sum_h b_V[h] @ W_O[h],
exact because softmax rows sum to 1).

Device kernel (per core). The SCORE path (QK projections + S=Q.K^T) runs in
fp8e4m3 with MatmulPerfMode.DoubleRow (2 fp8 MACs/PE-cell/cycle = 2x f16
matmul throughput, measured 216ns for K_eff=256,N=512 — same as one f16
K=128 matmul); the VALUE path (V projection, PV, output projection) stays
f16: fp8 quantization error on the score path averages out through softmax
(host-sim rel_absmax ~1.4e-2 vs the 2e-2 gate), but value-path fp8 error
(~3.6% rms) passes straight to the output and would fail.

  - x is fed twice: f16 x^T [128, KT=8, S] (V path) and fp8 x8
    [128, 2, KT2=4, S] with D-pairs packed in the DoubleRow slot dim
    (d = kt2*256 + slot*128 + p).
  - W_Q/W_K are host-packed fp8 at 64x scale (keeps the 0.02-std weights
    out of e4m3 denormals), wqk8 [128, 2, KT2, 2, NPAIR, 128]. A QK
    projection group is 4 DoubleRow matmuls (K_eff=256 each) instead of 8
    f16 ones. PSUM holds 64*q; eviction applies *1/16 (+4*bias) and writes
    fp8 Q8/K8 at 4x scale (sigma~2.6, e4m3-friendly); the *16 in the score
    product is folded into the exp scale (inv_sqrt_dh/16).
  - Scores matmul per j is a DoubleRow PAIR: head A packed [32,2] (e =
    slot*32 + p) at PE rows 0:32, head B at rows 64:96. Rows {0,64} are
    different PE quadrants so the two matmuls run fully concurrent
    (measured): one N-cycle pass for both heads vs ~1.7N for the old f16
    K=64 pair trick. Operand base partitions are restricted to {0,32,64}
    and quadrant concurrency needs {0,64}, so only 2 heads can fly at once.
  - Q8/K8 eviction cannot write the [32,2]-packed layout directly (it folds
    64 PSUM partitions onto 32) — evict full-width [128,SC] fp8 scratch
    (one DVE/ACT op, same cost as the old f16 eviction), then 4 tiny
    SBUF->SBUF DMAs fold it into Q8/K8. DMA queues are idle mid-kernel.
  - V computed in [k, e] layout from f16 x^T (stationary x^T tile, moving
    W_V, all 4 heads at once), stored as V'=[V|1...1] with the ones block
    replicated 64x so the PV matmul broadcasts the softmax denominator l
    across output partitions 64:128.
  - Scores computed TRANSPOSED: S^T[k, q], so softmax's sum rides the PV
    contraction: Z'[e|l, q] = V'.T @ exp(S^T) accumulated over k-tiles. No
    max-subtraction (scores are O(1), exp safe in f32).
  - Causal masking is multiplicative on exp(S^T), diagonal chunks only (on
    GpSimd); fully-masked column ranges are skipped via c0 slicing.
  - Normalization: l arrives pre-broadcast on PV-accumulator partitions
    64:128; wide DVE reciprocal_approx_fast + multiply. (Must stage l
    through SBUF — reciprocal on multi-matmul PSUM directly is garbage.)
  - Schedule (all tuned on HW, see git history of the f16 version):
      * Phase 1 computes only the first two q-chunks' Q/K projections
        (8 PSUM groups fed kt2-by-kt2 as the x8 DMA lands) and the first
        8 V tiles; the rest ride the flash loop as deadline-ordered PE
        fill work (fill_queue).
      * DMA order: bqk, wqk8+x8[chunks 0-1 cols] interleaved, f16 x^T
        ktile-by-ktile with wv at midpoint, x8[chunks 2-3 cols], wo.
      * exp->PV pipeline depth 2 (pends); out-proj METERED (every 3rd j)
        through middle chunks; pr-boundary cover steps; drain-phase
        normalize sliced per q-tile with out-proj interleaved.
      * Out-proj PSUM evicted on DVE during flash, ACT during drain; casts
        to f16 so the out DMA halves (host accumulates partials in f32).
  - CAUTION: instruction timings are extremely sensitive to SBUF tile
    layout (port contention). A/B any pool/tile change against the
    previous layout.
"""

import os
import numpy as np

P = 128
SC = 512  # q-chunk width (one PSUM bank of fp32)

_BUILD_CACHE = {}

WSCALE = 64.0   # host scale on W_Q/W_K before fp8 quantization
QSCALE = 4.0    # scale of Q8/K8 relative to true q,k
# eviction: psum = WSCALE * q  ->  Q8 = psum * (QSCALE/WSCALE) + QSCALE*b
EVSCALE = QSCALE / WSCALE
# score psum = QSCALE^2 * (q.k); fold into exp scale
SSCALE = 1.0 / (QSCALE * QSCALE)


def build_nc(S, Dm, NH, Dh, stage=99):
    """Build (and cache) the per-core Bass module. NH = heads per core."""
    key = (S, Dm, NH, Dh, stage)
    if key in _BUILD_CACHE:
        return _BUILD_CACHE[key]

    import concourse.bacc as bacc
    import concourse.mybir as mybir
    import concourse.tile as tile

    f32 = mybir.dt.float32
    f16 = mybir.dt.float16
    f8 = mybir.dt.float8e4
    DR = mybir.MatmulPerfMode.DoubleRow
    dt_w = f16   # value-path matmul dtype
    dt_m = f16   # mask dtype

    KT = Dm // P       # f16 k-tiles over the model dim
    KT2 = Dm // (2 * P)  # fp8 DoubleRow k-tiles (256 contraction each)
    NPAIR = NH // 2    # head pairs
    QC = S // SC       # q chunks
    NKT = S // P       # k-position tiles
    DH2 = Dm // SC     # output free-dim chunks
    assert Dh == 64 and NH % 2 == 0 and S % SC == 0 and Dm % SC == 0

    nc = bacc.Bacc(
        "TRN2",
        debug=False,
        enable_asserts=False,
        target_bir_lowering=False,
        num_devices=1,
    )

    xT_d = nc.dram_tensor("xT", [P, KT, S], f16, kind="ExternalInput")
    x8_d = nc.dram_tensor("x8", [P, 2, KT2, S], f8, kind="ExternalInput")
    wqk8_d = nc.dram_tensor(
        "wqk8", [P, 2, KT2, 2, NPAIR, P], f8, kind="ExternalInput"
    )
    wv_d = nc.dram_tensor("wv", [P, KT, NH * Dh], f16, kind="ExternalInput")
    wo_d = nc.dram_tensor("wo", [P, NPAIR, Dm], f16, kind="ExternalInput")
    bqk_d = nc.dram_tensor("bqk", [P, 2, NPAIR], f32, kind="ExternalInput")
    # output in f16 (halves the output DMA; host accumulates in f32)
    out_d = nc.dram_tensor("out", [S, Dm], f16, kind="ExternalOutput")

    Exp = mybir.ActivationFunctionType.Exp
    Ident = mybir.ActivationFunctionType.Identity
    exp_scale = float(SSCALE / np.sqrt(Dh))

    with tile.TileContext(nc) as tc:
        with tc.tile_pool(name="const", bufs=1) as cpool:
            wv = cpool.tile([P, KT, NH * Dh], f16)
            wo = cpool.tile([P, NPAIR, Dm], f16)
            bqk = cpool.tile([P, 2, NPAIR], f32)

            # fp8 Q/K in DoubleRow-packed layout: partition p in [0,32) +
            # slot s hold head A's e = s*32+p; partitions 64:96 head B.
            # (32:64 and 96:128 are dead — operand bases are {0,32,64} and
            # quadrant concurrency needs {0,64}.)
            Q8 = cpool.tile([P, 2, NPAIR, S], f8)
            K8 = cpool.tile([P, 2, NPAIR, S], f8)
            Vt = cpool.tile([P, NKT, NH, 2 * Dh], f16)

            # causal masks for the diagonal-chunk variants (S^T layout:
            # partition=k, free=q), built on GpSimd during the DMA wait
            masks = cpool.tile([P, SC // P, SC], dt_m)
            nc.gpsimd.memset(masks[:], 1.0)
            for v in range(SC // P):
                nc.gpsimd.affine_select(
                    out=masks[:, v, :],
                    in_=masks[:, v, :],
                    compare_op=mybir.AluOpType.is_ge,
                    fill=0.0,
                    base=-(v * P),
                    pattern=[[1, SC]],
                    channel_multiplier=-1,
                )

            # ---------- phase 1: projections for the first two q-chunks ----
            with (
                tc.tile_pool(name="p1", bufs=1) as p1pool,
                tc.tile_pool(name="ps1", bufs=8, space="PSUM") as ps1,
            ):
                wqk8 = cpool.tile([P, 2, KT2, 2, NPAIR, P], f8)
                x8 = cpool.tile([P, 2, KT2, S], f8)
                xT = cpool.tile([P, KT, S], f16)
                # DMA order: the fp8 QK stream first (it gates the flash
                # start), then the f16 x for the V path, then the deferred
                # x8 columns (feed the in-flash qk fills), then wo.
                nc.sync.dma_start(bqk[:], bqk_d[:])
                for kt2 in range(KT2):
                    nc.sync.dma_start(wqk8[:, :, kt2], wqk8_d[:, :, kt2])
                    nc.sync.dma_start(
                        x8[:, :, kt2, 0 : 2 * SC], x8_d[:, :, kt2, 0 : 2 * SC]
                    )
                for kt in range(KT):
                    nc.sync.dma_start(xT[:, kt, :], xT_d[:, kt, :])
                    if kt == KT // 2 - 1:
                        nc.sync.dma_start(wv[:], wv_d[:])
                for kt2 in range(KT2):
                    nc.sync.dma_start(
                        x8[:, :, kt2, 2 * SC : S], x8_d[:, :, kt2, 2 * SC : S]
                    )
                nc.sync.dma_start(wo[:], wo_d[:])

                # HAM warm-up: dummy matmuls during the initial DMA wait so
                # the PE clock-gate is at 8/8 when real work arrives
                wst = p1pool.tile([P, SC], f32)
                nc.vector.memset(wst[:], 1.0)
                # preload the Exp table on the Scalar engine now (idle)
                tpre = p1pool.tile([1, 2], f32)
                nc.scalar.activation(tpre[:], wst[0:1, 0:2], Exp)
                wrm = p1pool.tile([P, SC], f16)
                nc.vector.tensor_copy(wrm[:], wst[:])
                nwu = 10
                pwu = ps1.tile([P, SC], f32, tag="mm")
                for i in range(nwu):
                    nc.tensor.matmul(
                        pwu[:], wrm[:, 0:P], wrm[:],
                        start=(i == 0), stop=(i == nwu - 1),
                    )

                def fold_qk(u8, pj, pr, qc):
                    """4 SBUF->SBUF DMAs: unpacked fp8 [128,SC] eviction ->
                    DoubleRow-packed Q8/K8 slices. On the scalar HWDGE
                    queue: the sync queue is serialized behind the whole
                    multi-MB input stream (in-order per queue), which would
                    delay these folds — and the first flash scores — to
                    ~36us. The scalar queue is empty, and SBUF->SBUF steals
                    no HBM bandwidth."""
                    dst = Q8 if pj == 0 else K8
                    qs = slice(qc * SC, (qc + 1) * SC)
                    for base in (0, 64):
                        for s in (0, 1):
                            src = u8[base + 32 * s : base + 32 * s + 32, :]
                            nc.scalar.dma_start(
                                dst[base : base + 32, s, pr, qs], src
                            )

                # Q/K projections (first two q-chunks): 8 PSUM groups fed
                # kt2-by-kt2 as the x8 DMA lands
                for qg in range(0, min(2, QC), 2):
                    qcs = list(range(qg, min(qg + 2, QC)))
                    pss = {
                        (pr, pj, qc): ps1.tile(
                            [P, SC], f32, tag="mm", name=f"psqk_{pr}_{pj}_{qc}"
                        )
                        for pr in range(NPAIR)
                        for pj in range(2)
                        for qc in qcs
                    }
                    for kt2 in range(KT2):
                        st, sp = kt2 == 0, kt2 == KT2 - 1
                        for pr in range(NPAIR):
                            for pj in range(2):
                                for qc in qcs:
                                    xs = x8[:, :, kt2, qc * SC : (qc + 1) * SC]
                                    nc.tensor.matmul(
                                        pss[(pr, pj, qc)][:],
                                        wqk8[:, :, kt2, pj, pr, :], xs,
                                        start=st, stop=sp, perf_mode=DR,
                                    )
                    for pr in range(NPAIR):
                        for qc in qcs:
                            for pj in range(2):
                                # evict via ACT (idle in phase 1; Identity
                                # shares the Exp table) to full-width fp8
                                # scratch, then DMA-fold into Q8/K8
                                u8 = p1pool.tile(
                                    [P, SC], f8, name=f"u8_{pr}_{pj}_{qc}"
                                )
                                nc.scalar.activation(
                                    u8[:], pss[(pr, pj, qc)][:], Ident,
                                    bias=bqk[:, pj, pr : pr + 1],
                                    scale=EVSCALE,
                                )
                                fold_qk(u8, pj, pr, qc)

                # V tiles the first two flash chunks touch; rest deferred
                for qt in range(min(2 * (SC // P), NKT)):
                    psV = ps1.tile([P, NH * Dh], f32, tag="mm")
                    for kt in range(KT):
                        nc.tensor.matmul(
                            psV[:],
                            xT[:, kt, qt * P : (qt + 1) * P],
                            wv[:, kt, :],
                            start=(kt == 0), stop=(kt == KT - 1),
                        )
                    nc.vector.tensor_copy(
                        Vt[:, qt, :, 0:Dh],
                        psV[:].rearrange("p (h e) -> p h e", e=Dh),
                    )

                # V' ones block (broadcasts l onto PV partitions 64:128)
                cstage = p1pool.tile([P, 1, 1, Dh], f32)
                nc.vector.memset(cstage[:], 1.0)
                nc.vector.tensor_copy(
                    Vt[:, :, :, Dh : 2 * Dh],
                    cstage[:].to_broadcast((P, NKT, NH, Dh)),
                )

            # ---------- phases 2+3 ----------
            with tc.tile_pool(name="zt", bufs=1) as ztpool:
                ZTt = ztpool.tile([P, NPAIR, S], f16)
                self_flash(
                    nc, tc, stage, Exp, exp_scale, mybir,
                    Q8, K8, Vt, ZTt, wo, out_d, masks, xT, x8, wv, wqk8, bqk,
                    S, Dm, Dh, NPAIR, QC, SC, P, DH2, KT, KT2, NKT,
                    f16, dt_m, f32, f8, DR,
                )

    nc.compile()
    _BUILD_CACHE[key] = nc
    return nc


def self_flash(
    nc, tc, stage, Exp, exp_scale, mybir,
    Q8, K8, Vt, ZTt, wo, out_d, masks, xT, x8, wv, wqk8, bqk,
    S, Dm, Dh, NPAIR, QC, SC, P, DH2, KT, KT2, NKT,
    dt_w, dt_m, f32, f8, DR,
):
    NH = Vt.shape[2]
    # ---------- phases 2+3: flash attention (scores transposed, fp8
    # DoubleRow) with the output projection interleaved one q-chunk behind
    out_dt = dt_w
    mult, add = mybir.AluOpType.mult, mybir.AluOpType.add
    with (
        tc.tile_pool(name="e", bufs=4) as epool,
        tc.tile_pool(name="r", bufs=4) as rpool,
        tc.tile_pool(name="o", bufs=4) as opool,
        tc.tile_pool(name="pss", bufs=2, space="PSUM") as ps_s,
        tc.tile_pool(name="psz", bufs=4, space="PSUM") as psz,
    ):
        if stage <= 1:
            nc.sync.dma_start(out_d[0:P, :], ZTt[:, 0, 0:Dm])

        drain = [False]  # final-drain mode: outproj evictions move DVE->ACT

        def normalize(pr, qc, zA, zB):
            """ZT[:, q] = Z'[0:64, q] * (1 / l[q]); l arrives pre-broadcast
            on partitions 64:128 of the PV accumulators. DVE-only."""
            qs = slice(qc * SC, (qc + 1) * SC)
            rb = rpool.tile([64, 2, SC], f32, tag="rb")
            ls = rpool.tile([64, 2, SC], f32, tag="ls")
            nc.vector.tensor_copy(ls[:, 0, :], zA[Dh : 2 * Dh, :])
            nc.vector.tensor_copy(ls[:, 1, :], zB[Dh : 2 * Dh, :])
            nc.vector.reciprocal_approx_fast(rb[:], ls[:])
            nc.vector.tensor_mul(ZTt[0:64, pr, qs], zA[0:Dh, :], rb[:, 0, :])
            nc.vector.tensor_mul(ZTt[64:128, pr, qs], zB[0:Dh, :], rb[:, 1, :])

        def outproj_steps(qc):
            """Closures for this q-chunk's output projection, injected one at
            a time between later j-iterations to keep PE density high."""
            def step(t, dh2):
                def emit():
                    po = psz.tile([P, SC], f32, tag="z")
                    ds = slice(dh2 * SC, (dh2 + 1) * SC)
                    zs = slice(t * P, (t + 1) * P)
                    for pr in range(NPAIR):
                        nc.tensor.matmul(
                            po[:], ZTt[:, pr, zs], wo[:, pr, ds],
                            start=(pr == 0), stop=(pr == NPAIR - 1),
                        )
                    ot = opool.tile([P, SC], out_dt, tag="o")
                    # evict via DVE during flash, ACT during the final drain
                    if drain[0]:
                        nc.scalar.activation(
                            ot[:], po[:], mybir.ActivationFunctionType.Copy
                        )
                    else:
                        nc.vector.tensor_copy(ot[:], po[:])
                    nc.sync.dma_start(out_d[t * P : (t + 1) * P, ds], ot[:])
                return emit

            return [
                step(t, dh2)
                for t in range(qc * (SC // P), (qc + 1) * (SC // P))
                for dh2 in range(DH2)
            ]

        def v_step(qt):
            """One deferred V-projection group (f16): PE fill work."""
            def emit():
                psV = psz.tile([P, NH * Dh], f32, tag="z", name=f"psv_{qt}")
                for kt in range(KT):
                    nc.tensor.matmul(
                        psV[:],
                        xT[:, kt, qt * P : (qt + 1) * P],
                        wv[:, kt, :],
                        start=(kt == 0), stop=(kt == KT - 1),
                    )
                nc.vector.tensor_copy(
                    Vt[:, qt, :, 0:Dh],
                    psV[:].rearrange("p (h e) -> p h e", e=Dh),
                )
            return emit

        def qk_step(qc, pr, pj):
            """One deferred Q/K-projection group: 4 fp8 DoubleRow matmuls,
            DVE eviction to fp8 scratch, DMA-fold into Q8/K8."""
            def emit():
                ps = psz.tile([P, SC], f32, tag="z", name=f"psqk{qc}_{pr}_{pj}")
                qs = slice(qc * SC, (qc + 1) * SC)
                for kt2 in range(KT2):
                    nc.tensor.matmul(
                        ps[:], wqk8[:, :, kt2, pj, pr, :],
                        x8[:, :, kt2, qs],
                        start=(kt2 == 0), stop=(kt2 == KT2 - 1), perf_mode=DR,
                    )
                u8 = rpool.tile([P, SC], f8, tag="u8")
                nc.vector.tensor_scalar(
                    u8[:], ps[:], EVSCALE, bqk[:, pj, pr : pr + 1], mult, add
                )
                dst = Q8 if pj == 0 else K8
                for base in (0, 64):
                    for s in (0, 1):
                        nc.sync.dma_start(
                            dst[base : base + 32, s, pr, qs],
                            u8[base + 32 * s : base + 32 * s + 32, :],
                        )
            return emit

        fill_queue = []
        for qc2 in range(2, QC):
            for pr2 in range(NPAIR):
                for pj2 in range(2):
                    fill_queue.append((qc2, qk_step(qc2, pr2, pj2)))
            for qt in range(qc2 * (SC // P), (qc2 + 1) * (SC // P)):
                fill_queue.append((qc2, v_step(qt)))
        op_queue = []
        chunk_tail = None
        for qc in range(QC if stage >= 2 else 0):
            while fill_queue and fill_queue[0][0] <= qc:
                fill_queue.pop(0)[1]()
            for pr in range(NPAIR):
                hA, hB = 2 * pr, 2 * pr + 1
                zA = psz.tile([P, SC], f32, tag="z")
                zB = psz.tile([P, SC], f32, tag="z")
                jmax = (qc + 1) * (SC // P)
                pends = []  # exp->PV pipeline, depth 2

                def emit_pv(j, eAB, c0, jmax=jmax, zA=zA, zB=zB, hA=hA, hB=hB):
                    st, sp = j == 0, j == jmax - 1
                    cs = slice(c0, SC)
                    nc.tensor.matmul(
                        zA[:, cs], Vt[:, j, hA, :], eAB[:, 0, cs],
                        start=st, stop=sp,
                    )
                    nc.tensor.matmul(
                        zB[:, cs], Vt[:, j, hB, :], eAB[:, 1, cs],
                        start=st, stop=sp,
                    )

                for j in range(jmax):
                    v = j - (jmax - SC // P)
                    c0 = v * P if v > 0 else 0
                    cs = slice(c0, SC)
                    qf = slice(qc * SC + c0, (qc + 1) * SC)
                    sAB = ps_s.tile([P, 2, SC], f32, tag="s")
                    ks = slice(j * P, (j + 1) * P)
                    # fp8 DoubleRow score pair: head A rows 0:32, head B
                    # rows 64:96 (different PE quadrants -> concurrent)
                    nc.tensor.matmul(
                        sAB[:, 0, cs],
                        K8[0:32, :, pr, ks], Q8[0:32, :, pr, qf],
                        start=True, stop=True, perf_mode=DR,
                    )
                    nc.tensor.matmul(
                        sAB[:, 1, cs],
                        K8[64:96, :, pr, ks], Q8[64:96, :, pr, qf],
                        start=True, stop=True, perf_mode=DR,
                    )
                    eAB = epool.tile([P, 2, SC], dt_w, tag="e")
                    nc.scalar.activation(
                        eAB[:, :, cs], sAB[:, :, cs], Exp, scale=exp_scale
                    )
                    if v >= 0:  # chunk contains the causal diagonal
                        mv = slice(c0, min((v + 1) * P, SC))
                        nc.gpsimd.tensor_mul(
                            eAB[:, 0, mv], eAB[:, 0, mv], masks[:, v, mv]
                        )
                        nc.gpsimd.tensor_mul(
                            eAB[:, 1, mv], eAB[:, 1, mv], masks[:, v, mv]
                        )
                    if stage >= 3:
                        pends.append((j, eAB, c0))
                        if len(pends) > 3:
                            emit_pv(*pends.pop(0))
                        if j == 0 and chunk_tail is not None:
                            chunk_tail()
                            chunk_tail = None
                        elif j >= 1 and fill_queue:
                            fill_queue.pop(0)[1]()
                        elif j >= 2 and op_queue and (
                            qc == QC - 1 or j % 3 == 0
                        ):
                            op_queue.pop(0)()
                    else:
                        last_e = eAB
                if stage < 3:
                    if pr == 0 and qc == 0:
                        nc.sync.dma_start(out_d[0:P, 0:SC], last_e[:, 0, :])
                    continue

                # pr-boundary cover
                if op_queue and qc >= 1:
                    op_queue.pop(0)()

                def chunk_tail(pends=pends, pr=pr, qc=qc, zA=zA, zB=zB,
                               emit_pv=emit_pv):
                    for p in pends:
                        emit_pv(*p)
                    normalize(pr, qc, zA, zB)
                    return pr, qc, zA, zB

            if stage >= 5:
                op_queue.extend(outproj_steps(qc))
        drain[0] = True
        if chunk_tail is not None:
            pends, pr, qc, zA, zB = (chunk_tail.__defaults__[:5])
            for p in pends:
                chunk_tail.__defaults__[5](*p)
            rb = rpool.tile([64, 2, SC], f32, tag="rb")
            ls = rpool.tile([64, 2, SC], f32, tag="ls")
            for ti in range(SC // P):
                cl = slice(ti * P, (ti + 1) * P)
                qsl = slice(qc * SC + ti * P, qc * SC + (ti + 1) * P)
                nc.vector.tensor_copy(ls[:, 0, cl], zA[Dh : 2 * Dh, cl])
                nc.vector.tensor_copy(ls[:, 1, cl], zB[Dh : 2 * Dh, cl])
                nc.vector.reciprocal_approx_fast(rb[:, :, cl], ls[:, :, cl])
                nc.vector.tensor_mul(ZTt[0:64, pr, qsl], zA[0:Dh, cl], rb[:, 0, cl])
                nc.vector.tensor_mul(ZTt[64:128, pr, qsl], zB[0:Dh, cl], rb[:, 1, cl])
                for _ in range(DH2):
                    if op_queue:
                        op_queue.pop(0)()
        for step in op_queue:
            step()
        if stage == 4:
            nc.sync.dma_start(out_d[0:P, :], ZTt[:, 0, 0:Dm])


def pack_inputs(x_b, W_Q, W_K, W_V, W_O, b_Q, b_K, hds):
    """Host-side packing of one core's shard into the kernel's layouts."""
    import ml_dtypes

    f8 = ml_dtypes.float8_e4m3
    Dm, Dh = W_Q.shape[1], W_Q.shape[2]
    S = x_b.shape[0]
    NH = len(hds)
    NPAIR = NH // 2
    KT = Dm // P
    KT2 = Dm // (2 * P)

    xb = np.asarray(x_b, np.float32)
    xT = np.ascontiguousarray(
        xb.T.reshape(KT, P, S).transpose(1, 0, 2)
    ).astype(np.float16)
    # fp8 x for the QK path: d = kt2*256 + slot*128 + p
    x8 = np.ascontiguousarray(
        xb.T.reshape(KT2, 2, P, S).transpose(2, 1, 0, 3)
    ).astype(f8)

    def pack_w8(W):  # [H, Dm, Dh] -> [P, 2, KT2, NPAIR, 128] fp8 at 64x
        W4 = np.asarray(W, np.float32)[hds]  # [NH, Dm, Dh]
        t = W4.reshape(NPAIR, 2, KT2, 2, P, Dh).transpose(4, 3, 2, 0, 1, 5)
        return (WSCALE * t.reshape(P, 2, KT2, NPAIR, 2 * Dh))

    wqk8 = np.ascontiguousarray(
        np.stack([pack_w8(W_Q), pack_w8(W_K)], axis=3)  # [P,2,KT2,2,NPAIR,128]
    ).astype(f8)

    WV4 = np.asarray(W_V, np.float32)[hds]
    wv = np.ascontiguousarray(
        WV4.reshape(NH, KT, P, Dh).transpose(2, 1, 0, 3).reshape(P, KT, NH * Dh)
    ).astype(np.float16)

    WO4 = np.asarray(W_O, np.float32)[hds]
    wo = np.ascontiguousarray(
        WO4.reshape(NPAIR, 2, Dh, Dm).transpose(1, 2, 0, 3).reshape(P, NPAIR, Dm)
    ).astype(np.float16)

    def pack_b(b):  # [H, Dh] -> [P, NPAIR], pre-scaled by QSCALE
        b4 = np.asarray(b, np.float32)[hds]
        return QSCALE * b4.reshape(NPAIR, 2, Dh).transpose(1, 2, 0).reshape(P, NPAIR)

    bqk = np.ascontiguousarray(
        np.stack([pack_b(b_Q), pack_b(b_K)], axis=1)  # [P, 2, NPAIR]
    ).astype(np.float32)

    return {"xT": xT, "x8": x8, "wqk8": wqk8, "wv": wv, "wo": wo, "bqk": bqk}


def kernel(x, W_Q, W_K, W_V, W_O, b_Q, b_K, b_V, b_O, _trace=False):
    from concourse.bass_utils import run_bass_kernel_spmd

    x = np.asarray(x, np.float32)
    B, S, Dm = x.shape
    H, _, Dh = W_Q.shape
    NCORES = 8
    GB = NCORES // B        # head groups per batch element
    NH = H // GB            # heads per core

    nc = build_nc(S, Dm, NH, Dh)

    in_maps = []
    for c in range(NCORES):
        b, g = c // GB, c % GB
        hds = list(range(g * NH, (g + 1) * NH))
        in_maps.append(
            pack_inputs(x[b], W_Q, W_K, W_V, W_O, b_Q, b_K, hds)
        )

    try:
        res = run_bass_kernel_spmd(
            nc, in_maps, core_ids=list(range(NCORES)), trace=_trace
        )
    except Exception:
        # transient device hiccups usually clear on retry
        res = run_bass_kernel_spmd(
            nc, in_maps, core_ids=list(range(NCORES)), trace=_trace
        )

    out = np.zeros((B, S, Dm), np.float32)
    for c in range(NCORES):
        out[c // GB] += res.results[c]["out"]

    # biases that commute out of the device kernel (softmax rows sum to 1)
    corr = np.asarray(b_O, np.float32) + np.einsum(
        "he,hed->d",
        np.asarray(b_V, np.float32),
        np.asarray(W_O, np.float32),
    )
    out += corr[None, None, :]

    if _trace:
        kernel.last_results = res
    return out


# revision 27
# speedup vs baseline: 1.1460x; 1.0168x over previous
"""Causal multi-head attention layer for Trainium2 (Bass/Tile), 8 NeuronCores.

Problem: x[B=2,S=2048,D=1024], H=16 heads, Dh=64.
Sharding: data-parallel over batch (2) x tensor-parallel over head groups (4):
each of the 8 cores handles one batch element and 4 heads, producing a partial
output [S, D]; the host sums the 4 head-group partials per batch (the
"all-reduce after the W_O contraction" done host-side since we return full
output anyway) and adds biases that commute out (b_O and sum_h b_V[h] @ W_O[h],
exact because softmax rows sum to 1).

Device kernel (per core). The SCORE path (QK projections + S=Q.K^T) runs in
fp8e4m3 with MatmulPerfMode.DoubleRow (2 fp8 MACs/PE-cell/cycle = 2x f16
matmul throughput, measured 216ns for K_eff=256,N=512 — same as one f16
K=128 matmul); the VALUE path (V projection, PV, output projection) stays
f16: fp8 quantization error on the score path averages out through softmax
(host-sim rel_absmax ~1.4e-2 vs the 2e-2 gate), but value-path fp8 error
(~3.6% rms) passes straight to the output and would fail.

  - x is fed twice: f16 x^T [128, KT=8, S] (V path) and fp8 x8
    [128, 2, KT2=4, S] with D-pairs packed in the DoubleRow slot dim
    (d = kt2*256 + slot*128 + p).
  - W_Q/W_K are host-packed fp8 at 64x scale (keeps the 0.02-std weights
    out of e4m3 denormals), wqk8 [128, 2, KT2, 2, NPAIR, 128]. A QK
    projection group is 4 DoubleRow matmuls (K_eff=256 each) instead of 8
    f16 ones. PSUM holds 64*q; eviction applies *1/16 (+4*bias) and writes
    fp8 Q8/K8 at 4x scale (sigma~2.6, e4m3-friendly); the *16 in the score
    product is folded into the exp scale (inv_sqrt_dh/16).
  - Scores matmul per j is a DoubleRow PAIR: head A packed [32,2] (e =
    slot*32 + p) at PE rows 0:32, head B at rows 64:96. Rows {0,64} are
    different PE quadrants so the two matmuls run fully concurrent
    (measured): one N-cycle pass for both heads vs ~1.7N for the old f16
    K=64 pair trick. Operand base partitions are restricted to {0,32,64}
    and quadrant concurrency needs {0,64}, so only 2 heads can fly at once.
  - Q8/K8 eviction cannot write the [32,2]-packed layout directly (it folds
    64 PSUM partitions onto 32) — evict full-width [128,SC] fp8 scratch
    (one DVE/ACT op, same cost as the old f16 eviction), then 4 tiny
    SBUF->SBUF DMAs fold it into Q8/K8. DMA queues are idle mid-kernel.
  - V computed in [k, e] layout from f16 x^T (stationary x^T tile, moving
    W_V, all 4 heads at once), stored as V'=[V|1...1] with the ones block
    replicated 64x so the PV matmul broadcasts the softmax denominator l
    across output partitions 64:128.
  - Scores computed TRANSPOSED: S^T[k, q], so softmax's sum rides the PV
    contraction: Z'[e|l, q] = V'.T @ exp(S^T) accumulated over k-tiles. No
    max-subtraction (scores are O(1), exp safe in f32).
  - Causal masking is multiplicative on exp(S^T), diagonal chunks only (on
    GpSimd); fully-masked column ranges are skipped via c0 slicing.
  - Normalization: l arrives pre-broadcast on PV-accumulator partitions
    64:128; wide DVE reciprocal_approx_fast + multiply. (Must stage l
    through SBUF — reciprocal on multi-matmul PSUM directly is garbage.)
  - Schedule (all tuned on HW, see git history of the f16 version):
      * Phase 1 computes only the first two q-chunks' Q/K projections
        (8 PSUM groups fed kt2-by-kt2 as the x8 DMA lands) and the first
        8 V tiles; the rest ride the flash loop as deadline-ordered PE
        fill work (fill_queue).
      * DMA order: bqk, wqk8+x8[chunks 0-1 cols] interleaved, f16 x^T
        ktile-by-ktile with wv at midpoint, x8[chunks 2-3 cols], wo.
      * exp->PV pipeline depth 2 (pends); out-proj METERED (every 3rd j)
        through middle chunks; pr-boundary cover steps; drain-phase
        normalize sliced per q-tile with out-proj interleaved.
      * Out-proj PSUM evicted on DVE during flash, ACT during drain; casts
        to f16 so the out DMA halves (host accumulates partials in f32).
  - CAUTION: instruction timings are extremely sensitive to SBUF tile
    layout (port contention). A/B any pool/tile change against the
    previous layout.
"""

import os
import numpy as np

P = 128
SC = 512  # q-chunk width (one PSUM bank of fp32)

_BUILD_CACHE = {}

WSCALE = 64.0   # host scale on W_Q/W_K before fp8 quantization
QSCALE = 4.0    # scale of Q8/K8 relative to true q,k
# eviction: psum = WSCALE * q  ->  Q8 = psum * (QSCALE/WSCALE) + QSCALE*b
EVSCALE = QSCALE / WSCALE
# score psum = QSCALE^2 * (q.k); fold into exp scale
SSCALE = 1.0 / (QSCALE * QSCALE)


def build_nc(S, Dm, NH, Dh, stage=99):
    """Build (and cache) the per-core Bass module. NH = heads per core."""
    key = (S, Dm, NH, Dh, stage)
    if key in _BUILD_CACHE:
        return _BUILD_CACHE[key]

    import concourse.bacc as bacc
    import concourse.mybir as mybir
    import concourse.tile as tile

    f32 = mybir.dt.float32
    f16 = mybir.dt.float16
    f8 = mybir.dt.float8e4
    DR = mybir.MatmulPerfMode.DoubleRow
    dt_w = f16   # value-path matmul dtype
    dt_m = f16   # mask dtype

    KT = Dm // P       # f16 k-tiles over the model dim
    KT2 = Dm // (2 * P)  # fp8 DoubleRow k-tiles (256 contraction each)
    NPAIR = NH // 2    # head pairs
    QC = S // SC       # q chunks
    NKT = S // P       # k-position tiles
    DH2 = Dm // SC     # output free-dim chunks
    assert Dh == 64 and NH % 2 == 0 and S % SC == 0 and Dm % SC == 0

    nc = bacc.Bacc(
        "TRN2",
        debug=False,
        enable_asserts=False,
        target_bir_lowering=False,
        num_devices=1,
    )

    xT_d = nc.dram_tensor("xT", [P, KT, S], f16, kind="ExternalInput")
    x8_d = nc.dram_tensor("x8", [P, 2, KT2, S], f8, kind="ExternalInput")
    wqk8_d = nc.dram_tensor(
        "wqk8", [P, 2, KT2, 2, NPAIR, P], f8, kind="ExternalInput"
    )
    wv_d = nc.dram_tensor("wv", [P, KT, NH * Dh], f16, kind="ExternalInput")
    wo_d = nc.dram_tensor("wo", [P, NPAIR, Dm], f16, kind="ExternalInput")
    bqk_d = nc.dram_tensor("bqk", [P, 2, NPAIR], f32, kind="ExternalInput")
    # output in f16 (halves the output DMA; host accumulates in f32)
    out_d = nc.dram_tensor("out", [S, Dm], f16, kind="ExternalOutput")

    Exp = mybir.ActivationFunctionType.Exp
    Ident = mybir.ActivationFunctionType.Identity
    exp_scale = float(SSCALE / np.sqrt(Dh))

    with tile.TileContext(nc) as tc:
        with tc.tile_pool(name="const", bufs=1) as cpool:
            wv = cpool.tile([P, KT, NH * Dh], f16)
            wo = cpool.tile([P, NPAIR, Dm], f16)
            bqk = cpool.tile([P, 2, NPAIR], f32)

            # fp8 Q/K in DoubleRow-packed layout: partition p in [0,32) +
            # slot s hold head A's e = s*32+p; partitions 64:96 head B.
            # (32:64 and 96:128 are dead — operand bases are {0,32,64} and
            # quadrant concurrency needs {0,64}.)
            Q8 = cpool.tile([P, 2, NPAIR, S], f8)
            K8 = cpool.tile([P, 2, NPAIR, S], f8)
            Vt = cpool.tile([P, NKT, NH, 2 * Dh], f16)

            # causal masks for the diagonal-chunk variants (S^T layout:
            # partition=k, free=q), built on GpSimd during the DMA wait
            masks = cpool.tile([P, SC // P, SC], dt_m)
            nc.gpsimd.memset(masks[:], 1.0)
            for v in range(SC // P):
                nc.gpsimd.affine_select(
                    out=masks[:, v, :],
                    in_=masks[:, v, :],
                    compare_op=mybir.AluOpType.is_ge,
                    fill=0.0,
                    base=-(v * P),
                    pattern=[[1, SC]],
                    channel_multiplier=-1,
                )

            # ---------- phase 1: projections for the first two q-chunks ----
            with (
                tc.tile_pool(name="p1", bufs=1) as p1pool,
                tc.tile_pool(name="ps1", bufs=8, space="PSUM") as ps1,
            ):
                wqk8 = cpool.tile([P, 2, KT2, 2, NPAIR, P], f8)
                x8 = cpool.tile([P, 2, KT2, S], f8)
                xT = cpool.tile([P, KT, S], f16)
                # DMA order: the fp8 QK stream first (it gates the flash
                # start), then the f16 x for the V path, then the deferred
                # x8 columns (feed the in-flash qk fills), then wo.
                nc.sync.dma_start(bqk[:], bqk_d[:])
                for kt2 in range(KT2):
                    nc.sync.dma_start(wqk8[:, :, kt2], wqk8_d[:, :, kt2])
                    nc.sync.dma_start(
                        x8[:, :, kt2, 0 : 2 * SC], x8_d[:, :, kt2, 0 : 2 * SC]
                    )
                for kt in range(KT):
                    nc.sync.dma_start(xT[:, kt, :], xT_d[:, kt, :])
                    if kt == KT // 2 - 1:
                        nc.sync.dma_start(wv[:], wv_d[:])
                for kt2 in range(KT2):
                    nc.sync.dma_start(
                        x8[:, :, kt2, 2 * SC : S], x8_d[:, :, kt2, 2 * SC : S]
                    )
                nc.sync.dma_start(wo[:], wo_d[:])

                # HAM warm-up: dummy matmuls during the initial DMA wait so
                # the PE clock-gate is at 8/8 when real work arrives
                wst = p1pool.tile([P, SC], f32)
                nc.vector.memset(wst[:], 1.0)
                # preload the Exp table on the Scalar engine now (idle)
                tpre = p1pool.tile([1, 2], f32)
                nc.scalar.activation(tpre[:], wst[0:1, 0:2], Exp)
                wrm = p1pool.tile([P, SC], f16)
                nc.vector.tensor_copy(wrm[:], wst[:])
                nwu = 10
                pwu = ps1.tile([P, SC], f32, tag="mm")
                for i in range(nwu):
                    nc.tensor.matmul(
                        pwu[:], wrm[:, 0:P], wrm[:],
                        start=(i == 0), stop=(i == nwu - 1),
                    )

                def fold_qk(u8, pj, pr, qc):
                    """4 SBUF->SBUF DMAs: unpacked fp8 [128,SC] eviction ->
                    DoubleRow-packed Q8/K8 slices. On the scalar HWDGE
                    queue: the sync queue is serialized behind the whole
                    multi-MB input stream (in-order per queue), which would
                    delay these folds — and the first flash scores — to
                    ~36us. The scalar queue is empty, and SBUF->SBUF steals
                    no HBM bandwidth."""
                    dst = Q8 if pj == 0 else K8
                    qs = slice(qc * SC, (qc + 1) * SC)
                    for base in (0, 64):
                        for s in (0, 1):
                            src = u8[base + 32 * s : base + 32 * s + 32, :]
                            nc.scalar.dma_start(
                                dst[base : base + 32, s, pr, qs], src
                            )

                # Q/K projections (first two q-chunks): 8 PSUM groups fed
                # kt2-by-kt2 as the x8 DMA lands
                for qg in range(0, min(2, QC), 2):
                    qcs = list(range(qg, min(qg + 2, QC)))
                    pss = {
                        (pr, pj, qc): ps1.tile(
                            [P, SC], f32, tag="mm", name=f"psqk_{pr}_{pj}_{qc}"
                        )
                        for pr in range(NPAIR)
                        for pj in range(2)
                        for qc in qcs
                    }
                    for kt2 in range(KT2):
                        st, sp = kt2 == 0, kt2 == KT2 - 1
                        for pr in range(NPAIR):
                            for pj in range(2):
                                for qc in qcs:
                                    xs = x8[:, :, kt2, qc * SC : (qc + 1) * SC]
                                    nc.tensor.matmul(
                                        pss[(pr, pj, qc)][:],
                                        wqk8[:, :, kt2, pj, pr, :], xs,
                                        start=st, stop=sp, perf_mode=DR,
                                    )
                    for qc in qcs:  # qc-major: chunk 0's folds land first
                        for pr in range(NPAIR):
                            for pj in range(2):
                                # evict via ACT (idle in phase 1; Identity
                                # shares the Exp table) to full-width fp8
                                # scratch, then DMA-fold into Q8/K8
                                u8 = p1pool.tile(
                                    [P, SC], f8, name=f"u8_{pr}_{pj}_{qc}"
                                )
                                nc.scalar.activation(
                                    u8[:], pss[(pr, pj, qc)][:], Ident,
                                    bias=bqk[:, pj, pr : pr + 1],
                                    scale=EVSCALE,
                                )
                                fold_qk(u8, pj, pr, qc)

                # V tiles the first two flash chunks touch; rest deferred
                for qt in range(min(2 * (SC // P), NKT)):
                    psV = ps1.tile([P, NH * Dh], f32, tag="mm")
                    for kt in range(KT):
                        nc.tensor.matmul(
                            psV[:],
                            xT[:, kt, qt * P : (qt + 1) * P],
                            wv[:, kt, :],
                            start=(kt == 0), stop=(kt == KT - 1),
                        )
                    nc.vector.tensor_copy(
                        Vt[:, qt, :, 0:Dh],
                        psV[:].rearrange("p (h e) -> p h e", e=Dh),
                    )

                # V' ones block (broadcasts l onto PV partitions 64:128)
                cstage = p1pool.tile([P, 1, 1, Dh], f32)
                nc.vector.memset(cstage[:], 1.0)
                nc.vector.tensor_copy(
                    Vt[:, :, :, Dh : 2 * Dh],
                    cstage[:].to_broadcast((P, NKT, NH, Dh)),
                )

            # ---------- phases 2+3 ----------
            with tc.tile_pool(name="zt", bufs=1) as ztpool:
                ZTt = ztpool.tile([P, NPAIR, S], f16)
                self_flash(
                    nc, tc, stage, Exp, exp_scale, mybir,
                    Q8, K8, Vt, ZTt, wo, out_d, masks, xT, x8, wv, wqk8, bqk,
                    S, Dm, Dh, NPAIR, QC, SC, P, DH2, KT, KT2, NKT,
                    f16, dt_m, f32, f8, DR,
                )

    nc.compile()
    _BUILD_CACHE[key] = nc
    return nc


def self_flash(
    nc, tc, stage, Exp, exp_scale, mybir,
    Q8, K8, Vt, ZTt, wo, out_d, masks, xT, x8, wv, wqk8, bqk,
    S, Dm, Dh, NPAIR, QC, SC, P, DH2, KT, KT2, NKT,
    dt_w, dt_m, f32, f8, DR,
):
    NH = Vt.shape[2]
    # ---------- phases 2+3: flash attention (scores transposed, fp8
    # DoubleRow) with the output projection interleaved one q-chunk behind
    out_dt = dt_w
    mult, add = mybir.AluOpType.mult, mybir.AluOpType.add
    with (
        tc.tile_pool(name="e", bufs=4) as epool,
        tc.tile_pool(name="r", bufs=4) as rpool,
        tc.tile_pool(name="o", bufs=4) as opool,
        tc.tile_pool(name="pss", bufs=2, space="PSUM") as ps_s,
        tc.tile_pool(name="psz", bufs=4, space="PSUM") as psz,
    ):
        if stage <= 1:
            nc.sync.dma_start(out_d[0:P, :], ZTt[:, 0, 0:Dm])

        drain = [False]  # final-drain mode: outproj evictions move DVE->ACT

        def normalize(pr, qc, zA, zB):
            """ZT[:, q] = Z'[0:64, q] * (1 / l[q]); l arrives pre-broadcast
            on partitions 64:128 of the PV accumulators. DVE-only."""
            qs = slice(qc * SC, (qc + 1) * SC)
            rb = rpool.tile([64, 2, SC], f32, tag="rb")
            ls = rpool.tile([64, 2, SC], f32, tag="ls")
            nc.vector.tensor_copy(ls[:, 0, :], zA[Dh : 2 * Dh, :])
            nc.vector.tensor_copy(ls[:, 1, :], zB[Dh : 2 * Dh, :])
            nc.vector.reciprocal_approx_fast(rb[:], ls[:])
            nc.vector.tensor_mul(ZTt[0:64, pr, qs], zA[0:Dh, :], rb[:, 0, :])
            nc.vector.tensor_mul(ZTt[64:128, pr, qs], zB[0:Dh, :], rb[:, 1, :])

        def outproj_steps(qc):
            """Closures for this q-chunk's output projection, injected one at
            a time between later j-iterations to keep PE density high."""
            def step(t, dh2):
                def emit():
                    po = psz.tile([P, SC], f32, tag="z")
                    ds = slice(dh2 * SC, (dh2 + 1) * SC)
                    zs = slice(t * P, (t + 1) * P)
                    for pr in range(NPAIR):
                        nc.tensor.matmul(
                            po[:], ZTt[:, pr, zs], wo[:, pr, ds],
                            start=(pr == 0), stop=(pr == NPAIR - 1),
                        )
                    ot = opool.tile([P, SC], out_dt, tag="o")
                    # evict via DVE during flash, ACT during the final drain
                    if drain[0]:
                        nc.scalar.activation(
                            ot[:], po[:], mybir.ActivationFunctionType.Copy
                        )
                    else:
                        nc.vector.tensor_copy(ot[:], po[:])
                    nc.sync.dma_start(out_d[t * P : (t + 1) * P, ds], ot[:])
                return emit

            return [
                step(t, dh2)
                for t in range(qc * (SC // P), (qc + 1) * (SC // P))
                for dh2 in range(DH2)
            ]

        def v_step(qt):
            """One deferred V-projection group (f16): PE fill work."""
            def emit():
                psV = psz.tile([P, NH * Dh], f32, tag="z", name=f"psv_{qt}")
                for kt in range(KT):
                    nc.tensor.matmul(
                        psV[:],
                        xT[:, kt, qt * P : (qt + 1) * P],
                        wv[:, kt, :],
                        start=(kt == 0), stop=(kt == KT - 1),
                    )
                nc.vector.tensor_copy(
                    Vt[:, qt, :, 0:Dh],
                    psV[:].rearrange("p (h e) -> p h e", e=Dh),
                )
            return emit

        def qk_step(qc, pr, pj):
            """One deferred Q/K-projection group: 4 fp8 DoubleRow matmuls,
            DVE eviction to fp8 scratch, DMA-fold into Q8/K8."""
            def emit():
                ps = psz.tile([P, SC], f32, tag="z", name=f"psqk{qc}_{pr}_{pj}")
                qs = slice(qc * SC, (qc + 1) * SC)
                for kt2 in range(KT2):
                    nc.tensor.matmul(
                        ps[:], wqk8[:, :, kt2, pj, pr, :],
                        x8[:, :, kt2, qs],
                        start=(kt2 == 0), stop=(kt2 == KT2 - 1), perf_mode=DR,
                    )
                u8 = rpool.tile([P, SC], f8, tag="u8")
                nc.vector.tensor_scalar(
                    u8[:], ps[:], EVSCALE, bqk[:, pj, pr : pr + 1], mult, add
                )
                dst = Q8 if pj == 0 else K8
                for base in (0, 64):
                    for s in (0, 1):
                        nc.sync.dma_start(
                            dst[base : base + 32, s, pr, qs],
                            u8[base + 32 * s : base + 32 * s + 32, :],
                        )
            return emit

        fill_queue = []
        for qc2 in range(2, QC):
            for pr2 in range(NPAIR):
                for pj2 in range(2):
                    fill_queue.append((qc2, qk_step(qc2, pr2, pj2)))
            for qt in range(qc2 * (SC // P), (qc2 + 1) * (SC // P)):
                fill_queue.append((qc2, v_step(qt)))
        op_queue = []
        chunk_tail = None
        for qc in range(QC if stage >= 2 else 0):
            while fill_queue and fill_queue[0][0] <= qc:
                fill_queue.pop(0)[1]()
            for pr in range(NPAIR):
                hA, hB = 2 * pr, 2 * pr + 1
                zA = psz.tile([P, SC], f32, tag="z")
                zB = psz.tile([P, SC], f32, tag="z")
                jmax = (qc + 1) * (SC // P)
                pends = []  # exp->PV pipeline, depth 2

                def emit_pv(j, eAB, c0, jmax=jmax, zA=zA, zB=zB, hA=hA, hB=hB):
                    st, sp = j == 0, j == jmax - 1
                    cs = slice(c0, SC)
                    nc.tensor.matmul(
                        zA[:, cs], Vt[:, j, hA, :], eAB[:, 0, cs],
                        start=st, stop=sp,
                    )
                    nc.tensor.matmul(
                        zB[:, cs], Vt[:, j, hB, :], eAB[:, 1, cs],
                        start=st, stop=sp,
                    )

                for j in range(jmax):
                    v = j - (jmax - SC // P)
                    c0 = v * P if v > 0 else 0
                    cs = slice(c0, SC)
                    qf = slice(qc * SC + c0, (qc + 1) * SC)
                    sAB = ps_s.tile([P, 2, SC], f32, tag="s")
                    ks = slice(j * P, (j + 1) * P)
                    # fp8 DoubleRow score pair: head A rows 0:32, head B
                    # rows 64:96 (different PE quadrants -> concurrent)
                    nc.tensor.matmul(
                        sAB[:, 0, cs],
                        K8[0:32, :, pr, ks], Q8[0:32, :, pr, qf],
                        start=True, stop=True, perf_mode=DR,
                    )
                    nc.tensor.matmul(
                        sAB[:, 1, cs],
                        K8[64:96, :, pr, ks], Q8[64:96, :, pr, qf],
                        start=True, stop=True, perf_mode=DR,
                    )
                    eAB = epool.tile([P, 2, SC], dt_w, tag="e")
                    nc.scalar.activation(
                        eAB[:, :, cs], sAB[:, :, cs], Exp, scale=exp_scale
                    )
                    if v >= 0:  # chunk contains the causal diagonal
                        mv = slice(c0, min((v + 1) * P, SC))
                        nc.gpsimd.tensor_mul(
                            eAB[:, 0, mv], eAB[:, 0, mv], masks[:, v, mv]
                        )
                        nc.gpsimd.tensor_mul(
                            eAB[:, 1, mv], eAB[:, 1, mv], masks[:, v, mv]
                        )
                    if stage >= 3:
                        pends.append((j, eAB, c0))
                        if len(pends) > 3:
                            emit_pv(*pends.pop(0))
                        if j == 0 and chunk_tail is not None:
                            chunk_tail()
                            chunk_tail = None
                        elif j >= 1 and fill_queue:
                            fill_queue.pop(0)[1]()
                        elif j >= 2 and op_queue and (
                            qc == QC - 1 or j % 3 == 0
                        ):
                            op_queue.pop(0)()
                    else:
                        last_e = eAB
                if stage < 3:
                    if pr == 0 and qc == 0:
                        nc.sync.dma_start(out_d[0:P, 0:SC], last_e[:, 0, :])
                    continue

                # pr-boundary cover
                if op_queue and qc >= 1:
                    op_queue.pop(0)()

                def chunk_tail(pends=pends, pr=pr, qc=qc, zA=zA, zB=zB,
                               emit_pv=emit_pv):
                    for p in pends:
                        emit_pv(*p)
                    normalize(pr, qc, zA, zB)
                    return pr, qc, zA, zB

            if stage >= 5:
                op_queue.extend(outproj_steps(qc))
        drain[0] = True
        if chunk_tail is not None:
            pends, pr, qc, zA, zB = (chunk_tail.__defaults__[:5])
            for p in pends:
                chunk_tail.__defaults__[5](*p)
            rb = rpool.tile([64, 2, SC], f32, tag="rb")
            ls = rpool.tile([64, 2, SC], f32, tag="ls")
            for ti in range(SC // P):
                cl = slice(ti * P, (ti + 1) * P)
                qsl = slice(qc * SC + ti * P, qc * SC + (ti + 1) * P)
                nc.vector.tensor_copy(ls[:, 0, cl], zA[Dh : 2 * Dh, cl])
                nc.vector.tensor_copy(ls[:, 1, cl], zB[Dh : 2 * Dh, cl])
                nc.vector.reciprocal_approx_fast(rb[:, :, cl], ls[:, :, cl])
                nc.vector.tensor_mul(ZTt[0:64, pr, qsl], zA[0:Dh, cl], rb[:, 0, cl])
                nc.vector.tensor_mul(ZTt[64:128, pr, qsl], zB[0:Dh, cl], rb[:, 1, cl])
                for _ in range(DH2):
                    if op_queue:
                        op_queue.pop(0)()
        for step in op_queue:
            step()
        if stage == 4:
            nc.sync.dma_start(out_d[0:P, :], ZTt[:, 0, 0:Dm])


def pack_inputs(x_b, W_Q, W_K, W_V, W_O, b_Q, b_K, hds):
    """Host-side packing of one core's shard into the kernel's layouts."""
    import ml_dtypes

    f8 = ml_dtypes.float8_e4m3
    Dm, Dh = W_Q.shape[1], W_Q.shape[2]
    S = x_b.shape[0]
    NH = len(hds)
    NPAIR = NH // 2
    KT = Dm // P
    KT2 = Dm // (2 * P)

    xb = np.asarray(x_b, np.float32)
    xT = np.ascontiguousarray(
        xb.T.reshape(KT, P, S).transpose(1, 0, 2)
    ).astype(np.float16)
    # fp8 x for the QK path: d = kt2*256 + slot*128 + p
    x8 = np.ascontiguousarray(
        xb.T.reshape(KT2, 2, P, S).transpose(2, 1, 0, 3)
    ).astype(f8)

    def pack_w8(W):  # [H, Dm, Dh] -> [P, 2, KT2, NPAIR, 128] fp8 at 64x
        W4 = np.asarray(W, np.float32)[hds]  # [NH, Dm, Dh]
        t = W4.reshape(NPAIR, 2, KT2, 2, P, Dh).transpose(4, 3, 2, 0, 1, 5)
        return (WSCALE * t.reshape(P, 2, KT2, NPAIR, 2 * Dh))

    wqk8 = np.ascontiguousarray(
        np.stack([pack_w8(W_Q), pack_w8(W_K)], axis=3)  # [P,2,KT2,2,NPAIR,128]
    ).astype(f8)

    WV4 = np.asarray(W_V, np.float32)[hds]
    wv = np.ascontiguousarray(
        WV4.reshape(NH, KT, P, Dh).transpose(2, 1, 0, 3).reshape(P, KT, NH * Dh)
    ).astype(np.float16)

    WO4 = np.asarray(W_O, np.float32)[hds]
    wo = np.ascontiguousarray(
        WO4.reshape(NPAIR, 2, Dh, Dm).transpose(1, 2, 0, 3).reshape(P, NPAIR, Dm)
    ).astype(np.float16)

    def pack_b(b):  # [H, Dh] -> [P, NPAIR], pre-scaled by QSCALE
        b4 = np.asarray(b, np.float32)[hds]
        return QSCALE * b4.reshape(NPAIR, 2, Dh).transpose(1, 2, 0).reshape(P, NPAIR)

    bqk = np.ascontiguousarray(
        np.stack([pack_b(b_Q), pack_b(b_K)], axis=1)  # [P, 2, NPAIR]
    ).astype(np.float32)

    return {"xT": xT, "x8": x8, "wqk8": wqk8, "wv": wv, "wo": wo, "bqk": bqk}


def kernel(x, W_Q, W_K, W_V, W_O, b_Q, b_K, b_V, b_O, _trace=False):
    from concourse.bass_utils import run_bass_kernel_spmd

    x = np.asarray(x, np.float32)
    B, S, Dm = x.shape
    H, _, Dh = W_Q.shape
    NCORES = 8
    GB = NCORES // B        # head groups per batch element
    NH = H // GB            # heads per core

    nc = build_nc(S, Dm, NH, Dh)

    in_maps = []
    for c in range(NCORES):
        b, g = c // GB, c % GB
        hds = list(range(g * NH, (g + 1) * NH))
        in_maps.append(
            pack_inputs(x[b], W_Q, W_K, W_V, W_O, b_Q, b_K, hds)
        )

    try:
        res = run_bass_kernel_spmd(
            nc, in_maps, core_ids=list(range(NCORES)), trace=_trace
        )
    except Exception:
        # transient device hiccups usually clear on retry
        res = run_bass_kernel_spmd(
            nc, in_maps, core_ids=list(range(NCORES)), trace=_trace
        )

    out = np.zeros((B, S, Dm), np.float32)
    for c in range(NCORES):
        out[c // GB] += res.results[c]["out"]

    # biases that commute out of the device kernel (softmax rows sum to 1)
    corr = np.asarray(b_O, np.float32) + np.einsum(
        "he,hed->d",
        np.asarray(b_V, np.float32),
        np.asarray(W_O, np.float32),
    )
    out += corr[None, None, :]

    if _trace:
        kernel.last_results = res
    return out


# revision 36
# speedup vs baseline: 1.2644x; 1.1033x over previous
"""Causal multi-head attention layer for Trainium2 (Bass/Tile), 8 NeuronCores.

Problem: x[B=2,S=2048,D=1024], H=16 heads, Dh=64.
Sharding: data-parallel over batch (2) x tensor-parallel over head groups (4):
each of the 8 cores handles one batch element and 4 heads, producing a partial
output [S, D]; the host sums the 4 head-group partials per batch (the
"all-reduce after the W_O contraction" done host-side since we return full
output anyway) and adds biases that commute out (b_O and sum_h b_V[h] @ W_O[h],
exact because softmax rows sum to 1).

Device kernel (per core), all operands resident in SBUF:
  - x^T is fed pre-transposed from host: [128, KT=8, S] (D on partitions).
  - Q^T, K^T computed head-PAIR-packed: [128, NPAIR, S] (partitions 0:64 =
    head 2*pr dims, 64:128 = head 2*pr+1). W as stationary [128,128], x^T
    moving N=512.
  - V computed in [k, e] layout (x^T stationary, W_V moving N=256, all 4
    heads at once) and stored with an appended [1, 0] column pair: V'=[V|1|0].
  - Scores computed TRANSPOSED: S^T[k, q] = (K^T tile).T @ Q^T chunk, so
    softmax's sum lands on the matmul contraction instead of needing row
    reductions: Z'[e|1|0, q] = V'.T @ exp(S^T) accumulated over k-tiles gives
    both the unnormalized attention output (rows 0:64) and the softmax
    denominator l (row 64) in one accumulation. No max-subtraction is needed:
    scores are O(1) here, exp is safe in fp32.
  - Both heads of a pair write one 2-bank PSUM tile (disjoint PE row groups,
    so their K=64 matmuls run concurrently) and share a single 1024-wide
    ACTIVATE(Exp) to amortize the ~352-cycle ACT fixed cost.
  - Causal masking is multiplicative on exp(S^T), diagonal chunks only (on
    GpSimd, which is otherwise idle); fully-masked column ranges of diagonal
    chunks are skipped in the scores/exp/PV instructions.
  - The ones block of V' is replicated 64x, so l lands pre-broadcast on
    PV-accumulator partitions 64:128 and normalization is a wide DVE
    reciprocal_approx_fast + multiply — no cross-partition traffic. (The
    approx reciprocal must read the multi-matmul PSUM accumulation via an
    SBUF staging copy; reading PSUM directly returns garbage on HW.)
  - The kernel is PE-throughput-bound end to end (~91% Tensor busy in the
    flash region), so everything revolves around keeping the PE stream
    dense and dependency-free:
      * Phase 1 computes only the first two q-chunks' Q/K projections
        (8 PSUM groups fed ktile-by-ktile as the x^T DMA lands) and the
        first 8 V tiles; the rest of the Q/K and V projections ride inside
        the flash loop as deadline-ordered PE fill work (fill_queue), so
        the ACT exp stream starts ~25us earlier.
      * exp→PV runs at pipeline depth 2 (pends): the PV consuming exp(j)
        is emitted at j+2, so its ACT/GpSimd semaphores are long-satisfied
        and the PE never stalls on the hop (-11us vs depth 1).
      * The output projection (single K=128 matmuls per head pair — the
        pair-sum rides the contraction) is METERED (every 3rd j) through
        the middle chunks so a backlog of real PE work survives into the
        last chunk, whose own scores+PV underfill the ACT-paced loop; this
        replaced the old dummy filler matmuls and keeps the HAM clock-gate
        at 8/8 through the whole flash region without burning power budget.
      * Out-proj PSUM is evicted on DVE during flash (ACT paces the exp
        stream there) but on ACT during the final drain (ACT is idle then,
        DVE runs the normalize chains); the eviction casts to f16 so the
        out DMA traffic halves (host accumulates partials in f32).
  - Dummy warm-up matmuls run during the initial DMA load to ramp the PE
    p-state; input DMAs are interleaved ktile-by-ktile in first-use order
    (bqk first: it gates the first Q/K evictions and thus the flash start).
  - CAUTION: instruction *timings* here are extremely sensitive to SBUF
    tile layout. Innocuous-looking changes that shift pool allocations
    (adding a tile, growing a pool's bufs) have reproducibly slowed EVERY
    engine's instructions ~20% (SBUF port contention). Keep changes
    allocation-neutral or A/B against the previous layout.
"""

import os
import numpy as np

# 'f16'   = float16 operands: 2-byte moving operand streams at 1 PE
#           cycle/row (4-byte fp32/fp32r cost 2), 11-bit mantissa
# 'fp32r' = fp32 bits, single-pass reduced-precision PE mode (2 cyc/row)
# 'bf16'  = bf16 storage/matmuls (1 cyc/row, 8-bit mantissa)
# 'fp32'  = exact fp32 matmuls (two-pass, 4 cyc/row)
MM_MODE = os.environ.get("ATTN_MM_MODE", "f16")

P = 128
SC = 512  # q-chunk width (one PSUM bank of fp32)

# The Q/K PROJECTIONS run in fp8e4m3 with MatmulPerfMode.DoubleRow (2 fp8
# MACs/PE-cell/cycle): a projection group is 4 K_eff=256 DR matmuls instead
# of 8 f16 ones — measured 2x. Weights are host-scaled by WSCALE before fp8
# quantization (0.02-std weights would land in e4m3 denormals); the PSUM
# holds WSCALE*q and the f16 eviction applies 1/WSCALE (+bias), so Q/K in
# SBUF and everything downstream (f16 scores, exp scale) are unchanged.
# Score-path fp8 error averages out through softmax: rel_absmax ~1.0e-2 vs
# the 2e-2 gate (host-sim; value path in fp8 would fail at ~4e-2, so V/PV/
# out-proj stay f16). With the projections 2x cheaper, ALL FOUR q-chunks'
# QK projections fit in phase 1 (two 8-PSUM-bank waves) and the in-flash
# qk fill steps disappear — that removes ~16k ns of f16 projection work
# from the PE-bound flash region.
WSCALE = 64.0
EVSCALE = 1.0 / WSCALE

_BUILD_CACHE = {}


def _np_sb(mm_mode):
    if mm_mode == "bf16":
        import ml_dtypes

        return np.dtype(ml_dtypes.bfloat16)
    if mm_mode == "f16":
        return np.dtype(np.float16)
    return np.dtype(np.float32)


def build_nc(S, Dm, NH, Dh, mm_mode, stage=99):
    """Build (and cache) the per-core Bass module. NH = heads per core."""
    key = (S, Dm, NH, Dh, mm_mode, stage)
    if key in _BUILD_CACHE:
        return _BUILD_CACHE[key]

    import concourse.bacc as bacc
    import concourse.mybir as mybir
    import concourse.tile as tile

    f32 = mybir.dt.float32
    # dt_w: dtype of every matmul operand. float32r data is fp32 bits that the
    # PE consumes in a single-pass reduced-precision mode; the BIR verifier
    # requires every fp32r matmul operand to be *produced* with float32r dtype
    # (DMA pass-through from a float32r DRAM tensor, or a compute-engine
    # write; memset cannot produce it).
    dt_w = {
        "bf16": mybir.dt.bfloat16,
        "f16": mybir.dt.float16,
        "fp32": mybir.dt.float32,
        "fp32r": mybir.dt.float32r,
    }[mm_mode]
    # dtype for non-matmul elementwise tiles (masks)
    dt_m = {
        "bf16": mybir.dt.bfloat16,
        "f16": mybir.dt.float16,
    }.get(mm_mode, mybir.dt.float32)

    KT = Dm // P       # k-tiles over the model dim (contraction of projections)
    NPAIR = NH // 2    # head pairs
    QC = S // SC       # q chunks
    NKT = S // P       # k-position tiles
    DH2 = Dm // SC     # output free-dim chunks
    assert Dh == 64 and NH % 2 == 0 and S % SC == 0 and Dm % SC == 0

    nc = bacc.Bacc(
        "TRN2",
        debug=False,
        enable_asserts=False,
        target_bir_lowering=False,
        num_devices=1,
    )

    f8 = mybir.dt.float8e4
    DR = mybir.MatmulPerfMode.DoubleRow
    KT2 = Dm // (2 * P)  # fp8 DoubleRow k-tiles (256 contraction each)

    xT_d = nc.dram_tensor("xT", [P, KT, S], dt_w, kind="ExternalInput")
    x8_d = nc.dram_tensor("x8", [P, 2, KT2, S], f8, kind="ExternalInput")
    wqk8_d = nc.dram_tensor(
        "wqk8", [P, 2, KT2, 2, NPAIR, P], f8, kind="ExternalInput"
    )
    wv_d = nc.dram_tensor("wv", [P, KT, NH * Dh], dt_w, kind="ExternalInput")
    wo_d = nc.dram_tensor("wo", [P, NPAIR, Dm], dt_w, kind="ExternalInput")
    bqk_d = nc.dram_tensor("bqk", [P, 2, NPAIR], f32, kind="ExternalInput")
    # output in the 2-byte matmul dtype (halves the output DMA traffic; the
    # host accumulates head-group partials in f32, so only one rounding)
    dt_out = dt_w if mybir.dt.size(dt_w) == 2 else f32
    out_d = nc.dram_tensor("out", [S, Dm], dt_out, kind="ExternalOutput")

    def mm(ap):
        return ap

    Exp = mybir.ActivationFunctionType.Exp
    inv_sqrt_dh = 1.0 / float(np.sqrt(Dh))

    with tile.TileContext(nc) as tc:
        with tc.tile_pool(name="const", bufs=1) as cpool:
            # ---------- constants (DMAs emitted inside phase 1, ordered by
            # first use, so the PE starts after ~2 ktiles instead of the
            # whole 16MB input load) ----------
            wv = cpool.tile([P, KT, NH * Dh], dt_w)
            wo = cpool.tile([P, NPAIR, Dm], dt_w)
            bqk = cpool.tile([P, 2, NPAIR], f32)

            QTt = cpool.tile([P, NPAIR, S], dt_w)
            KTt = cpool.tile([P, NPAIR, S], dt_w)
            # V' = [V | 1...1]: the ones block is REPLICATED 64x so the PV
            # matmul broadcasts the softmax denominator l across output
            # partitions 64:128 (M=128 costs the same N cycles as M=65, and
            # 64-partition l lets the reciprocal run wide on DVE).
            Vt = cpool.tile([P, NKT, NH, 2 * Dh], dt_w)

            # causal masks for the 4 diagonal-chunk variants: keep (1.0) where
            # q >= k + v*128, else 0.0 (S^T layout: partition=k, free=q).
            # Built first, in a never-reused pool, so the GpSimd work (and its
            # library load) happens during the initial DMA wait.
            masks = cpool.tile([P, SC // P, SC], dt_m)
            nc.gpsimd.memset(masks[:], 1.0)
            for v in range(SC // P):
                nc.gpsimd.affine_select(
                    out=masks[:, v, :],
                    in_=masks[:, v, :],
                    compare_op=mybir.AluOpType.is_ge,
                    fill=0.0,
                    base=-(v * P),
                    pattern=[[1, SC]],
                    channel_multiplier=-1,
                )

            # ---------- phase 1: projections for the first two q-chunks
            # only; chunks 2..QC-1 are deferred into the flash loop as PE
            # fill work (so the ACT exp stream starts ~25us earlier) ----------
            with (
                tc.tile_pool(name="p1", bufs=1) as p1pool,
                tc.tile_pool(name="ps1", bufs=8, space="PSUM") as ps1,
            ):
                wqk8 = cpool.tile([P, 2, KT2, 2, NPAIR, P], f8)
                x8 = cpool.tile([P, 2, KT2, S], f8)
                xT = cpool.tile([P, KT, S], dt_w)  # outlives phase 1: the
                # deferred V-projection groups read it inside the flash loop
                # fp8 QK stream first (it arrival-paces the projection
                # waves and is only 2.6MB), then the f16 x for the V path.
                # NOTE: issuing wv/wo early on the parallel Scalar HWDGE
                # queue measured WORSE — they steal HBM bandwidth from the
                # xT stream exactly while the QK matmuls are arrival-paced.
                nc.sync.dma_start(bqk[:], bqk_d[:])
                for kt2 in range(KT2):
                    nc.sync.dma_start(wqk8[:, :, kt2], wqk8_d[:, :, kt2])
                    nc.sync.dma_start(x8[:, :, kt2, :], x8_d[:, :, kt2, :])
                for kt in range(KT):
                    nc.sync.dma_start(xT[:, kt, :], xT_d[:, kt, :])
                    if kt == KT // 2 - 1:
                        nc.sync.dma_start(wv[:], wv_d[:])
                nc.sync.dma_start(wo[:], wo_d[:])

                # HAM warm-up: dummy matmuls during the initial DMA wait so
                # the PE clock-gate is at 8/8 when real work arrives
                wst = p1pool.tile([P, SC], f32)
                nc.vector.memset(wst[:], 1.0)
                # preload the Exp table on the Scalar engine now (it's idle);
                # otherwise the first flash exp pays the ~1.3us table load
                # on the critical path
                tpre = p1pool.tile([1, 2], f32)
                nc.scalar.activation(tpre[:], wst[0:1, 0:2], Exp)
                wrm = p1pool.tile([P, SC], dt_w)
                nc.vector.tensor_copy(wrm[:], wst[:])
                nwu = 6 if dt_w is mybir.dt.float32r else 10
                pwu = ps1.tile([P, SC], f32, tag="mm")
                for i in range(nwu):
                    nc.tensor.matmul(
                        pwu[:], mm(wrm[:, 0:P]), mm(wrm[:]),
                        start=(i == 0), stop=(i == nwu - 1),
                    )

                # Q/K projections for ALL q-chunks (fp8 DoubleRow, 2x f16
                # throughput): two waves of 8 PSUM groups; each wave is fed
                # kt2-by-kt2 so the arriving x8 stream feeds 8 matmuls per
                # tile (the first pass is DMA-paced and would otherwise
                # leave the PE idle, re-throttling the clock-gate)
                for qg in range(0, QC, 2):
                    qcs = list(range(qg, min(qg + 2, QC)))
                    pss = {
                        (pr, pj, qc): ps1.tile(
                            [P, SC], f32, tag="mm", name=f"psqk_{pr}_{pj}_{qc}"
                        )
                        for pr in range(NPAIR)
                        for pj in range(2)
                        for qc in qcs
                    }
                    for kt2 in range(KT2):
                        st, sp = kt2 == 0, kt2 == KT2 - 1
                        for pr in range(NPAIR):
                            for pj in range(2):
                                for qc in qcs:
                                    xs = x8[:, :, kt2, qc * SC : (qc + 1) * SC]
                                    nc.tensor.matmul(
                                        pss[(pr, pj, qc)][:],
                                        wqk8[:, :, kt2, pj, pr, :], xs,
                                        start=st, stop=sp, perf_mode=DR,
                                    )
                    for pr in range(NPAIR):
                        for qc in qcs:
                            qs1 = slice(qc * SC, (qc + 1) * SC)
                            # evict via ACT (idle during phase 1; Identity
                            # shares the preloaded Exp table so no table
                            # reload) — keeps DVE free for the V-tile CASTs
                            # so the V groups' PSUM recycles sooner. The
                            # 1/WSCALE undoes the host-side fp8 weight scale.
                            nc.scalar.activation(
                                QTt[:, pr, qs1], pss[(pr, 0, qc)][:],
                                mybir.ActivationFunctionType.Identity,
                                bias=bqk[:, 0, pr : pr + 1], scale=EVSCALE,
                            )
                            nc.scalar.activation(
                                KTt[:, pr, qs1], pss[(pr, 1, qc)][:],
                                mybir.ActivationFunctionType.Identity,
                                bias=bqk[:, 1, pr : pr + 1], scale=EVSCALE,
                            )

                # only the V tiles the first two flash chunks touch; the rest
                # are deferred into the flash loop as PE fill work
                for qt in range(min(2 * (SC // P), NKT)):
                    psV = ps1.tile([P, NH * Dh], f32, tag="mm")
                    for kt in range(KT):
                        nc.tensor.matmul(
                            psV[:],
                            mm(xT[:, kt, qt * P : (qt + 1) * P]),
                            mm(wv[:, kt, :]),
                            start=(kt == 0), stop=(kt == KT - 1),
                        )
                    nc.vector.tensor_copy(
                        Vt[:, qt, :, 0:Dh],
                        psV[:].rearrange("p (h e) -> p h e", e=Dh),
                    )

                # memset can't write float32r: stage the V' ones in f32, copy
                # over with a free-dim broadcast (needed first by the PV
                # matmuls in phase 2)
                cstage = p1pool.tile([P, 1, 1, Dh], f32)
                nc.vector.memset(cstage[:], 1.0)
                nc.vector.tensor_copy(
                    Vt[:, :, :, Dh : 2 * Dh],
                    cstage[:].to_broadcast((P, NKT, NH, Dh)),
                )

            # ---------- phases 2+3 ----------
            with tc.tile_pool(name="zt", bufs=1) as ztpool:
                ZTt = ztpool.tile([P, NPAIR, S], dt_w)
                self_flash(
                    nc, tc, stage, mm, Exp, inv_sqrt_dh, mybir,
                    QTt, KTt, Vt, ZTt, wo, out_d, masks, xT, wv, None, None,
                    S, Dm, Dh, NPAIR, QC, SC, P, DH2, KT, NKT, dt_w, dt_m, f32,
                )

    nc.compile()
    _BUILD_CACHE[key] = nc
    return nc


def self_flash(
    nc, tc, stage, mm, Exp, inv_sqrt_dh, mybir,
    QTt, KTt, Vt, ZTt, wo, out_d, masks, xT, wv, wqk, bqk,
    S, Dm, Dh, NPAIR, QC, SC, P, DH2, KT, NKT, dt_w, dt_m, f32,
):
    # ---------- phases 2+3: flash attention (scores transposed) with the
    # output projection interleaved one q-chunk behind ----------
    out_dt = dt_w if mybir.dt.size(dt_w) == 2 else f32
    with (
        tc.tile_pool(name="e", bufs=4) as epool,
        tc.tile_pool(name="r", bufs=4) as rpool,
        tc.tile_pool(name="o", bufs=4) as opool,
        tc.tile_pool(name="pss", bufs=2, space="PSUM") as ps_s,
        tc.tile_pool(name="psz", bufs=4, space="PSUM") as psz,
    ):
        if stage <= 1:
            nc.sync.dma_start(out_d[0:P, :], QTt[:, 0, 0:Dm])

        drain = [False]  # final-drain mode: outproj evictions move DVE→ACT

        def normalize(pr, qc, zA, zB):
            """ZT[:, q] = Z'[0:64, q] * (1 / l[q]); l arrives pre-broadcast
            on partitions 64:128 of the PV accumulators. DVE-only.
            (reciprocal_approx_fast must not read multi-matmul PSUM
            accumulations directly — stage l through SBUF first.)"""
            qs = slice(qc * SC, (qc + 1) * SC)
            rb = rpool.tile([64, 2, SC], f32, tag="rb")
            ls = rpool.tile([64, 2, SC], f32, tag="ls")
            nc.vector.tensor_copy(ls[:, 0, :], zA[Dh : 2 * Dh, :])
            nc.vector.tensor_copy(ls[:, 1, :], zB[Dh : 2 * Dh, :])
            nc.vector.reciprocal_approx_fast(rb[:], ls[:])
            nc.vector.tensor_mul(ZTt[0:64, pr, qs], zA[0:Dh, :], rb[:, 0, :])
            nc.vector.tensor_mul(ZTt[64:128, pr, qs], zB[0:Dh, :], rb[:, 1, :])

        def outproj_steps(qc):
            """Closures for this q-chunk's output projection, injected one at
            a time between later j-iterations to keep PE density high.
            out[q, d] = sum_h Z_h[q, :] @ W_O[h]; each K=128 matmul sums a
            head pair inside the contraction."""
            def step(t, dh2):
                def emit():
                    po = psz.tile([P, SC], f32, tag="z")
                    ds = slice(dh2 * SC, (dh2 + 1) * SC)
                    zs = slice(t * P, (t + 1) * P)
                    for pr in range(NPAIR):
                        nc.tensor.matmul(
                            po[:], mm(ZTt[:, pr, zs]), mm(wo[:, pr, ds]),
                            start=(pr == 0), stop=(pr == NPAIR - 1),
                        )
                    ot = opool.tile([P, SC], out_dt, tag="o")
                    # evict via DVE during flash (GpSimd cannot read PSUM;
                    # keeping this off the Scalar engine frees the exp stream
                    # that paces flash), but via ACT during the final drain:
                    # ACT is idle there and the DVE FIFO must stay clear for
                    # the per-tile normalize slices that gate these very
                    # steps (alternating engines here measured WORSE).
                    # Casting to the 2-byte output dtype halves the out DMA.
                    if drain[0]:
                        nc.scalar.activation(
                            ot[:], po[:], mybir.ActivationFunctionType.Copy
                        )
                    else:
                        nc.vector.tensor_copy(ot[:], po[:])
                    nc.sync.dma_start(out_d[t * P : (t + 1) * P, ds], ot[:])
                return emit

            return [
                step(t, dh2)
                for t in range(qc * (SC // P), (qc + 1) * (SC // P))
                for dh2 in range(DH2)
            ]

        def v_step(qt):
            """One deferred V-projection group: pure PE fill work for the
            flash loop. Must run before the chunk that reads Vt[qt]
            (qt tiles 4k..4k+3 are consumed first by q-chunk k)."""
            def emit():
                psV = psz.tile([P, NH * Dh], f32, tag="z", name=f"psv_{qt}")
                for kt in range(KT):
                    nc.tensor.matmul(
                        psV[:],
                        mm(xT[:, kt, qt * P : (qt + 1) * P]),
                        mm(wv[:, kt, :]),
                        start=(kt == 0), stop=(kt == KT - 1),
                    )
                nc.vector.tensor_copy(
                    Vt[:, qt, :, 0:Dh],
                    psV[:].rearrange("p (h e) -> p h e", e=Dh),
                )
            return emit

        NH = Vt.shape[2]
        # deadline-ordered fill work: (need-by-chunk, emit). All QK
        # projections now run in phase 1 (fp8 DR made them 2x cheaper), so
        # only the deferred V-projection groups ride the flash loop.
        fill_queue = []
        for qc2 in range(2, QC):
            for qt in range(qc2 * (SC // P), (qc2 + 1) * (SC // P)):
                fill_queue.append((qc2, v_step(qt)))
        op_queue = []
        chunk_tail = None
        for qc in range(QC if stage >= 2 else 0):
            # deadline safety net: any fill this chunk depends on that the
            # in-loop pops didn't get to yet runs now, up front
            while fill_queue and fill_queue[0][0] <= qc:
                fill_queue.pop(0)[1]()
            for pr in range(NPAIR):
                hA, hB = 2 * pr, 2 * pr + 1
                zA = psz.tile([P, SC], f32, tag="z")
                zB = psz.tile([P, SC], f32, tag="z")
                jmax = (qc + 1) * (SC // P)
                pends = []  # exp→PV pipeline, depth 2: the PV consuming
                # exp(j) is emitted at iteration j+2, so its semaphores are
                # long-satisfied and the PE never stalls on the ACT/GpSimd hop

                def emit_pv(j, eAB, c0, jmax=jmax, zA=zA, zB=zB, hA=hA, hB=hB):
                    st, sp = j == 0, j == jmax - 1
                    cs = slice(c0, SC)
                    nc.tensor.matmul(
                        zA[:, cs], mm(Vt[:, j, hA, :]), mm(eAB[:, 0, cs]),
                        start=st, stop=sp,
                    )
                    nc.tensor.matmul(
                        zB[:, cs], mm(Vt[:, j, hB, :]), mm(eAB[:, 1, cs]),
                        start=st, stop=sp,
                    )

                for j in range(jmax):
                    v = j - (jmax - SC // P)
                    # causal: columns below the diagonal tile are fully
                    # masked; skip them (fp32r moving dims must stay >= 256;
                    # 2-byte dtypes can slice all the way down)
                    if dt_w is mybir.dt.float32r:
                        c0 = min(v * P, 2 * P) if v > 0 else 0
                    else:
                        c0 = v * P if v > 0 else 0
                    cs = slice(c0, SC)
                    qf = slice(qc * SC + c0, (qc + 1) * SC)
                    sAB = ps_s.tile([P, 2, SC], f32, tag="s")
                    ks = slice(j * P, (j + 1) * P)
                    nc.tensor.matmul(
                        sAB[:, 0, cs],
                        mm(KTt[0:64, pr, ks]), mm(QTt[0:64, pr, qf]),
                        start=True, stop=True,
                    )
                    nc.tensor.matmul(
                        sAB[:, 1, cs],
                        mm(KTt[64:128, pr, ks]), mm(QTt[64:128, pr, qf]),
                        start=True, stop=True,
                    )
                    eAB = epool.tile([P, 2, SC], dt_w, tag="e")
                    nc.scalar.activation(
                        eAB[:, :, cs], sAB[:, :, cs], Exp, scale=inv_sqrt_dh
                    )
                    if v >= 0:  # chunk contains the causal diagonal
                        mv = slice(c0, min((v + 1) * P, SC))
                        nc.gpsimd.tensor_mul(
                            eAB[:, 0, mv], eAB[:, 0, mv], masks[:, v, mv]
                        )
                        nc.gpsimd.tensor_mul(
                            eAB[:, 1, mv], eAB[:, 1, mv], masks[:, v, mv]
                        )
                    if stage >= 3:
                        pends.append((j, eAB, c0))
                        if len(pends) > 3:
                            emit_pv(*pends.pop(0))
                        if j == 0 and chunk_tail is not None:
                            # cross-chunk pipeline: the previous chunk's last
                            # PV + normalize go here, AFTER this chunk's first
                            # scores pair is queued, so the exp stream never
                            # stalls at a chunk boundary
                            chunk_tail()
                            chunk_tail = None
                        elif j >= 1 and fill_queue:
                            fill_queue.pop(0)[1]()
                        elif j >= 2 and op_queue and (
                            qc == QC - 1 or j % 3 == 0
                        ):
                            # meter the out-proj drip in middle chunks so a
                            # backlog of real PE work survives into the last
                            # chunk, whose own PE work (scores+PV) underfills
                            # the ACT-paced loop — deliberate fill, replacing
                            # the old dummy filler matmuls
                            op_queue.pop(0)()
                    else:
                        last_e = eAB
                if stage < 3:
                    if pr == 0 and qc == 0:
                        nc.sync.dma_start(out_d[0:P, 0:SC], last_e[:, 0, :])
                    continue

                # pr-boundary cover: the next pair's first scores matmul
                # waits ~1us for ACT to drain this pair's last exps (ps_s
                # buffer release); give the PE an independent out-proj step
                # here so it works through that window instead of stalling
                if op_queue and qc >= 1:
                    op_queue.pop(0)()

                def chunk_tail(pends=pends, pr=pr, qc=qc, zA=zA, zB=zB,
                               emit_pv=emit_pv):
                    for p in pends:
                        emit_pv(*p)
                    normalize(pr, qc, zA, zB)
                    return pr, qc, zA, zB

            if stage >= 5:
                op_queue.extend(outproj_steps(qc))
        drain[0] = True
        if chunk_tail is not None:
            # final drain, pipelined: slice the last chunk's normalize per
            # q-tile and interleave that tile's out-proj steps, so the PE
            # overlaps the DVE multiplies instead of waiting for the full
            # chunk-width normalize
            pends, pr, qc, zA, zB = (chunk_tail.__defaults__[:5])
            for p in pends:
                chunk_tail.__defaults__[5](*p)
            rb = rpool.tile([64, 2, SC], f32, tag="rb")
            ls = rpool.tile([64, 2, SC], f32, tag="ls")
            for ti in range(SC // P):
                # whole DVE chain sliced per q-tile: each tile's out-proj
                # matmuls overlap the next tile's copies/reciprocal
                cl = slice(ti * P, (ti + 1) * P)
                qsl = slice(qc * SC + ti * P, qc * SC + (ti + 1) * P)
                nc.vector.tensor_copy(ls[:, 0, cl], zA[Dh : 2 * Dh, cl])
                nc.vector.tensor_copy(ls[:, 1, cl], zB[Dh : 2 * Dh, cl])
                nc.vector.reciprocal_approx_fast(rb[:, :, cl], ls[:, :, cl])
                nc.vector.tensor_mul(ZTt[0:64, pr, qsl], zA[0:Dh, cl], rb[:, 0, cl])
                nc.vector.tensor_mul(ZTt[64:128, pr, qsl], zB[0:Dh, cl], rb[:, 1, cl])
                for _ in range(DH2):
                    if op_queue:
                        op_queue.pop(0)()
        for step in op_queue:
            step()
        if stage == 4:
            nc.sync.dma_start(out_d[0:P, :], ZTt[:, 0, 0:Dm])


def pack_inputs(x_b, W_Q, W_K, W_V, W_O, b_Q, b_K, hds, mm_mode):
    """Host-side packing of one core's shard into the kernel's layouts."""
    import ml_dtypes

    f8np = ml_dtypes.float8_e4m3
    npdt = _np_sb(mm_mode)
    Dm, Dh = W_Q.shape[1], W_Q.shape[2]
    S = x_b.shape[0]
    NH = len(hds)
    NPAIR = NH // 2
    KT = Dm // P
    KT2 = Dm // (2 * P)

    xb = np.asarray(x_b, np.float32)
    xT = np.ascontiguousarray(
        xb.T.reshape(KT, P, S).transpose(1, 0, 2)
    ).astype(npdt)
    # fp8 x for the QK projections: d = kt2*256 + slot*128 + p
    x8 = np.ascontiguousarray(
        xb.T.reshape(KT2, 2, P, S).transpose(2, 1, 0, 3)
    ).astype(f8np)

    def pack_w8(W):  # [H, Dm, Dh] -> [P, 2, KT2, NPAIR, 2*Dh] fp8 @ WSCALE
        W4 = np.asarray(W, np.float32)[hds]  # [NH, Dm, Dh]
        t = W4.reshape(NPAIR, 2, KT2, 2, P, Dh).transpose(4, 3, 2, 0, 1, 5)
        return WSCALE * t.reshape(P, 2, KT2, NPAIR, 2 * Dh)

    wqk8 = np.ascontiguousarray(
        np.stack([pack_w8(W_Q), pack_w8(W_K)], axis=3)  # [P,2,KT2,2,NPAIR,128]
    ).astype(f8np)

    WV4 = np.asarray(W_V)[hds]  # [NH, Dm, Dh]
    wv = np.ascontiguousarray(
        WV4.reshape(NH, KT, P, Dh).transpose(2, 1, 0, 3).reshape(P, KT, NH * Dh)
    ).astype(npdt)

    WO4 = np.asarray(W_O)[hds]  # [NH, Dh, Dm]
    wo = np.ascontiguousarray(
        WO4.reshape(NPAIR, 2, Dh, Dm).transpose(1, 2, 0, 3).reshape(P, NPAIR, Dm)
    ).astype(npdt)

    def pack_b(b):  # [H, Dh] -> [P, NPAIR]
        b4 = np.asarray(b)[hds]
        return b4.reshape(NPAIR, 2, Dh).transpose(1, 2, 0).reshape(P, NPAIR)

    bqk = np.ascontiguousarray(
        np.stack([pack_b(b_Q), pack_b(b_K)], axis=1)  # [P, 2, NPAIR]
    ).astype(np.float32)

    return {"xT": xT, "x8": x8, "wqk8": wqk8, "wv": wv, "wo": wo, "bqk": bqk}


def kernel(x, W_Q, W_K, W_V, W_O, b_Q, b_K, b_V, b_O, _trace=False):
    from concourse.bass_utils import run_bass_kernel_spmd

    x = np.asarray(x, np.float32)
    B, S, Dm = x.shape
    H, _, Dh = W_Q.shape
    NCORES = 8
    GB = NCORES // B        # head groups per batch element
    NH = H // GB            # heads per core

    nc = build_nc(S, Dm, NH, Dh, MM_MODE)

    in_maps = []
    for c in range(NCORES):
        b, g = c // GB, c % GB
        hds = list(range(g * NH, (g + 1) * NH))
        in_maps.append(
            pack_inputs(x[b], W_Q, W_K, W_V, W_O, b_Q, b_K, hds, MM_MODE)
        )

    try:
        res = run_bass_kernel_spmd(
            nc, in_maps, core_ids=list(range(NCORES)), trace=_trace
        )
    except Exception:
        # transient device hiccups (e.g. a wedged core from a previous run)
        # usually clear on retry
        res = run_bass_kernel_spmd(
            nc, in_maps, core_ids=list(range(NCORES)), trace=_trace
        )

    out = np.zeros((B, S, Dm), np.float32)
    for c in range(NCORES):
        out[c // GB] += res.results[c]["out"]

    # biases that commute out of the device kernel (softmax rows sum to 1)
    corr = np.asarray(b_O, np.float32) + np.einsum(
        "he,hed->d",
        np.asarray(b_V, np.float32),
        np.asarray(W_O, np.float32),
    )
    out += corr[None, None, :]

    if _trace:
        kernel.last_results = res
    return out



# revision 37
# speedup vs baseline: 1.2741x; 1.0076x over previous
"""Causal multi-head attention layer for Trainium2 (Bass/Tile), 8 NeuronCores.

Problem: x[B=2,S=2048,D=1024], H=16 heads, Dh=64.
Sharding: data-parallel over batch (2) x tensor-parallel over head groups (4):
each of the 8 cores handles one batch element and 4 heads, producing a partial
output [S, D]; the host sums the 4 head-group partials per batch (the
"all-reduce after the W_O contraction" done host-side since we return full
output anyway) and adds biases that commute out (b_O and sum_h b_V[h] @ W_O[h],
exact because softmax rows sum to 1).

Device kernel (per core), all operands resident in SBUF:
  - x^T is fed pre-transposed from host: [128, KT=8, S] (D on partitions).
  - Q^T, K^T computed head-PAIR-packed: [128, NPAIR, S] (partitions 0:64 =
    head 2*pr dims, 64:128 = head 2*pr+1). W as stationary [128,128], x^T
    moving N=512.
  - V computed in [k, e] layout (x^T stationary, W_V moving N=256, all 4
    heads at once) and stored with an appended [1, 0] column pair: V'=[V|1|0].
  - Scores computed TRANSPOSED: S^T[k, q] = (K^T tile).T @ Q^T chunk, so
    softmax's sum lands on the matmul contraction instead of needing row
    reductions: Z'[e|1|0, q] = V'.T @ exp(S^T) accumulated over k-tiles gives
    both the unnormalized attention output (rows 0:64) and the softmax
    denominator l (row 64) in one accumulation. No max-subtraction is needed:
    scores are O(1) here, exp is safe in fp32.
  - Both heads of a pair write one 2-bank PSUM tile (disjoint PE row groups,
    so their K=64 matmuls run concurrently) and share a single 1024-wide
    ACTIVATE(Exp) to amortize the ~352-cycle ACT fixed cost.
  - Causal masking is multiplicative on exp(S^T), diagonal chunks only (on
    GpSimd, which is otherwise idle); fully-masked column ranges of diagonal
    chunks are skipped in the scores/exp/PV instructions.
  - The ones block of V' is replicated 64x, so l lands pre-broadcast on
    PV-accumulator partitions 64:128 and normalization is a wide DVE
    reciprocal_approx_fast + multiply — no cross-partition traffic. (The
    approx reciprocal must read the multi-matmul PSUM accumulation via an
    SBUF staging copy; reading PSUM directly returns garbage on HW.)
  - The kernel is PE-throughput-bound end to end (~91% Tensor busy in the
    flash region), so everything revolves around keeping the PE stream
    dense and dependency-free:
      * Phase 1 computes only the first two q-chunks' Q/K projections
        (8 PSUM groups fed ktile-by-ktile as the x^T DMA lands) and the
        first 8 V tiles; the rest of the Q/K and V projections ride inside
        the flash loop as deadline-ordered PE fill work (fill_queue), so
        the ACT exp stream starts ~25us earlier.
      * exp→PV runs at pipeline depth 2 (pends): the PV consuming exp(j)
        is emitted at j+2, so its ACT/GpSimd semaphores are long-satisfied
        and the PE never stalls on the hop (-11us vs depth 1).
      * The output projection (single K=128 matmuls per head pair — the
        pair-sum rides the contraction) is METERED (every 3rd j) through
        the middle chunks so a backlog of real PE work survives into the
        last chunk, whose own scores+PV underfill the ACT-paced loop; this
        replaced the old dummy filler matmuls and keeps the HAM clock-gate
        at 8/8 through the whole flash region without burning power budget.
      * Out-proj PSUM is evicted on DVE during flash (ACT paces the exp
        stream there) but on ACT during the final drain (ACT is idle then,
        DVE runs the normalize chains); the eviction casts to f16 so the
        out DMA traffic halves (host accumulates partials in f32).
  - Dummy warm-up matmuls run during the initial DMA load to ramp the PE
    p-state; input DMAs are interleaved ktile-by-ktile in first-use order
    (bqk first: it gates the first Q/K evictions and thus the flash start).
  - CAUTION: instruction *timings* here are extremely sensitive to SBUF
    tile layout. Innocuous-looking changes that shift pool allocations
    (adding a tile, growing a pool's bufs) have reproducibly slowed EVERY
    engine's instructions ~20% (SBUF port contention). Keep changes
    allocation-neutral or A/B against the previous layout.
"""

import os
import numpy as np

# 'f16'   = float16 operands: 2-byte moving operand streams at 1 PE
#           cycle/row (4-byte fp32/fp32r cost 2), 11-bit mantissa
# 'fp32r' = fp32 bits, single-pass reduced-precision PE mode (2 cyc/row)
# 'bf16'  = bf16 storage/matmuls (1 cyc/row, 8-bit mantissa)
# 'fp32'  = exact fp32 matmuls (two-pass, 4 cyc/row)
MM_MODE = os.environ.get("ATTN_MM_MODE", "f16")

P = 128
SC = 512  # q-chunk width (one PSUM bank of fp32)

# The Q/K PROJECTIONS run in fp8e4m3 with MatmulPerfMode.DoubleRow (2 fp8
# MACs/PE-cell/cycle): a projection group is 4 K_eff=256 DR matmuls instead
# of 8 f16 ones — measured 2x. Weights are host-scaled by WSCALE before fp8
# quantization (0.02-std weights would land in e4m3 denormals); the PSUM
# holds WSCALE*q and the f16 eviction applies 1/WSCALE (+bias), so Q/K in
# SBUF and everything downstream (f16 scores, exp scale) are unchanged.
# Score-path fp8 error averages out through softmax: rel_absmax ~1.0e-2 vs
# the 2e-2 gate (host-sim; value path in fp8 would fail at ~4e-2, so V/PV/
# out-proj stay f16). With the projections 2x cheaper, ALL FOUR q-chunks'
# QK projections fit in phase 1 (two 8-PSUM-bank waves) and the in-flash
# qk fill steps disappear — that removes ~16k ns of f16 projection work
# from the PE-bound flash region.
WSCALE = 64.0
EVSCALE = 1.0 / WSCALE

_BUILD_CACHE = {}


def _np_sb(mm_mode):
    if mm_mode == "bf16":
        import ml_dtypes

        return np.dtype(ml_dtypes.bfloat16)
    if mm_mode == "f16":
        return np.dtype(np.float16)
    return np.dtype(np.float32)


def build_nc(S, Dm, NH, Dh, mm_mode, stage=99):
    """Build (and cache) the per-core Bass module. NH = heads per core."""
    key = (S, Dm, NH, Dh, mm_mode, stage)
    if key in _BUILD_CACHE:
        return _BUILD_CACHE[key]

    import concourse.bacc as bacc
    import concourse.mybir as mybir
    import concourse.tile as tile

    f32 = mybir.dt.float32
    # dt_w: dtype of every matmul operand. float32r data is fp32 bits that the
    # PE consumes in a single-pass reduced-precision mode; the BIR verifier
    # requires every fp32r matmul operand to be *produced* with float32r dtype
    # (DMA pass-through from a float32r DRAM tensor, or a compute-engine
    # write; memset cannot produce it).
    dt_w = {
        "bf16": mybir.dt.bfloat16,
        "f16": mybir.dt.float16,
        "fp32": mybir.dt.float32,
        "fp32r": mybir.dt.float32r,
    }[mm_mode]
    # dtype for non-matmul elementwise tiles (masks)
    dt_m = {
        "bf16": mybir.dt.bfloat16,
        "f16": mybir.dt.float16,
    }.get(mm_mode, mybir.dt.float32)

    KT = Dm // P       # k-tiles over the model dim (contraction of projections)
    NPAIR = NH // 2    # head pairs
    QC = S // SC       # q chunks
    NKT = S // P       # k-position tiles
    DH2 = Dm // SC     # output free-dim chunks
    assert Dh == 64 and NH % 2 == 0 and S % SC == 0 and Dm % SC == 0

    nc = bacc.Bacc(
        "TRN2",
        debug=False,
        enable_asserts=False,
        target_bir_lowering=False,
        num_devices=1,
    )

    f8 = mybir.dt.float8e4
    DR = mybir.MatmulPerfMode.DoubleRow
    KT2 = Dm // (2 * P)  # fp8 DoubleRow k-tiles (256 contraction each)

    xT_d = nc.dram_tensor("xT", [P, KT, S], dt_w, kind="ExternalInput")
    x8_d = nc.dram_tensor("x8", [P, 2, KT2, S], f8, kind="ExternalInput")
    wqk8_d = nc.dram_tensor(
        "wqk8", [P, 2, KT2, 2, NPAIR, P], f8, kind="ExternalInput"
    )
    wv_d = nc.dram_tensor("wv", [P, KT, NH * Dh], dt_w, kind="ExternalInput")
    wo_d = nc.dram_tensor("wo", [P, NPAIR, Dm], dt_w, kind="ExternalInput")
    bqk_d = nc.dram_tensor("bqk", [P, 2, NPAIR], f32, kind="ExternalInput")
    # output in the 2-byte matmul dtype (halves the output DMA traffic; the
    # host accumulates head-group partials in f32, so only one rounding)
    dt_out = dt_w if mybir.dt.size(dt_w) == 2 else f32
    out_d = nc.dram_tensor("out", [S, Dm], dt_out, kind="ExternalOutput")

    def mm(ap):
        return ap

    Exp = mybir.ActivationFunctionType.Exp
    inv_sqrt_dh = 1.0 / float(np.sqrt(Dh))

    with tile.TileContext(nc) as tc:
        with tc.tile_pool(name="const", bufs=1) as cpool:
            # ---------- constants (DMAs emitted inside phase 1, ordered by
            # first use, so the PE starts after ~2 ktiles instead of the
            # whole 16MB input load) ----------
            wv = cpool.tile([P, KT, NH * Dh], dt_w)
            wo = cpool.tile([P, NPAIR, Dm], dt_w)
            bqk = cpool.tile([P, 2, NPAIR], f32)

            QTt = cpool.tile([P, NPAIR, S], dt_w)
            KTt = cpool.tile([P, NPAIR, S], dt_w)
            # V' = [V | 1...1]: the ones block is REPLICATED 64x so the PV
            # matmul broadcasts the softmax denominator l across output
            # partitions 64:128 (M=128 costs the same N cycles as M=65, and
            # 64-partition l lets the reciprocal run wide on DVE).
            Vt = cpool.tile([P, NKT, NH, 2 * Dh], dt_w)

            # causal masks for the 4 diagonal-chunk variants: keep (1.0) where
            # q >= k + v*128, else 0.0 (S^T layout: partition=k, free=q).
            # Built first, in a never-reused pool, so the GpSimd work (and its
            # library load) happens during the initial DMA wait.
            masks = cpool.tile([P, SC // P, SC], dt_m)
            nc.gpsimd.memset(masks[:], 1.0)
            for v in range(SC // P):
                nc.gpsimd.affine_select(
                    out=masks[:, v, :],
                    in_=masks[:, v, :],
                    compare_op=mybir.AluOpType.is_ge,
                    fill=0.0,
                    base=-(v * P),
                    pattern=[[1, SC]],
                    channel_multiplier=-1,
                )

            # ---------- phase 1: projections for the first two q-chunks
            # only; chunks 2..QC-1 are deferred into the flash loop as PE
            # fill work (so the ACT exp stream starts ~25us earlier) ----------
            with (
                tc.tile_pool(name="p1", bufs=1) as p1pool,
                tc.tile_pool(name="ps1", bufs=8, space="PSUM") as ps1,
            ):
                wqk8 = cpool.tile([P, 2, KT2, 2, NPAIR, P], f8)
                x8 = cpool.tile([P, 2, KT2, S], f8)
                xT = cpool.tile([P, KT, S], dt_w)  # outlives phase 1: the
                # deferred V-projection groups read it inside the flash loop
                # fp8 QK stream first (it arrival-paces the projection
                # waves and is only 2.6MB), then the f16 x for the V path.
                # NOTE: issuing wv/wo early on the parallel Scalar HWDGE
                # queue measured WORSE — they steal HBM bandwidth from the
                # xT stream exactly while the QK matmuls are arrival-paced.
                nc.sync.dma_start(bqk[:], bqk_d[:])
                for kt2 in range(KT2):
                    nc.sync.dma_start(wqk8[:, :, kt2], wqk8_d[:, :, kt2])
                    nc.sync.dma_start(x8[:, :, kt2, :], x8_d[:, :, kt2, :])
                for kt in range(KT):
                    nc.sync.dma_start(xT[:, kt, :], xT_d[:, kt, :])
                    if kt == KT // 2 - 1:
                        nc.sync.dma_start(wv[:], wv_d[:])
                nc.sync.dma_start(wo[:], wo_d[:])

                # HAM warm-up: dummy matmuls during the initial DMA wait so
                # the PE clock-gate is at 8/8 when real work arrives
                wst = p1pool.tile([P, SC], f32)
                nc.vector.memset(wst[:], 1.0)
                # preload the Exp table on the Scalar engine now (it's idle);
                # otherwise the first flash exp pays the ~1.3us table load
                # on the critical path
                tpre = p1pool.tile([1, 2], f32)
                nc.scalar.activation(tpre[:], wst[0:1, 0:2], Exp)
                wrm = p1pool.tile([P, SC], dt_w)
                nc.vector.tensor_copy(wrm[:], wst[:])
                nwu = 6
                pwu = ps1.tile([P, SC], f32, tag="mm")
                for i in range(nwu):
                    nc.tensor.matmul(
                        pwu[:], mm(wrm[:, 0:P]), mm(wrm[:]),
                        start=(i == 0), stop=(i == nwu - 1),
                    )

                # Q/K projections for ALL q-chunks (fp8 DoubleRow, 2x f16
                # throughput): two waves of 8 PSUM groups; each wave is fed
                # kt2-by-kt2 so the arriving x8 stream feeds 8 matmuls per
                # tile (the first pass is DMA-paced and would otherwise
                # leave the PE idle, re-throttling the clock-gate)
                for qg in range(0, QC, 2):
                    qcs = list(range(qg, min(qg + 2, QC)))
                    pss = {
                        (pr, pj, qc): ps1.tile(
                            [P, SC], f32, tag="mm", name=f"psqk_{pr}_{pj}_{qc}"
                        )
                        for pr in range(NPAIR)
                        for pj in range(2)
                        for qc in qcs
                    }
                    for kt2 in range(KT2):
                        st, sp = kt2 == 0, kt2 == KT2 - 1
                        for pr in range(NPAIR):
                            for pj in range(2):
                                for qc in qcs:
                                    xs = x8[:, :, kt2, qc * SC : (qc + 1) * SC]
                                    nc.tensor.matmul(
                                        pss[(pr, pj, qc)][:],
                                        wqk8[:, :, kt2, pj, pr, :], xs,
                                        start=st, stop=sp, perf_mode=DR,
                                    )
                    for pr in range(NPAIR):
                        for qc in qcs:
                            qs1 = slice(qc * SC, (qc + 1) * SC)
                            # evict via ACT (idle during phase 1; Identity
                            # shares the preloaded Exp table so no table
                            # reload) — keeps DVE free for the V-tile CASTs
                            # so the V groups' PSUM recycles sooner. The
                            # 1/WSCALE undoes the host-side fp8 weight scale.
                            nc.scalar.activation(
                                QTt[:, pr, qs1], pss[(pr, 0, qc)][:],
                                mybir.ActivationFunctionType.Identity,
                                bias=bqk[:, 0, pr : pr + 1], scale=EVSCALE,
                            )
                            nc.scalar.activation(
                                KTt[:, pr, qs1], pss[(pr, 1, qc)][:],
                                mybir.ActivationFunctionType.Identity,
                                bias=bqk[:, 1, pr : pr + 1], scale=EVSCALE,
                            )

                # only the V tiles the first flash chunk touches; the rest
                # are deferred into the flash loop as PE fill work (chunk 0
                # has PE slack: its exp stream is short but non-zero)
                for qt in range(min(SC // P, NKT)):
                    psV = ps1.tile([P, NH * Dh], f32, tag="mm")
                    for kt in range(KT):
                        nc.tensor.matmul(
                            psV[:],
                            mm(xT[:, kt, qt * P : (qt + 1) * P]),
                            mm(wv[:, kt, :]),
                            start=(kt == 0), stop=(kt == KT - 1),
                        )
                    nc.vector.tensor_copy(
                        Vt[:, qt, :, 0:Dh],
                        psV[:].rearrange("p (h e) -> p h e", e=Dh),
                    )

                # memset can't write float32r: stage the V' ones in f32, copy
                # over with a free-dim broadcast (needed first by the PV
                # matmuls in phase 2)
                cstage = p1pool.tile([P, 1, 1, Dh], f32)
                nc.vector.memset(cstage[:], 1.0)
                nc.vector.tensor_copy(
                    Vt[:, :, :, Dh : 2 * Dh],
                    cstage[:].to_broadcast((P, NKT, NH, Dh)),
                )

            # ---------- phases 2+3 ----------
            with tc.tile_pool(name="zt", bufs=1) as ztpool:
                ZTt = ztpool.tile([P, NPAIR, S], dt_w)
                self_flash(
                    nc, tc, stage, mm, Exp, inv_sqrt_dh, mybir,
                    QTt, KTt, Vt, ZTt, wo, out_d, masks, xT, wv, None, None,
                    S, Dm, Dh, NPAIR, QC, SC, P, DH2, KT, NKT, dt_w, dt_m, f32,
                )

    nc.compile()
    _BUILD_CACHE[key] = nc
    return nc


def self_flash(
    nc, tc, stage, mm, Exp, inv_sqrt_dh, mybir,
    QTt, KTt, Vt, ZTt, wo, out_d, masks, xT, wv, wqk, bqk,
    S, Dm, Dh, NPAIR, QC, SC, P, DH2, KT, NKT, dt_w, dt_m, f32,
):
    # ---------- phases 2+3: flash attention (scores transposed) with the
    # output projection interleaved one q-chunk behind ----------
    out_dt = dt_w if mybir.dt.size(dt_w) == 2 else f32
    with (
        tc.tile_pool(name="e", bufs=4) as epool,
        tc.tile_pool(name="r", bufs=4) as rpool,
        tc.tile_pool(name="o", bufs=4) as opool,
        tc.tile_pool(name="pss", bufs=2, space="PSUM") as ps_s,
        tc.tile_pool(name="psz", bufs=4, space="PSUM") as psz,
    ):
        if stage <= 1:
            nc.sync.dma_start(out_d[0:P, :], QTt[:, 0, 0:Dm])

        drain = [False]  # final-drain mode: outproj evictions move DVE→ACT

        def normalize(pr, qc, zA, zB):
            """ZT[:, q] = Z'[0:64, q] * (1 / l[q]); l arrives pre-broadcast
            on partitions 64:128 of the PV accumulators. DVE-only.
            (reciprocal_approx_fast must not read multi-matmul PSUM
            accumulations directly — stage l through SBUF first.)"""
            qs = slice(qc * SC, (qc + 1) * SC)
            rb = rpool.tile([64, 2, SC], f32, tag="rb")
            ls = rpool.tile([64, 2, SC], f32, tag="ls")
            nc.vector.tensor_copy(ls[:, 0, :], zA[Dh : 2 * Dh, :])
            nc.vector.tensor_copy(ls[:, 1, :], zB[Dh : 2 * Dh, :])
            nc.vector.reciprocal_approx_fast(rb[:], ls[:])
            nc.vector.tensor_mul(ZTt[0:64, pr, qs], zA[0:Dh, :], rb[:, 0, :])
            nc.vector.tensor_mul(ZTt[64:128, pr, qs], zB[0:Dh, :], rb[:, 1, :])

        def outproj_steps(qc):
            """Closures for this q-chunk's output projection, injected one at
            a time between later j-iterations to keep PE density high.
            out[q, d] = sum_h Z_h[q, :] @ W_O[h]; each K=128 matmul sums a
            head pair inside the contraction."""
            def step(t, dh2):
                def emit():
                    po = psz.tile([P, SC], f32, tag="z")
                    ds = slice(dh2 * SC, (dh2 + 1) * SC)
                    zs = slice(t * P, (t + 1) * P)
                    for pr in range(NPAIR):
                        nc.tensor.matmul(
                            po[:], mm(ZTt[:, pr, zs]), mm(wo[:, pr, ds]),
                            start=(pr == 0), stop=(pr == NPAIR - 1),
                        )
                    ot = opool.tile([P, SC], out_dt, tag="o")
                    # evict via DVE during flash (GpSimd cannot read PSUM;
                    # keeping this off the Scalar engine frees the exp stream
                    # that paces flash), but via ACT during the final drain:
                    # ACT is idle there and the DVE FIFO must stay clear for
                    # the per-tile normalize slices that gate these very
                    # steps (alternating engines here measured WORSE).
                    # Casting to the 2-byte output dtype halves the out DMA.
                    if drain[0]:
                        nc.scalar.activation(
                            ot[:], po[:], mybir.ActivationFunctionType.Copy
                        )
                    else:
                        nc.vector.tensor_copy(ot[:], po[:])
                    nc.sync.dma_start(out_d[t * P : (t + 1) * P, ds], ot[:])
                return emit

            return [
                step(t, dh2)
                for t in range(qc * (SC // P), (qc + 1) * (SC // P))
                for dh2 in range(DH2)
            ]

        def v_step(qt):
            """One deferred V-projection group: pure PE fill work for the
            flash loop. Must run before the chunk that reads Vt[qt]
            (qt tiles 4k..4k+3 are consumed first by q-chunk k)."""
            def emit():
                psV = psz.tile([P, NH * Dh], f32, tag="z", name=f"psv_{qt}")
                for kt in range(KT):
                    nc.tensor.matmul(
                        psV[:],
                        mm(xT[:, kt, qt * P : (qt + 1) * P]),
                        mm(wv[:, kt, :]),
                        start=(kt == 0), stop=(kt == KT - 1),
                    )
                nc.vector.tensor_copy(
                    Vt[:, qt, :, 0:Dh],
                    psV[:].rearrange("p (h e) -> p h e", e=Dh),
                )
            return emit

        NH = Vt.shape[2]
        # deadline-ordered fill work: (need-by-chunk, emit). All QK
        # projections now run in phase 1 (fp8 DR made them 2x cheaper), so
        # only the deferred V-projection groups ride the flash loop.
        fill_queue = []
        for qc2 in range(1, QC):
            for qt in range(qc2 * (SC // P), (qc2 + 1) * (SC // P)):
                fill_queue.append((qc2, v_step(qt)))
        op_queue = []
        chunk_tail = None
        for qc in range(QC if stage >= 2 else 0):
            # deadline safety net: any fill this chunk depends on that the
            # in-loop pops didn't get to yet runs now, up front
            while fill_queue and fill_queue[0][0] <= qc:
                fill_queue.pop(0)[1]()
            for pr in range(NPAIR):
                hA, hB = 2 * pr, 2 * pr + 1
                zA = psz.tile([P, SC], f32, tag="z")
                zB = psz.tile([P, SC], f32, tag="z")
                jmax = (qc + 1) * (SC // P)
                pends = []  # exp→PV pipeline, depth 2: the PV consuming
                # exp(j) is emitted at iteration j+2, so its semaphores are
                # long-satisfied and the PE never stalls on the ACT/GpSimd hop

                def emit_pv(j, eAB, c0, jmax=jmax, zA=zA, zB=zB, hA=hA, hB=hB):
                    st, sp = j == 0, j == jmax - 1
                    cs = slice(c0, SC)
                    nc.tensor.matmul(
                        zA[:, cs], mm(Vt[:, j, hA, :]), mm(eAB[:, 0, cs]),
                        start=st, stop=sp,
                    )
                    nc.tensor.matmul(
                        zB[:, cs], mm(Vt[:, j, hB, :]), mm(eAB[:, 1, cs]),
                        start=st, stop=sp,
                    )

                for j in range(jmax):
                    v = j - (jmax - SC // P)
                    # causal: columns below the diagonal tile are fully
                    # masked; skip them (fp32r moving dims must stay >= 256;
                    # 2-byte dtypes can slice all the way down)
                    if dt_w is mybir.dt.float32r:
                        c0 = min(v * P, 2 * P) if v > 0 else 0
                    else:
                        c0 = v * P if v > 0 else 0
                    cs = slice(c0, SC)
                    qf = slice(qc * SC + c0, (qc + 1) * SC)
                    sAB = ps_s.tile([P, 2, SC], f32, tag="s")
                    ks = slice(j * P, (j + 1) * P)
                    nc.tensor.matmul(
                        sAB[:, 0, cs],
                        mm(KTt[0:64, pr, ks]), mm(QTt[0:64, pr, qf]),
                        start=True, stop=True,
                    )
                    nc.tensor.matmul(
                        sAB[:, 1, cs],
                        mm(KTt[64:128, pr, ks]), mm(QTt[64:128, pr, qf]),
                        start=True, stop=True,
                    )
                    eAB = epool.tile([P, 2, SC], dt_w, tag="e")
                    nc.scalar.activation(
                        eAB[:, :, cs], sAB[:, :, cs], Exp, scale=inv_sqrt_dh
                    )
                    if v >= 0:  # chunk contains the causal diagonal
                        mv = slice(c0, min((v + 1) * P, SC))
                        nc.gpsimd.tensor_mul(
                            eAB[:, 0, mv], eAB[:, 0, mv], masks[:, v, mv]
                        )
                        nc.gpsimd.tensor_mul(
                            eAB[:, 1, mv], eAB[:, 1, mv], masks[:, v, mv]
                        )
                    if stage >= 3:
                        pends.append((j, eAB, c0))
                        if len(pends) > 3:
                            emit_pv(*pends.pop(0))
                        if j == 0 and chunk_tail is not None:
                            # cross-chunk pipeline: the previous chunk's last
                            # PV + normalize go here, AFTER this chunk's first
                            # scores pair is queued, so the exp stream never
                            # stalls at a chunk boundary
                            chunk_tail()
                            chunk_tail = None
                        elif j >= 1 and fill_queue:
                            fill_queue.pop(0)[1]()
                        elif j >= 2 and op_queue and (
                            qc == QC - 1 or j % 3 == 0
                        ):
                            # meter the out-proj drip in middle chunks so a
                            # backlog of real PE work survives into the last
                            # chunk, whose own PE work (scores+PV) underfills
                            # the ACT-paced loop — deliberate fill, replacing
                            # the old dummy filler matmuls
                            op_queue.pop(0)()
                    else:
                        last_e = eAB
                if stage < 3:
                    if pr == 0 and qc == 0:
                        nc.sync.dma_start(out_d[0:P, 0:SC], last_e[:, 0, :])
                    continue

                # pr-boundary cover: the next pair's first scores matmul
                # waits ~1us for ACT to drain this pair's last exps (ps_s
                # buffer release); give the PE an independent out-proj step
                # here so it works through that window instead of stalling
                if op_queue and qc >= 1:
                    op_queue.pop(0)()

                def chunk_tail(pends=pends, pr=pr, qc=qc, zA=zA, zB=zB,
                               emit_pv=emit_pv):
                    for p in pends:
                        emit_pv(*p)
                    normalize(pr, qc, zA, zB)
                    return pr, qc, zA, zB

            if stage >= 5:
                op_queue.extend(outproj_steps(qc))
        drain[0] = True
        if chunk_tail is not None:
            # final drain, pipelined: slice the last chunk's normalize per
            # q-tile and interleave that tile's out-proj steps, so the PE
            # overlaps the DVE multiplies instead of waiting for the full
            # chunk-width normalize
            pends, pr, qc, zA, zB = (chunk_tail.__defaults__[:5])
            for p in pends:
                chunk_tail.__defaults__[5](*p)
            rb = rpool.tile([64, 2, SC], f32, tag="rb")
            ls = rpool.tile([64, 2, SC], f32, tag="ls")
            for ti in range(SC // P):
                # whole DVE chain sliced per q-tile: each tile's out-proj
                # matmuls overlap the next tile's copies/reciprocal
                cl = slice(ti * P, (ti + 1) * P)
                qsl = slice(qc * SC + ti * P, qc * SC + (ti + 1) * P)
                nc.vector.tensor_copy(ls[:, 0, cl], zA[Dh : 2 * Dh, cl])
                nc.vector.tensor_copy(ls[:, 1, cl], zB[Dh : 2 * Dh, cl])
                nc.vector.reciprocal_approx_fast(rb[:, :, cl], ls[:, :, cl])
                nc.vector.tensor_mul(ZTt[0:64, pr, qsl], zA[0:Dh, cl], rb[:, 0, cl])
                nc.vector.tensor_mul(ZTt[64:128, pr, qsl], zB[0:Dh, cl], rb[:, 1, cl])
                for _ in range(DH2):
                    if op_queue:
                        op_queue.pop(0)()
        for step in op_queue:
            step()
        if stage == 4:
            nc.sync.dma_start(out_d[0:P, :], ZTt[:, 0, 0:Dm])


def pack_inputs(x_b, W_Q, W_K, W_V, W_O, b_Q, b_K, hds, mm_mode):
    """Host-side packing of one core's shard into the kernel's layouts."""
    import ml_dtypes

    f8np = ml_dtypes.float8_e4m3
    npdt = _np_sb(mm_mode)
    Dm, Dh = W_Q.shape[1], W_Q.shape[2]
    S = x_b.shape[0]
    NH = len(hds)
    NPAIR = NH // 2
    KT = Dm // P
    KT2 = Dm // (2 * P)

    xb = np.asarray(x_b, np.float32)
    xT = np.ascontiguousarray(
        xb.T.reshape(KT, P, S).transpose(1, 0, 2)
    ).astype(npdt)
    # fp8 x for the QK projections: d = kt2*256 + slot*128 + p
    x8 = np.ascontiguousarray(
        xb.T.reshape(KT2, 2, P, S).transpose(2, 1, 0, 3)
    ).astype(f8np)

    def pack_w8(W):  # [H, Dm, Dh] -> [P, 2, KT2, NPAIR, 2*Dh] fp8 @ WSCALE
        W4 = np.asarray(W, np.float32)[hds]  # [NH, Dm, Dh]
        t = W4.reshape(NPAIR, 2, KT2, 2, P, Dh).transpose(4, 3, 2, 0, 1, 5)
        return WSCALE * t.reshape(P, 2, KT2, NPAIR, 2 * Dh)

    wqk8 = np.ascontiguousarray(
        np.stack([pack_w8(W_Q), pack_w8(W_K)], axis=3)  # [P,2,KT2,2,NPAIR,128]
    ).astype(f8np)

    WV4 = np.asarray(W_V)[hds]  # [NH, Dm, Dh]
    wv = np.ascontiguousarray(
        WV4.reshape(NH, KT, P, Dh).transpose(2, 1, 0, 3).reshape(P, KT, NH * Dh)
    ).astype(npdt)

    WO4 = np.asarray(W_O)[hds]  # [NH, Dh, Dm]
    wo = np.ascontiguousarray(
        WO4.reshape(NPAIR, 2, Dh, Dm).transpose(1, 2, 0, 3).reshape(P, NPAIR, Dm)
    ).astype(npdt)

    def pack_b(b):  # [H, Dh] -> [P, NPAIR]
        b4 = np.asarray(b)[hds]
        return b4.reshape(NPAIR, 2, Dh).transpose(1, 2, 0).reshape(P, NPAIR)

    bqk = np.ascontiguousarray(
        np.stack([pack_b(b_Q), pack_b(b_K)], axis=1)  # [P, 2, NPAIR]
    ).astype(np.float32)

    return {"xT": xT, "x8": x8, "wqk8": wqk8, "wv": wv, "wo": wo, "bqk": bqk}


def kernel(x, W_Q, W_K, W_V, W_O, b_Q, b_K, b_V, b_O, _trace=False):
    from concourse.bass_utils import run_bass_kernel_spmd

    x = np.asarray(x, np.float32)
    B, S, Dm = x.shape
    H, _, Dh = W_Q.shape
    NCORES = 8
    GB = NCORES // B        # head groups per batch element
    NH = H // GB            # heads per core

    nc = build_nc(S, Dm, NH, Dh, MM_MODE)

    in_maps = []
    for c in range(NCORES):
        b, g = c // GB, c % GB
        hds = list(range(g * NH, (g + 1) * NH))
        in_maps.append(
            pack_inputs(x[b], W_Q, W_K, W_V, W_O, b_Q, b_K, hds, MM_MODE)
        )

    try:
        res = run_bass_kernel_spmd(
            nc, in_maps, core_ids=list(range(NCORES)), trace=_trace
        )
    except Exception:
        # transient device hiccups (e.g. a wedged core from a previous run)
        # usually clear on retry
        res = run_bass_kernel_spmd(
            nc, in_maps, core_ids=list(range(NCORES)), trace=_trace
        )

    out = np.zeros((B, S, Dm), np.float32)
    for c in range(NCORES):
        out[c // GB] += res.results[c]["out"]

    # biases that commute out of the device kernel (softmax rows sum to 1)
    corr = np.asarray(b_O, np.float32) + np.einsum(
        "he,hed->d",
        np.asarray(b_V, np.float32),
        np.asarray(W_O, np.float32),
    )
    out += corr[None, None, :]

    if _trace:
        kernel.last_results = res
    return out

